# revision 1
# baseline (speedup 1.0000x reference)
"""Causal self-attention (B=2, T=2048, C=1024, H=16) on 8 TRN2 NeuronCores.

Sharding: core = b*4 + hg  (data parallel over batch, tensor parallel over
4 head-groups of 4 heads). Each core computes its head-group's attention and
a partial output projection; the host sums the 4 partials per batch and adds
b_proj.

Per-core device program (v3 - phase-fused):
  - x/Wqk/Wv/q/k/v/p in fp16 (walrus rejects mixed 16/32-bit matmuls, so the
    whole attention path is 16-bit); y is float32r; PSUM accumulation fp32;
    partial outputs returned in fp16 (halves the 8MB output DMA).
  - xT streams in 512-column chunks so the fused ch-major qkv loop starts as
    soon as the first block lands; x@Wv touches only the 3 useful column
    ranges of the ones-augmented v block (the bias matmul initializes pads).
  - v_aug carries a per-head ones column so the o-matmul accumulates the
    softmax denominator D on PSUM partition {64,0,96,32}[h] while the head's
    v columns land exactly on its yT rows.
  - Attention runs per (head-pair, 512-wide query chunk): both heads share
    one [128,1024] score tile and ONE exp (ACT paces attention, so fewer,
    wider exps matter); o-matmuls trail the scores by one key block so the
    exp latency never stalls the in-order PE. Triangular masks on GPSIMD.
  - Normalization splits: PSUM->SBUF copies + 1/D recips (DVE) run at chunk
    end; the 1/D broadcast matmuls + yT scale-muls are deferred into the
    next chunk's loop. Projection column-tiles are likewise slotted into
    later ACT-paced attention chunks once their query range is normalized,
    leaving only 4 projection tiles after the last attention chunk.
"""

import math

import numpy as np

import concourse.bass as bass
import concourse.bacc as bacc
import concourse.mybir as mybir
from concourse import tile
from concourse.bass_utils import run_bass_kernel_spmd

B, T, C, H = 2, 2048, 1024, 16
HD = C // H   # 64
HPG = 4       # heads per group
NG = 4        # head groups
NCORES = 8

F32 = mybir.dt.float32
F32R = mybir.dt.float32r
BF16 = mybir.dt.bfloat16
F16 = mybir.dt.float16
AF = mybir.ActivationFunctionType
SCALE = 1.0 / math.sqrt(C)  # 1/32

# Per-head layout of the v_aug stationary block: (col offset, width,
# v-column offset within block, ones-column offset within block).
# v columns sit at PSUM rows (h%2)*64..+64; ones column on a 32-aligned row.
V_BLK = [
    (0, 65, 0, 64),      # h0: v@0-63,  D@64
    (65, 128, 64, 0),    # h1: v@64-127, D@0
    (193, 97, 0, 96),    # h2: v@0-63,  D@96
    (290, 128, 64, 32),  # h3: v@64-127, D@32
]
VW = 418  # total v_aug width
DROW = [64, 0, 96, 32]  # PSUM partition of D per head


def build_program(reps=1, qk_bias=False):
    nc = bacc.Bacc()

    xT = nc.dram_tensor("xT", [C, T], F16, kind="ExternalInput")
    wqk = nc.dram_tensor("wqk", [C, 512], F16, kind="ExternalInput")
    bqk = nc.dram_tensor("bqk", [128, 4], F32, kind="ExternalInput")
    wv = nc.dram_tensor("wv", [C, VW], F16, kind="ExternalInput")
    bv = nc.dram_tensor("bv", [1, VW], F16, kind="ExternalInput")
    wp = nc.dram_tensor("wp", [256, 1024], F32R, kind="ExternalInput")
    mask = nc.dram_tensor("mask", [128, 128], F16, kind="ExternalInput")
    ones = nc.dram_tensor("ones", [1, 128], F16, kind="ExternalInput")
    onesf = nc.dram_tensor("onesf", [128, 128], F32R, kind="ExternalInput")
    out = nc.dram_tensor("out", [T, C], F16, kind="ExternalOutput")

    with tile.TileContext(nc) as tc:
        with (
            tc.tile_pool(name="big", bufs=32) as big_pool,
            tc.tile_pool(name="pp", bufs=6) as p_pool,
            tc.tile_pool(name="osb", bufs=3) as o_pool,
            tc.tile_pool(name="wqk", bufs=8) as wqk_pool,
            tc.tile_pool(name="wv", bufs=8) as wv_pool,
            tc.tile_pool(name="qkT", bufs=4) as qkT_pool,
            tc.tile_pool(name="vsb", bufs=16) as v_pool,
            tc.tile_pool(name="yT", bufs=2) as yT_pool,
            tc.tile_pool(name="wp", bufs=2) as wp_pool,
            tc.tile_pool(name="consts", bufs=1) as c_pool,
            tc.tile_pool(name="psA", bufs=2, space="PSUM") as psA,
            tc.tile_pool(name="psB", bufs=2, space="PSUM") as psB,
            tc.tile_pool(name="psC", bufs=2, space="PSUM") as psC,
        ):
          for rep in range(reps):
            # ---- loads. xT comes in 512-col chunks, ch-major, so the qkv
            # loop can start as soon as the first column block lands; wv/wp
            # are deferred behind the ch0 prologue on the gpsimd queue. ----
            d128 = c_pool.tile([128, T], F32, tag="d128")
            r128 = c_pool.tile([128, T], F32R, tag="r128")
            wqk_sb, wv_sb = [], []
            for ct in range(8):
                w_ = wqk_pool.tile([128, 512], F16, tag="wqk", name=f"wqk{ct}")
                weng = nc.scalar if ct % 2 == 0 else nc.sync
                weng.dma_start(w_[:], wqk[ct * 128:(ct + 1) * 128, :])
                wqk_sb.append(w_)
            # gpsimd SWDGE issue costs ~1us per DMA: order by first use.
            bv_sb = c_pool.tile([1, VW], F16, tag="bv")
            nc.gpsimd.dma_start(bv_sb[:], bv[:])
            ones_sb = c_pool.tile([1, 128], F16, tag="ones")
            nc.gpsimd.dma_start(ones_sb[:], ones[:])
            for ct in range(8):
                t_ = wv_pool.tile([128, VW], F16, tag="wv", name=f"wv{ct}")
                nc.gpsimd.dma_start(t_[:], wv[ct * 128:(ct + 1) * 128, :])
                wv_sb.append(t_)
            mask_sb = c_pool.tile([128, 128], F16, tag="mask")
            nc.gpsimd.dma_start(mask_sb[:], mask[:])
            onesf_sb = c_pool.tile([128, 128], F32R, tag="onesf")
            nc.gpsimd.dma_start(onesf_sb[:], onesf[:])
            if qk_bias:
                bqk_sb = c_pool.tile([128, 4], F32, tag="bqk")
                nc.gpsimd.dma_start(bqk_sb[:], bqk[:])
            # xtc[ct][ch] covers xT[ct*128:+128, ch*512:+512]
            xtc = [[None] * 4 for _ in range(8)]
            qdma = [nc.sync, nc.scalar]
            for ch in range(4):
                for ct in range(8):
                    t_ = big_pool.tile([128, 512], F16, tag="big",
                                       name=f"xt{ct}_{ch}")
                    qdma[(ch * 8 + ct) % 2].dma_start(
                        t_[:], xT[ct * 128:(ct + 1) * 128,
                                  ch * 512:(ch + 1) * 512])
                    xtc[ct][ch] = t_
            # wp last: not needed until the first projection tile (~70us)
            wp_sb = []
            for mt in range(2):
                t_ = wp_pool.tile([128, 1024], F32R, tag="wp", name=f"wp{mt}")
                qdma[mt % 2].dma_start(t_[:], wp[mt * 128:(mt + 1) * 128, :])
                wp_sb.append(t_)

            qkT_sb = [
                qkT_pool.tile([128, T], F16, tag="qkT", name=f"qkT{j}")
                for j in range(4)
            ]
            yT_sb = [
                yT_pool.tile([128, T], F32R, tag="yT", name=f"yT{m}")
                for m in range(2)
            ]

            # x@Wv only on columns that hold real v data; the bias matmul
            # (start=True) initializes the full VW span incl. ones/pad cols.
            V_RANGES = [(0, 64), (129, 128), (354, 64)]
            v_sb = [None] * 16

            # ---- emitters ----------------------------------------------
            def emit_qk_jt(ch, jt):
                ps = psB.tile([128, 512], F32, tag="B", name="qk_ps")
                for ct in range(8):
                    nc.tensor.matmul(
                        ps[:, 0:512],
                        wqk_sb[ct][:, jt * 128:(jt + 1) * 128],
                        xtc[ct][ch][:, 0:512],
                        start=(ct == 0),
                        stop=(ct == 7),
                    )
                dst = qkT_sb[jt][:, ch * 512:(ch + 1) * 512]
                if qk_bias:
                    nc.vector.tensor_scalar_add(
                        dst, ps[:, 0:512], bqk_sb[:, jt:jt + 1]
                    )
                elif jt % 2 == 0:
                    nc.vector.tensor_copy(dst, ps[:, 0:512])
                else:
                    nc.scalar.copy(dst, ps[:, 0:512])

            def emit_v_tt(ch, tt):
                ps = psB.tile([128, 512], F32, tag="B", name="v_ps")
                nc.tensor.matmul(
                    ps[:, 0:VW],
                    ones_sb[0:1, 0:128],
                    bv_sb[0:1, 0:VW],
                    start=True,
                    stop=False,
                )
                xblk = (tt % 4) * 128
                for ct in range(8):
                    for ri, (ro, rw) in enumerate(V_RANGES):
                        nc.tensor.matmul(
                            ps[:, ro:ro + rw],
                            xtc[ct][ch][:, xblk:xblk + 128],
                            wv_sb[ct][:, ro:ro + rw],
                            start=False,
                            stop=(ct == 7 and ri == len(V_RANGES) - 1),
                            skip_group_check=True,
                        )
                t_ = v_pool.tile([128, VW], F16, tag="v", name=f"v{tt}")
                eng = nc.scalar.copy if ch == 0 else (
                    nc.vector.tensor_copy if tt % 2 == 0 else nc.scalar.copy)
                eng(t_[:], ps[:, 0:VW])
                v_sb[tt] = t_

            def emit_proj_tt(tt):
                o_sb = o_pool.tile([128, 1024], F16, tag="o", name=f"o_sb{tt}")
                for nch in range(2):
                    ps = psC.tile([128, 512], F32, tag="C", name="pj_ps")
                    for mt in range(2):
                        nc.tensor.matmul(
                            ps[:, 0:512],
                            yT_sb[mt][:, tt * 128:(tt + 1) * 128],
                            wp_sb[mt][:, nch * 512:(nch + 1) * 512],
                            start=(mt == 0),
                            stop=(mt == 1),
                        )
                    # DVE while attention runs (ACT exp-saturated); the tail
                    # tiles (tt>=12) split DVE/ACT since exps are done by then
                    if tt >= 12 and nch == 1:
                        nc.scalar.copy(
                            o_sb[:, nch * 512:(nch + 1) * 512], ps[:, 0:512]
                        )
                    else:
                        nc.vector.tensor_copy(
                            o_sb[:, nch * 512:(nch + 1) * 512], ps[:, 0:512]
                        )
                qdma[tt % 2].dma_start(out[tt * 128:(tt + 1) * 128, :], o_sb[:])

            def emit_qkv_ch(ch, slots=()):
                """One 512-column block of q/k/v projections; `slots` are
                deferred PE jobs (norm-backs, proj tiles) woven between the
                eight matmul chains to ride out their exp/DVE dependencies."""
                slots = list(slots)
                units = [lambda jt=jt: emit_qk_jt(ch, jt) for jt in (0, 2, 1, 3)]
                units += [lambda tt=tt: emit_v_tt(ch, tt)
                          for tt in range(4 * ch, 4 * ch + 4)]
                for i, u in enumerate(units):
                    u()
                    if slots:
                        slots.pop(0)()
                for s in slots:
                    s()

            # ---- attention chunk: heads of a pair share one [128,1024]
            # score tile and ONE exp; o-matmuls trail by one key block so the
            # exp latency never stalls the in-order PE stream.
            def attn_qc(hp, qc, slots=()):
                slots = list(slots)
                q_tile = qkT_sb[hp]
                k_tile = qkT_sb[2 + hp]
                q0 = qc * 512
                njt = 4 * qc + 4
                o_t = [
                    psB.tile([128, 512], F32, tag="B", name=f"o{hp}_{qc}_{hl}")
                    for hl in range(2)
                ]
                pend = None

                def flush_pend():
                    jp, p_prev = pend
                    Lp = max(0, jp * 128 - q0)
                    for hl in range(2):
                        blk_off, blk_w, _, _ = V_BLK[2 * hp + hl]
                        nc.tensor.matmul(
                            o_t[hl][0:blk_w, Lp:512],
                            v_sb[jp][:, blk_off:blk_off + blk_w],
                            p_prev[:, hl * 512 + Lp:hl * 512 + 512],
                            start=(jp == 0),
                            stop=(jp == njt - 1),
                            skip_group_check=True,
                        )

                for jt in range(njt):
                    L = max(0, jt * 128 - q0)
                    s_ps = psA.tile([128, 1024], F32, tag="A", name="s_ps")
                    for hl in range(2):
                        qrow = hl * 64
                        nc.tensor.matmul(
                            s_ps[:, hl * 512 + L:hl * 512 + 512],
                            k_tile[qrow:qrow + 64, jt * 128:(jt + 1) * 128],
                            q_tile[qrow:qrow + 64, q0 + L:q0 + 512],
                            start=True,
                            stop=True,
                        )
                    p_sb = p_pool.tile([128, 1024], F16, tag="p", name="p_sb")
                    nc.scalar.activation(
                        p_sb[:, L:1024], s_ps[:, L:1024], AF.Exp, scale=SCALE
                    )
                    if jt >= 4 * qc:  # diagonal block (both heads)
                        nc.gpsimd.tensor_mul(
                            p_sb[:, L:L + 128], p_sb[:, L:L + 128], mask_sb[:]
                        )
                        nc.gpsimd.tensor_mul(
                            p_sb[:, 512 + L:512 + L + 128],
                            p_sb[:, 512 + L:512 + L + 128],
                            mask_sb[:],
                        )
                    if pend is not None:
                        flush_pend()
                    pend = (jt, p_sb)
                    if jt % 2 == 1 and slots:
                        slots.pop(0)()
                flush_pend()
                for s in slots:
                    s()
                # PSUM->SBUF copies + 1/D (DVE); the rb broadcasts + muls are
                # deferred (returned) so this DVE chain never blocks the PE.
                qsl = slice(q0, q0 + 512)
                pa, pb = DROW[2 * hp], DROW[2 * hp + 1]
                last = hp == 1 and qc == 3  # ACT is free after the last exps
                for hl in range(2):
                    h = 2 * hp + hl
                    qrow = hl * 64
                    blk_off, blk_w, v_off, one_off = V_BLK[h]
                    nc.vector.tensor_copy(
                        yT_sb[hp][qrow:qrow + 64, qsl],
                        o_t[hl][v_off:v_off + 64, 0:512],
                    )
                    deng = nc.scalar.copy if last else nc.vector.tensor_copy
                    deng(
                        d128[DROW[h]:DROW[h] + 1, qsl],
                        o_t[hl][one_off:one_off + 1, 0:512],
                    )
                with nc.allow_low_precision(reason="1/D f32r feeds mm"):
                    nc.vector.reciprocal(r128[pa:pa + 1, qsl],
                                         d128[pa:pa + 1, qsl])
                    nc.vector.reciprocal(r128[pb:pb + 1, qsl],
                                         d128[pb:pb + 1, qsl])

                def norm_back():
                    rb = []
                    for pp_ in (pa, pb):
                        t_ = psC.tile([128, 512], F32, tag="C", name="rb")
                        nc.tensor.matmul(
                            t_[:, 0:512],
                            onesf_sb[pp_:pp_ + 1, 0:128],
                            r128[pp_:pp_ + 1, qsl],
                            start=True,
                            stop=True,
                            tile_position=(pp_, 0),
                        )
                        rb.append(t_)
                    nc.vector.tensor_mul(
                        yT_sb[hp][0:64, qsl], yT_sb[hp][0:64, qsl],
                        rb[0][0:64, 0:512],
                    )
                    nc.vector.tensor_mul(
                        yT_sb[hp][64:128, qsl], yT_sb[hp][64:128, qsl],
                        rb[1][64:128, 0:512],
                    )

                return norm_back

            # ---- schedule: attention chunk qc only needs x-columns <= qc,
            # so qkv block ch and attention chunk qc=ch-1 interleave; the
            # PE-heavy qkv chains keep the PE fed while ACT grinds exps.
            def pj(t):
                return lambda: emit_proj_tt(t)

            emit_qkv_ch(0)
            nbA0 = attn_qc(0, 0)
            nbB0 = attn_qc(1, 0, [nbA0])
            emit_qkv_ch(1)
            nbA1 = attn_qc(0, 1, [nbB0])
            nbB1 = attn_qc(1, 1, [nbA1, pj(0)])
            emit_qkv_ch(2)
            nbA2 = attn_qc(0, 2, [nbB1, pj(1), pj(2)])
            nbB2 = attn_qc(1, 2, [nbA2, pj(3), pj(4), pj(5)])
            emit_qkv_ch(3)
            nbA3 = attn_qc(0, 3, [nbB2, pj(6), pj(7), pj(8), pj(9)])
            nbB3 = attn_qc(1, 3, [nbA3, pj(10), pj(11)])
            nbB3()
            for tt in range(12, 16):
                emit_proj_tt(tt)

    if not nc.is_finalized():
        nc.finalize()
    return nc


def host_prep(x, W_attn, b_attn, W_proj):
    bf = np.float16
    x = np.ascontiguousarray(np.asarray(x, np.float32))
    W_attn = np.ascontiguousarray(np.asarray(W_attn, np.float32))
    b_attn = np.ascontiguousarray(np.asarray(b_attn, np.float32))
    W_proj = np.ascontiguousarray(np.asarray(W_proj, np.float32))
    mask = np.triu(np.ones((128, 128), bf))
    ones = np.ones((1, 128), bf)
    onesf = np.ones((128, 128), np.float32)
    per_group = []
    for hg in range(NG):
        heads = [hg * HPG + i for i in range(HPG)]
        wq = np.concatenate([W_attn[:, h * HD:(h + 1) * HD] for h in heads], axis=1)
        wk = np.concatenate(
            [W_attn[:, C + h * HD:C + (h + 1) * HD] for h in heads], axis=1
        )
        wqk_ = np.ascontiguousarray(np.concatenate([wq, wk], axis=1).astype(bf))
        bq = np.concatenate([b_attn[h * HD:(h + 1) * HD] for h in heads])
        bk = np.concatenate([b_attn[C + h * HD:C + (h + 1) * HD] for h in heads])
        bqk_ = np.ascontiguousarray(np.concatenate([bq, bk]).reshape(4, 128).T)
        wv_ = np.zeros((C, VW), np.float32)
        bv_ = np.zeros((1, VW), np.float32)
        for i, h in enumerate(heads):
            blk_off, blk_w, v_off, one_off = V_BLK[i]
            wv_[:, blk_off + v_off:blk_off + v_off + 64] = \
                W_attn[:, 2 * C + h * HD:2 * C + (h + 1) * HD]
            bv_[0, blk_off + v_off:blk_off + v_off + 64] = \
                b_attn[2 * C + h * HD:2 * C + (h + 1) * HD]
            bv_[0, blk_off + one_off] = 1.0
        wp_ = np.ascontiguousarray(
            np.concatenate([W_proj[h * HD:(h + 1) * HD, :] for h in heads], axis=0)
        )
        per_group.append((wqk_, bqk_, wv_.astype(bf), bv_.astype(bf), wp_))
    in_maps = []
    for b in range(B):
        xT_b = np.ascontiguousarray(x[b].T.astype(bf))
        for hg in range(NG):
            wqk_, bqk_, wv_, bv_, wp_ = per_group[hg]
            in_maps.append(
                dict(xT=xT_b, wqk=wqk_, bqk=bqk_, wv=wv_, bv=bv_, wp=wp_,
                     mask=mask, ones=ones, onesf=onesf)
            )
    return in_maps


_prog_cache = {}


def _get_program(qk_bias=False):
    key = ("nc", qk_bias)
    if key not in _prog_cache:
        _prog_cache[key] = build_program(qk_bias=qk_bias)
    return _prog_cache[key]


def run_cores(in_maps, trace=False, qk_bias=False, **kw):
    return run_bass_kernel_spmd(
        _get_program(qk_bias), in_maps, list(range(NCORES)), trace=trace, **kw
    )


def kernel(x, W_attn, b_attn, W_proj, b_proj):
    in_maps = host_prep(x, W_attn, b_attn, W_proj)
    qk_bias = bool(np.any(np.asarray(b_attn, np.float32)[: 2 * C]))
    br = run_cores(in_maps, qk_bias=qk_bias)
    b_proj = np.asarray(b_proj, np.float32)
    y = np.zeros((B, T, C), np.float32)
    for b in range(B):
        acc = np.zeros((T, C), np.float32)
        for hg in range(NG):
            acc += np.asarray(br.results[b * NG + hg]["out"])
        y[b] = acc + b_proj[None, :]
    return y



# revision 19
# speedup vs baseline: 1.2375x; 1.2375x over previous
"""Causal self-attention (B=2, T=2048, C=1024, H=16) on 8 TRN2 NeuronCores.

Sharding: core = b*4 + hg (data parallel over batch, tensor parallel over
4 head-groups of 4 heads). Each core computes its head-group's attention and
a partial output projection; the host sums the 4 partials per batch and adds
b_proj.

Per-core device program (v4 — fp8 DoubleRow scores + moving-v AV):
  - qk projection writes q8/k8 as fp8e4 [128, 2, T] tiles: partitions
    32h..32h+31 hold head h, slab i = head-dim half i. wqk's column order is
    permuted on the host so each [128,512] PSUM block lands with ONE copy.
  - scores use fp8 DoubleRow matmuls (0.5 cyc/row): per (head, key-block)
    one matmul, stationary k8 [32,2,128], moving q8 [32,2,512-L].
    Quantization error ~1% total, well under the 2e-2 gate.
  - AV is restructured: stationary p [128 keys, 128 q], moving v_aug
    [128, 65] (64 v cols + ones col accumulating the softmax denominator D
    per query ON the query partition). 65 moving cols per (key-block,
    q-subblock, head) beats the old 512-wide moving-p form ~2x, and D lands
    as a per-partition scalar so normalization is a DVE tensor_scalar
    (no PE broadcast matmuls).
  - y [q, hd] is normalized via reciprocal+tensor_scalar then PE-transposed
    (fp16, via identity) into yT for the projection. wp/yT are fp16.
  - ACT runs exps only; all copies go to DVE/Pool; DMA issue on SP/ACT
    (prologue) and gpsimd SWDGE for weights/consts.
  - PSUM: scores 2x[128,1024] (4 banks) + two AV group tiles [128,260]
    (qs01/qs23 x hl, 65-wide blocks, pending-zero init, 1 bank each) +
    shared [128,512] pool for qkv/proj/transpose (2 banks) = 8 banks.
  - Schedule: attention chunk (hp, qc) interleaves with qkv column-block
    ch=qc; qkv units and projection tiles are woven into the ACT-paced
    jt loops as slots to keep the PE fed.
"""

import math

import numpy as np

import concourse.bass as bass
import concourse.bacc as bacc
import concourse.mybir as mybir
from concourse import tile
from concourse.bass_utils import run_bass_kernel_spmd

B, T, C, H = 2, 2048, 1024, 16
HD = C // H   # 64
HPG = 4       # heads per group
NG = 4        # head groups
NCORES = 8
VW = 260      # v_aug width: 4 heads x (64 v + 1 ones)

F32 = mybir.dt.float32
F16 = mybir.dt.float16
F8 = mybir.dt.float8e4
AF = mybir.ActivationFunctionType
DR = mybir.MatmulPerfMode.DoubleRow
SCALE = 1.0 / math.sqrt(C)  # 1/32


def build_program(reps=1, qk_bias=False):
    nc = bacc.Bacc()

    xT = nc.dram_tensor("xT", [8, 128, T], F16, kind="ExternalInput")
    wqk = nc.dram_tensor("wqk", [128, 8, 512], F16, kind="ExternalInput")
    bqk = nc.dram_tensor("bqk", [128, 4], F32, kind="ExternalInput")
    wv = nc.dram_tensor("wv", [C, VW], F16, kind="ExternalInput")
    bv = nc.dram_tensor("bv", [1, VW], F16, kind="ExternalInput")
    wp = nc.dram_tensor("wp", [128, 2, 1024], F16, kind="ExternalInput")
    mask = nc.dram_tensor("mask", [128, 128], F16, kind="ExternalInput")
    ones = nc.dram_tensor("ones", [1, 128], F16, kind="ExternalInput")
    id16 = nc.dram_tensor("id16", [128, 128], F16, kind="ExternalInput")
    out = nc.dram_tensor("out", [T, C], F16, kind="ExternalOutput")

    with tile.TileContext(nc) as tc:
        with (
            tc.tile_pool(name="big", bufs=8) as big_pool,
            tc.tile_pool(name="wqk", bufs=1) as wqk_pool,
            tc.tile_pool(name="wv", bufs=8) as wv_pool,
            tc.tile_pool(name="wp", bufs=1) as wp_pool,
            tc.tile_pool(name="qk8", bufs=2) as qk8_pool,
            tc.tile_pool(name="pp", bufs=4) as p_pool,
            tc.tile_pool(name="vsb", bufs=16) as v_pool,
            tc.tile_pool(name="yT", bufs=2) as yT_pool,
            tc.tile_pool(name="yn", bufs=3) as yn_pool,
            tc.tile_pool(name="rr", bufs=3) as r_pool,
            tc.tile_pool(name="osb", bufs=3) as o_pool,
            tc.tile_pool(name="consts", bufs=1) as c_pool,
            tc.tile_pool(name="psA", bufs=2, space="PSUM") as psA,
            tc.tile_pool(name="psVa", bufs=1, space="PSUM") as psVa,
            tc.tile_pool(name="psVb", bufs=1, space="PSUM") as psVb,
            tc.tile_pool(name="psB", bufs=2, space="PSUM") as psB,
        ):
          for rep in range(reps):
            # ---- loads. HWDGE issue is a serialized ~630ns/DMA device, so
            # inputs use few big DMAs: wqk in 2 halves, x ch0 per-ct (fine
            # grain feeds the first chains), x ch1-3 as one [128,1536] DMA
            # per ct, wp as one DMA. consts/wv go via gpsimd SWDGE (bypasses
            # HWDGE entirely).
            qdma = [nc.sync, nc.scalar]
            wqk_sb = wqk_pool.tile([128, 8, 512], F16, tag="wqk")
            nc.sync.dma_start(wqk_sb[:, 0:4, :], wqk[:, 0:4, :])
            nc.scalar.dma_start(wqk_sb[:, 4:8, :], wqk[:, 4:8, :])
            mask_sb = c_pool.tile([128, 128], F16, tag="mask")
            nc.gpsimd.dma_start(mask_sb[:], mask[:])
            id_sb = c_pool.tile([128, 128], F16, tag="id16")
            nc.gpsimd.dma_start(id_sb[:], id16[:])
            ones_sb = c_pool.tile([1, 128], F16, tag="ones")
            nc.gpsimd.dma_start(ones_sb[:], ones[:])
            bv_sb = c_pool.tile([1, VW], F16, tag="bv")
            nc.gpsimd.dma_start(bv_sb[:], bv[:])
            if qk_bias:
                bqk_sb = c_pool.tile([128, 4], F32, tag="bqk")
                nc.gpsimd.dma_start(bqk_sb[:], bqk[:])
            wv_sb = []
            for ct in range(8):
                t_ = wv_pool.tile([128, VW], F16, tag="wv", name=f"wv{ct}")
                nc.gpsimd.dma_start(t_[:], wv[ct * 128:(ct + 1) * 128, :])
                wv_sb.append(t_)
            xtc0, xtcR = [], []
            for ct in range(8):
                t_ = big_pool.tile([128, 512], F16, tag="big0",
                                   name=f"x0_{ct}")
                qdma[ct % 2].dma_start(t_[:], xT[ct, :, 0:512])
                xtc0.append(t_)
            for ct in range(8):
                t_ = big_pool.tile([128, 1536], F16, tag="bigR",
                                   name=f"xR_{ct}")
                qdma[ct % 2].dma_start(t_[:], xT[ct, :, 512:2048])
                xtcR.append(t_)
            wp_sb = wp_pool.tile([128, 2, 1024], F16, tag="wp")
            nc.sync.dma_start(wp_sb[:], wp[:])

            def xs(ct, ch, c0, c1):
                if ch == 0:
                    return xtc0[ct][:, c0:c1]
                base = (ch - 1) * 512
                return xtcR[ct][:, base + c0:base + c1]

            q8 = qk8_pool.tile([128, 2, T], F8, tag="qk8", name="q8")
            k8 = qk8_pool.tile([128, 2, T], F8, tag="qk8", name="k8")
            yT_sb = [
                yT_pool.tile([128, T], F16, tag="yT", name=f"yT{m}")
                for m in range(2)
            ]
            v_sb = [None] * 16

            # ---- emitters ----------------------------------------------
            def emit_qk_jt(ch, jt):
                """qk projection block: psum rows = 4 heads x 32 hd-half."""
                ps = psB.tile([128, 512], F32, tag="mm", name=f"qk{ch}_{jt}")
                for ct in range(8):
                    nc.tensor.matmul(
                        ps[:, 0:512],
                        wqk_sb[:, ct:ct + 1, jt * 128:(jt + 1) * 128],
                        xs(ct, ch, 0, 512),
                        start=(ct == 0),
                        stop=(ct == 7),
                    )
                dst_t = q8 if jt < 2 else k8
                dst = dst_t[:, jt % 2:jt % 2 + 1, ch * 512:(ch + 1) * 512]
                if qk_bias:
                    nc.vector.tensor_scalar_add(dst, ps[:, 0:512],
                                                bqk_sb[:, jt:jt + 1])
                elif ch == 0 and jt % 2 == 1:
                    nc.scalar.copy(dst, ps[:, 0:512])
                else:
                    nc.vector.tensor_copy(dst, ps[:, 0:512])

            def emit_v_tt(ch, tt):
                ps = psB.tile([128, 512], F32, tag="mm", name=f"v{tt}")
                nc.tensor.matmul(
                    ps[:, 0:VW],
                    ones_sb[0:1, 0:128],
                    bv_sb[0:1, 0:VW],
                    start=True,
                    stop=False,
                )
                xblk = (tt % 4) * 128
                for ct in range(8):
                    nc.tensor.matmul(
                        ps[:, 0:VW],
                        xs(ct, ch, xblk, xblk + 128),
                        wv_sb[ct][:, 0:VW],
                        start=False,
                        stop=(ct == 7),
                        skip_group_check=True,
                    )
                t_ = v_pool.tile([128, VW], F16, tag="v", name=f"v{tt}")
                # tt==0 runs in the prologue where ACT is still idle
                if tt == 0:
                    nc.scalar.copy(t_[:], ps[:, 0:VW])
                else:
                    nc.vector.tensor_copy(t_[:], ps[:, 0:VW])
                v_sb[tt] = t_

            def emit_proj_nch(tt, nch):
                ps = psB.tile([128, 512], F32, tag="mm", name=f"pj{tt}_{nch}")
                for mt in range(2):
                    nc.tensor.matmul(
                        ps[:, 0:512],
                        yT_sb[mt][:, tt * 128:(tt + 1) * 128],
                        wp_sb[:, mt:mt + 1, nch * 512:(nch + 1) * 512],
                        start=(mt == 0),
                        stop=(mt == 1),
                    )
                o = o_pool.tile([128, 512], F16, tag="o", name=f"o{tt}_{nch}")
                nc.vector.tensor_copy(o[:], ps[:, 0:512])
                nc.sync.dma_start(
                    out[tt * 128:(tt + 1) * 128, nch * 512:(nch + 1) * 512],
                    o[:],
                )

            def pj(tt):
                return [lambda: emit_proj_nch(tt, 0),
                        lambda: emit_proj_nch(tt, 1)]

            # ---- attention chunk ---------------------------------------
            def attn_qc(hp, qc, slots=(), post=()):
                """post[qs]: thunks emitted right after finish_qs(qs) —
                for work that depends on this chunk's own yT writes."""
                slots = list(slots)
                post = {i: list(p) for i, p in enumerate(post)}
                q0 = qc * 512
                njt = 4 * qc + 4
                av = [
                    psVa.tile([128, VW], F32, tag="avA", name=f"av{hp}{qc}a"),
                    psVb.tile([128, VW], F32, tag="avB", name=f"av{hp}{qc}b"),
                ]
                started = [False, False]
                pend = None

                def emit_av(jp, pp):
                    for qs in range(max(0, jp - 4 * qc), 4):
                        ti = qs // 2
                        st = not started[ti]
                        started[ti] = True
                        for hl in range(2):
                            g = 2 * (qs % 2) + hl
                            nc.tensor.matmul(
                                av[ti][:, 65 * g:65 * g + 65],
                                pp[:, hl * 512 + qs * 128:
                                   hl * 512 + qs * 128 + 128],
                                v_sb[jp][:, 65 * (2 * hp + hl):
                                         65 * (2 * hp + hl) + 65],
                                start=(st and hl == 0),
                                stop=(jp == 4 * qc + qs),
                                skip_group_check=True,
                            )

                def finish_qs(qs):
                    ti, g0 = qs // 2, 2 * (qs % 2)
                    rs = r_pool.tile([128, 2], F32, tag="r",
                                     name=f"r{hp}_{qc}_{qs}")
                    yn = yn_pool.tile([128, 128], F16, tag="yn",
                                      name=f"yn{hp}_{qc}_{qs}")
                    for hl in range(2):
                        c0 = 65 * (g0 + hl)
                        nc.vector.reciprocal(rs[:, hl:hl + 1],
                                             av[ti][:, c0 + 64:c0 + 65])
                        nc.vector.tensor_scalar_mul(
                            yn[:, 64 * hl:64 * hl + 64],
                            av[ti][:, c0:c0 + 64],
                            rs[:, hl:hl + 1],
                        )
                    tp = psB.tile([128, 128], F16, tag="mm",
                                  name=f"tp{hp}_{qc}_{qs}")
                    nc.tensor.transpose(tp[:, 0:128], yn[:, 0:128],
                                        id_sb[:, 0:128])
                    nc.vector.tensor_copy(
                        yT_sb[hp][:, q0 + qs * 128:q0 + qs * 128 + 128],
                        tp[:, 0:128],
                    )

                for jt in range(njt):
                    L = max(0, jt * 128 - q0)
                    s_ps = psA.tile([128, 1024], F32, tag="A", name="s_ps")
                    for hl in range(2):
                        h = 2 * hp + hl
                        nc.tensor.matmul(
                            s_ps[:, hl * 512 + L:(hl + 1) * 512],
                            k8[32 * h:32 * h + 32, :,
                               jt * 128:(jt + 1) * 128],
                            q8[32 * h:32 * h + 32, :, q0 + L:q0 + 512],
                            start=True,
                            stop=True,
                            perf_mode=DR,
                            tile_position=(32 * h, 0),
                        )
                    p_sb = p_pool.tile([128, 1024], F16, tag="p", name="p_sb")
                    nc.scalar.activation(
                        p_sb[:, L:1024], s_ps[:, L:1024], AF.Exp, scale=SCALE
                    )
                    if jt >= 4 * qc:
                        nc.gpsimd.tensor_mul(
                            p_sb[:, L:L + 128], p_sb[:, L:L + 128], mask_sb[:]
                        )
                        nc.gpsimd.tensor_mul(
                            p_sb[:, 512 + L:512 + L + 128],
                            p_sb[:, 512 + L:512 + L + 128],
                            mask_sb[:],
                        )
                    if pend is not None:
                        jp, pp = pend
                        emit_av(jp, pp)
                        if jp >= 4 * qc:
                            qs_done = jp - 4 * qc
                            finish_qs(qs_done)
                            for s in post.pop(qs_done, ()):
                                s()
                    pend = (jt, p_sb)
                    if slots:
                        slots.pop(0)()
                jp, pp = pend
                emit_av(jp, pp)
                finish_qs(jp - 4 * qc)
                for s in slots:
                    s()
                for qs_done in sorted(post):
                    for s in post[qs_done]:
                        s()

            # ---- schedule ----------------------------------------------
            def qk_u(ch, jt):
                return lambda: emit_qk_jt(ch, jt)

            def v_u(ch, tt):
                return lambda: emit_v_tt(ch, tt)

            # prologue: qk blocks of ch0 + v0 (A0's first AV needs it)
            for jt in range(4):
                emit_qk_jt(0, jt)
            emit_v_tt(0, 0)
            attn_qc(0, 0, [v_u(0, 1), v_u(0, 2), v_u(0, 3)])
            # B0 carries the ch1 qk blocks (paced by the x ch1-3 DMAs)
            attn_qc(1, 0, [qk_u(1, 0), qk_u(1, 1), qk_u(1, 2), qk_u(1, 3)])
            attn_qc(0, 1, [v_u(1, 4), v_u(1, 5), v_u(1, 6), v_u(1, 7)])
            attn_qc(1, 1, [qk_u(2, 0), qk_u(2, 1), qk_u(2, 2), qk_u(2, 3)])
            attn_qc(0, 2, [v_u(2, 8), v_u(2, 9), v_u(2, 10), v_u(2, 11)]
                    + pj(0) + pj(1))
            attn_qc(1, 2, [qk_u(3, 0), qk_u(3, 1), qk_u(3, 2), qk_u(3, 3)]
                    + pj(2) + pj(3))
            attn_qc(0, 3, [v_u(3, 12), v_u(3, 13), v_u(3, 14), v_u(3, 15)]
                    + pj(4) + pj(5) + pj(6) + pj(7))
            # pj(12..15) read yT columns B3 itself writes: emit each right
            # after B3's finish_qs for that query sub-block.
            attn_qc(1, 3, pj(8) + pj(9) + pj(10) + pj(11),
                    post=[pj(12), pj(13), pj(14), pj(15)])

    if not nc.is_finalized():
        nc.finalize()
    return nc


def host_prep(x, W_attn, b_attn, W_proj):
    bf = np.float16
    x = np.ascontiguousarray(np.asarray(x, np.float32))
    W_attn = np.ascontiguousarray(np.asarray(W_attn, np.float32))
    b_attn = np.ascontiguousarray(np.asarray(b_attn, np.float32))
    W_proj = np.ascontiguousarray(np.asarray(W_proj, np.float32))
    mask = np.triu(np.ones((128, 128), np.float32)).astype(bf)
    ones = np.ones((1, 128), bf)
    id16 = np.eye(128, dtype=bf)
    per_group = []
    for hg in range(NG):
        heads = [hg * HPG + i for i in range(HPG)]
        # wqk column blocks jt: 0=q hd-lo, 1=q hd-hi, 2=k hd-lo, 3=k hd-hi;
        # within a block, 4 heads x 32 (head-major)
        cols, bias = [], []
        for base in (0, C):  # q then k
            for half in (0, 32):
                for h in heads:
                    c0 = base + h * HD + half
                    cols.append(W_attn[:, c0:c0 + 32])
                    bias.append(b_attn[c0:c0 + 32])
        # [C, 512] -> [128, 8, 512]: partition p, ct-slab, col
        wqk_ = np.ascontiguousarray(
            np.concatenate(cols, axis=1).astype(bf)
            .reshape(8, 128, 512).transpose(1, 0, 2))
        bqk_ = np.ascontiguousarray(
            np.concatenate(bias).reshape(4, 128).T.astype(np.float32))
        wv_ = np.zeros((C, VW), np.float32)
        bv_ = np.zeros((1, VW), np.float32)
        for i, h in enumerate(heads):
            wv_[:, 65 * i:65 * i + 64] = \
                W_attn[:, 2 * C + h * HD:2 * C + (h + 1) * HD]
            bv_[0, 65 * i:65 * i + 64] = \
                b_attn[2 * C + h * HD:2 * C + (h + 1) * HD]
            bv_[0, 65 * i + 64] = 1.0
        # [256, 1024] -> [128, 2, 1024]
        wp_ = np.ascontiguousarray(
            np.concatenate([W_proj[h * HD:(h + 1) * HD, :] for h in heads],
                           axis=0).astype(bf)
            .reshape(2, 128, 1024).transpose(1, 0, 2))
        per_group.append((wqk_, bqk_, wv_.astype(bf), bv_.astype(bf), wp_))
    in_maps = []
    for b in range(B):
        xT_b = np.ascontiguousarray(
            x[b].T.astype(bf).reshape(8, 128, T))
        for hg in range(NG):
            wqk_, bqk_, wv_, bv_, wp_ = per_group[hg]
            in_maps.append(
                dict(xT=xT_b, wqk=wqk_, bqk=bqk_, wv=wv_, bv=bv_, wp=wp_,
                     mask=mask, ones=ones, id16=id16)
            )
    return in_maps


_prog_cache = {}


def _get_program(qk_bias=False):
    key = ("nc", qk_bias)
    if key not in _prog_cache:
        _prog_cache[key] = build_program(qk_bias=qk_bias)
    return _prog_cache[key]


def run_cores(in_maps, trace=False, qk_bias=False, **kw):
    return run_bass_kernel_spmd(
        _get_program(qk_bias), in_maps, list(range(NCORES)), trace=trace, **kw
    )


def kernel(x, W_attn, b_attn, W_proj, b_proj):
    in_maps = host_prep(x, W_attn, b_attn, W_proj)
    qk_bias = bool(np.any(np.asarray(b_attn, np.float32)[: 2 * C]))
    br = run_cores(in_maps, qk_bias=qk_bias)
    b_proj = np.asarray(b_proj, np.float32)
    y = np.zeros((B, T, C), np.float32)
    for b in range(B):
        acc = np.zeros((T, C), np.float32)
        for hg in range(NG):
            acc += np.asarray(br.results[b * NG + hg]["out"])
        y[b] = acc + b_proj[None, :]
    return y


# revision 35
# speedup vs baseline: 1.3765x; 1.1124x over previous
"""Causal self-attention (B=2, T=2048, C=1024, H=16) on 8 TRN2 NeuronCores.

Sharding: core = b*4 + hg (data parallel over batch, tensor parallel over
4 head-groups of 4 heads). Each core computes its head-group's attention and
a partial output projection; the host sums the 4 partials per batch and adds
b_proj.

Per-core device program (v4 — fp8 DoubleRow scores + moving-v AV):
  - qk projection writes q8/k8 as fp8e4 [128, 2, T] tiles: partitions
    32h..32h+31 hold head h, slab i = head-dim half i. wqk's column order is
    permuted on the host so each [128,512] PSUM block lands with ONE copy.
  - scores use fp8 DoubleRow matmuls (0.5 cyc/row): per (head, key-block)
    one matmul, stationary k8 [32,2,128], moving q8 [32,2,512-L].
    Quantization error ~1% total, well under the 2e-2 gate.
  - AV is restructured: stationary p [128 keys, 128 q], moving v_aug
    [128, 65] (64 v cols + ones col accumulating the softmax denominator D
    per query ON the query partition). 65 moving cols per (key-block,
    q-subblock, head) beats the old 512-wide moving-p form ~2x, and D lands
    as a per-partition scalar so normalization is a DVE tensor_scalar
    (no PE broadcast matmuls).
  - y [q, hd] is normalized via reciprocal+tensor_scalar then PE-transposed
    (fp16, via identity) into yT for the projection. wp/yT are fp16.
  - ACT runs exps only; all copies go to DVE/Pool; DMA issue on SP/ACT
    (prologue) and gpsimd SWDGE for weights/consts.
  - PSUM: scores 2x[128,1024] (4 banks) + two AV group tiles [128,260]
    (qs01/qs23 x hl, 65-wide blocks, pending-zero init, 1 bank each) +
    shared [128,512] pool for qkv/proj/transpose (2 banks) = 8 banks.
  - Schedule: attention chunk (hp, qc) interleaves with qkv column-block
    ch=qc; qkv units and projection tiles are woven into the ACT-paced
    jt loops as slots to keep the PE fed.
"""

import math

import numpy as np

import concourse.bass as bass
import concourse.bacc as bacc
import concourse.mybir as mybir
from concourse import tile
from concourse.bass_utils import run_bass_kernel_spmd

B, T, C, H = 2, 2048, 1024, 16
HD = C // H   # 64
HPG = 4       # heads per group
NG = 4        # head groups
NCORES = 8
VW = 260      # v_aug width: 4 heads x (64 v + 1 ones)

F32 = mybir.dt.float32
F16 = mybir.dt.float16
F8 = mybir.dt.float8e4
AF = mybir.ActivationFunctionType
DR = mybir.MatmulPerfMode.DoubleRow
SCALE = 1.0 / math.sqrt(C)  # 1/32


def build_program(reps=1, qk_bias=False):
    nc = bacc.Bacc()

    xT = nc.dram_tensor("xT", [8, 128, T], F16, kind="ExternalInput")
    wqk = nc.dram_tensor("wqk", [128, 8, 512], F16, kind="ExternalInput")
    bqk = nc.dram_tensor("bqk", [128, 4], F32, kind="ExternalInput")
    wv = nc.dram_tensor("wv", [C, VW], F16, kind="ExternalInput")
    bv = nc.dram_tensor("bv", [1, VW], F16, kind="ExternalInput")
    wp = nc.dram_tensor("wp", [128, 2, 1024], F16, kind="ExternalInput")
    mask = nc.dram_tensor("mask", [128, 128], F16, kind="ExternalInput")
    ones = nc.dram_tensor("ones", [1, 128], F16, kind="ExternalInput")
    id16 = nc.dram_tensor("id16", [128, 128], F16, kind="ExternalInput")
    out = nc.dram_tensor("out", [T, C], F16, kind="ExternalOutput")

    with tile.TileContext(nc) as tc:
        with (
            tc.tile_pool(name="big", bufs=8) as big_pool,
            tc.tile_pool(name="wqk", bufs=1) as wqk_pool,
            tc.tile_pool(name="wv", bufs=8) as wv_pool,
            tc.tile_pool(name="wp", bufs=1) as wp_pool,
            tc.tile_pool(name="qk8", bufs=2) as qk8_pool,
            tc.tile_pool(name="pp", bufs=6) as p_pool,
            tc.tile_pool(name="vsb", bufs=16) as v_pool,
            tc.tile_pool(name="yT", bufs=2) as yT_pool,
            tc.tile_pool(name="yn", bufs=3) as yn_pool,
            tc.tile_pool(name="rr", bufs=3) as r_pool,
            tc.tile_pool(name="osb", bufs=3) as o_pool,
            tc.tile_pool(name="consts", bufs=1) as c_pool,
            tc.tile_pool(name="psA", bufs=2, space="PSUM") as psA,
            tc.tile_pool(name="psVa", bufs=1, space="PSUM") as psVa,
            tc.tile_pool(name="psVb", bufs=1, space="PSUM") as psVb,
            tc.tile_pool(name="psB", bufs=2, space="PSUM") as psB,
        ):
          for rep in range(reps):
            # ---- loads. HWDGE issue is a serialized ~630ns/DMA device, so
            # inputs use few big DMAs: wqk in 2 halves, x ch0 per-ct (fine
            # grain feeds the first chains), x ch1-3 as one [128,1536] DMA
            # per ct, wp as one DMA. consts/wv go via gpsimd SWDGE (bypasses
            # HWDGE entirely).
            qdma = [nc.sync, nc.scalar]
            wqk_sb = wqk_pool.tile([128, 8, 512], F16, tag="wqk")
            for qtr in range(4):
                qdma[qtr % 2].dma_start(
                    wqk_sb[:, 2 * qtr:2 * qtr + 2, :],
                    wqk[:, 2 * qtr:2 * qtr + 2, :])
            mask_sb = c_pool.tile([128, 128], F16, tag="mask")
            nc.gpsimd.dma_start(mask_sb[:], mask[:])
            id_sb = c_pool.tile([128, 128], F16, tag="id16")
            nc.gpsimd.dma_start(id_sb[:], id16[:])
            ones_sb = c_pool.tile([1, 128], F16, tag="ones")
            nc.gpsimd.dma_start(ones_sb[:], ones[:])
            bv_sb = c_pool.tile([1, VW], F16, tag="bv")
            nc.gpsimd.dma_start(bv_sb[:], bv[:])
            if qk_bias:
                bqk_sb = c_pool.tile([128, 4], F32, tag="bqk")
                nc.gpsimd.dma_start(bqk_sb[:], bqk[:])
            wv_sb = []
            for ct in range(8):
                t_ = wv_pool.tile([128, VW], F16, tag="wv", name=f"wv{ct}")
                nc.gpsimd.dma_start(t_[:], wv[ct * 128:(ct + 1) * 128, :])
                wv_sb.append(t_)
            xtc0, xtcR = [], []
            for ct in range(8):
                t_ = big_pool.tile([128, 512], F16, tag="big0",
                                   name=f"x0_{ct}")
                qdma[ct % 2].dma_start(t_[:], xT[ct, :, 0:512])
                xtc0.append(t_)
            for ct in range(8):
                t_ = big_pool.tile([128, 1536], F16, tag="bigR",
                                   name=f"xR_{ct}")
                qdma[ct % 2].dma_start(t_[:], xT[ct, :, 512:2048])
                xtcR.append(t_)
            wp_sb = wp_pool.tile([128, 2, 1024], F16, tag="wp")
            nc.sync.dma_start(wp_sb[:], wp[:])

            def xs(ct, ch, c0, c1):
                if ch == 0:
                    return xtc0[ct][:, c0:c1]
                base = (ch - 1) * 512
                return xtcR[ct][:, base + c0:base + c1]

            q8 = qk8_pool.tile([128, 2, T], F8, tag="qk8", name="q8")
            k8 = qk8_pool.tile([128, 2, T], F8, tag="qk8", name="k8")
            yT_sb = [
                yT_pool.tile([128, T], F16, tag="yT", name=f"yT{m}")
                for m in range(2)
            ]
            v_sb = [None] * 16

            # ---- emitters ----------------------------------------------
            def qk_parts(ch, jt):
                """qk projection block as two half-chains (finer slots).
                psum rows = 4 heads x 32 hd-half."""
                cell = []

                def half(h0):
                    if h0 == 0:
                        cell.append(psB.tile([128, 512], F32, tag="mm",
                                             name=f"qk{ch}_{jt}"))
                    ps = cell[0]
                    for ct in range(h0, h0 + 4):
                        nc.tensor.matmul(
                            ps[:, 0:512],
                            wqk_sb[:, ct:ct + 1, jt * 128:(jt + 1) * 128],
                            xs(ct, ch, 0, 512),
                            start=(ct == 0),
                            stop=(ct == 7),
                        )
                    if h0 == 0:
                        return
                    dst_t = q8 if jt < 2 else k8
                    dst = dst_t[:, jt % 2:jt % 2 + 1, ch * 512:(ch + 1) * 512]
                    if qk_bias:
                        nc.vector.tensor_scalar_add(dst, ps[:, 0:512],
                                                    bqk_sb[:, jt:jt + 1])
                    elif ch == 0 and jt % 2 == 1:
                        nc.scalar.copy(dst, ps[:, 0:512])
                    else:
                        nc.vector.tensor_copy(dst, ps[:, 0:512])

                return [lambda: half(0), lambda: half(4)]

            def emit_qk_jt(ch, jt):
                for p in qk_parts(ch, jt):
                    p()

            def v_parts(ch, tt):
                """v_aug projection block as two half-chains."""
                cell = []
                xblk = (tt % 4) * 128

                def half(h0):
                    if h0 == 0:
                        cell.append(psB.tile([128, 512], F32, tag="mm",
                                             name=f"v{tt}"))
                        nc.tensor.matmul(
                            cell[0][:, 0:VW],
                            ones_sb[0:1, 0:128],
                            bv_sb[0:1, 0:VW],
                            start=True,
                            stop=False,
                        )
                    ps = cell[0]
                    for ct in range(h0, h0 + 4):
                        nc.tensor.matmul(
                            ps[:, 0:VW],
                            xs(ct, ch, xblk, xblk + 128),
                            wv_sb[ct][:, 0:VW],
                            start=False,
                            stop=(ct == 7),
                            skip_group_check=True,
                        )
                    if h0 == 0:
                        return
                    t_ = v_pool.tile([128, VW], F16, tag="v", name=f"v{tt}")
                    # tt==0 runs in the prologue where ACT is still idle
                    if tt == 0:
                        nc.scalar.copy(t_[:], ps[:, 0:VW])
                    else:
                        nc.vector.tensor_copy(t_[:], ps[:, 0:VW])
                    v_sb[tt] = t_

                return [lambda: half(0), lambda: half(4)]

            def emit_v_tt(ch, tt):
                for p in v_parts(ch, tt):
                    p()

            def emit_proj_nch(tt, nch):
                ps = psB.tile([128, 512], F32, tag="mm", name=f"pj{tt}_{nch}")
                for mt in range(2):
                    nc.tensor.matmul(
                        ps[:, 0:512],
                        yT_sb[mt][:, tt * 128:(tt + 1) * 128],
                        wp_sb[:, mt:mt + 1, nch * 512:(nch + 1) * 512],
                        start=(mt == 0),
                        stop=(mt == 1),
                    )
                o = o_pool.tile([128, 512], F16, tag="o", name=f"o{tt}_{nch}")
                # tt>=14 is emitted after the last exp: ACT is idle there and
                # this skips the DVE tail backlog
                if tt >= 14:
                    nc.scalar.copy(o[:], ps[:, 0:512])
                else:
                    nc.vector.tensor_copy(o[:], ps[:, 0:512])
                nc.sync.dma_start(
                    out[tt * 128:(tt + 1) * 128, nch * 512:(nch + 1) * 512],
                    o[:],
                )

            def pj(tt):
                return [lambda: emit_proj_nch(tt, 0),
                        lambda: emit_proj_nch(tt, 1)]

            # ---- attention chunk ---------------------------------------
            def attn_qc(hp, qc, slots=(), post=()):
                """post[qs]: thunks emitted right after finish_qs(qs) —
                for work that depends on this chunk's own yT writes."""
                slots = list(slots)
                post = {i: list(p) for i, p in enumerate(post)}
                q0 = qc * 512
                njt = 4 * qc + 4
                av = [
                    psVa.tile([128, VW], F32, tag="avA", name=f"av{hp}{qc}a"),
                    psVb.tile([128, VW], F32, tag="avB", name=f"av{hp}{qc}b"),
                ]
                started = [False, False]
                pend, trail = [], 2
                n_slots, n_popped = len(slots), 0

                def emit_av(jp, pp):
                    for qs in range(max(0, jp - 4 * qc), 4):
                        ti = qs // 2
                        st = not started[ti]
                        started[ti] = True
                        for hl in range(2):
                            g = 2 * (qs % 2) + hl
                            nc.tensor.matmul(
                                av[ti][:, 65 * g:65 * g + 65],
                                pp[:, hl * 512 + qs * 128:
                                   hl * 512 + qs * 128 + 128],
                                v_sb[jp][:, 65 * (2 * hp + hl):
                                         65 * (2 * hp + hl) + 65],
                                start=(st and hl == 0),
                                stop=(jp == 4 * qc + qs),
                                skip_group_check=True,
                            )

                def finish_qs(qs):
                    """Normalize (recip + scale) now; return a thunk doing
                    the PE transpose + yT copy, deferred so the PE never
                    waits on this DVE chain."""
                    ti, g0 = qs // 2, 2 * (qs % 2)
                    rs = r_pool.tile([128, 2], F32, tag="r",
                                     name=f"r{hp}_{qc}_{qs}")
                    yn = yn_pool.tile([128, 128], F16, tag="yn",
                                      name=f"yn{hp}_{qc}_{qs}")
                    # the very last finishes run after the final exp: use the
                    # idle ACT engine instead of the backlogged DVE
                    on_act = hp == 1 and qc == 3 and qs >= 2
                    for hl in range(2):
                        c0 = 65 * (g0 + hl)
                        nc.vector.reciprocal(rs[:, hl:hl + 1],
                                             av[ti][:, c0 + 64:c0 + 65])
                        if on_act:
                            nc.scalar.activation(
                                yn[:, 64 * hl:64 * hl + 64],
                                av[ti][:, c0:c0 + 64],
                                AF.Copy, scale=rs[:, hl:hl + 1])
                        else:
                            nc.vector.tensor_scalar_mul(
                                yn[:, 64 * hl:64 * hl + 64],
                                av[ti][:, c0:c0 + 64],
                                rs[:, hl:hl + 1],
                            )

                    def transp():
                        tp = psB.tile([128, 128], F16, tag="mm",
                                      name=f"tp{hp}_{qc}_{qs}")
                        nc.tensor.transpose(tp[:, 0:128], yn[:, 0:128],
                                            id_sb[:, 0:128])
                        nc.vector.tensor_copy(
                            yT_sb[hp][:, q0 + qs * 128:q0 + qs * 128 + 128],
                            tp[:, 0:128],
                        )

                    return transp

                for jt in range(njt):
                    L = max(0, jt * 128 - q0)
                    s_ps = psA.tile([128, 1024], F32, tag="A", name="s_ps")
                    for hl in range(2):
                        h = 2 * hp + hl
                        nc.tensor.matmul(
                            s_ps[:, hl * 512 + L:(hl + 1) * 512],
                            k8[32 * h:32 * h + 32, :,
                               jt * 128:(jt + 1) * 128],
                            q8[32 * h:32 * h + 32, :, q0 + L:q0 + 512],
                            start=True,
                            stop=True,
                            perf_mode=DR,
                            tile_position=(32 * h, 0),
                        )
                    p_sb = p_pool.tile([128, 1024], F16, tag="p", name="p_sb")
                    nc.scalar.activation(
                        p_sb[:, L:1024], s_ps[:, L:1024], AF.Exp, scale=SCALE
                    )
                    if jt >= 4 * qc:
                        # DVE (not Pool): the mask sits on the exp->AV
                        # critical path; Pool's Q7 launch + 0.42x multiply
                        # would add ~400ns to every diagonal block.
                        nc.vector.tensor_mul(
                            p_sb[:, L:L + 128], p_sb[:, L:L + 128], mask_sb[:]
                        )
                        nc.vector.tensor_mul(
                            p_sb[:, 512 + L:512 + L + 128],
                            p_sb[:, 512 + L:512 + L + 128],
                            mask_sb[:],
                        )
                    # pace slots evenly across the chunk's jts; popped before
                    # the trailing AV so filler hides the exp latency
                    while slots and n_popped < (jt + 1) * n_slots / njt:
                        slots.pop(0)()
                        n_popped += 1
                    pend.append((jt, p_sb))
                    if len(pend) > trail:
                        jp, pp = pend.pop(0)
                        emit_av(jp, pp)
                        if jp >= 4 * qc:
                            qs_done = jp - 4 * qc
                            finish_qs(qs_done)
                            for s in post.pop(qs_done, ()):
                                s()
                for jp, pp in pend:
                    emit_av(jp, pp)
                    if jp >= 4 * qc:
                        qs_done = jp - 4 * qc
                        finish_qs(qs_done)
                        for s in post.pop(qs_done, ()):
                            s()
                for s in slots:
                    s()
                for qs_done in sorted(post):
                    for s in post[qs_done]:
                        s()

            # ---- schedule ----------------------------------------------
            def qk_u(ch, jt):
                return qk_parts(ch, jt)

            def v_u(ch, tt):
                return v_parts(ch, tt)

            # prologue: qk blocks of ch0 + v0 (A0's first AV needs it)
            for jt in range(4):
                emit_qk_jt(0, jt)
            emit_v_tt(0, 0)
            attn_qc(0, 0, v_u(0, 1) + v_u(0, 2) + v_u(0, 3))
            # B0 carries the ch1 qk blocks (paced by the x ch1-3 DMAs)
            attn_qc(1, 0, qk_u(1, 0) + qk_u(1, 1) + qk_u(1, 2) + qk_u(1, 3))
            attn_qc(0, 1, v_u(1, 4) + v_u(1, 5) + v_u(1, 6) + v_u(1, 7))
            attn_qc(1, 1, qk_u(2, 0) + qk_u(2, 1) + qk_u(2, 2) + qk_u(2, 3))
            attn_qc(0, 2, v_u(2, 8) + v_u(2, 9) + v_u(2, 10) + v_u(2, 11)
                    + pj(0) + pj(1))
            attn_qc(1, 2, qk_u(3, 0) + qk_u(3, 1) + qk_u(3, 2) + qk_u(3, 3)
                    + pj(2) + pj(3))
            attn_qc(0, 3, v_u(3, 12) + v_u(3, 13) + v_u(3, 14) + v_u(3, 15)
                    + pj(4) + pj(5) + pj(6) + pj(7))
            # pj(12..15) read yT columns B3 itself writes: emit each right
            # after B3's finish_qs for that query sub-block.
            attn_qc(1, 3, pj(8) + pj(9) + pj(10) + pj(11),
                    post=[pj(12), pj(13), pj(14), pj(15)])

    if not nc.is_finalized():
        nc.finalize()
    return nc


def host_prep(x, W_attn, b_attn, W_proj):
    bf = np.float16
    x = np.ascontiguousarray(np.asarray(x, np.float32))
    W_attn = np.ascontiguousarray(np.asarray(W_attn, np.float32))
    b_attn = np.ascontiguousarray(np.asarray(b_attn, np.float32))
    W_proj = np.ascontiguousarray(np.asarray(W_proj, np.float32))
    mask = np.triu(np.ones((128, 128), np.float32)).astype(bf)
    ones = np.ones((1, 128), bf)
    id16 = np.eye(128, dtype=bf)
    per_group = []
    for hg in range(NG):
        heads = [hg * HPG + i for i in range(HPG)]
        # wqk column blocks jt: 0=q hd-lo, 1=q hd-hi, 2=k hd-lo, 3=k hd-hi;
        # within a block, 4 heads x 32 (head-major)
        cols, bias = [], []
        for base in (0, C):  # q then k
            for half in (0, 32):
                for h in heads:
                    c0 = base + h * HD + half
                    cols.append(W_attn[:, c0:c0 + 32])
                    bias.append(b_attn[c0:c0 + 32])
        # [C, 512] -> [128, 8, 512]: partition p, ct-slab, col
        wqk_ = np.ascontiguousarray(
            np.concatenate(cols, axis=1).astype(bf)
            .reshape(8, 128, 512).transpose(1, 0, 2))
        bqk_ = np.ascontiguousarray(
            np.concatenate(bias).reshape(4, 128).T.astype(np.float32))
        wv_ = np.zeros((C, VW), np.float32)
        bv_ = np.zeros((1, VW), np.float32)
        for i, h in enumerate(heads):
            wv_[:, 65 * i:65 * i + 64] = \
                W_attn[:, 2 * C + h * HD:2 * C + (h + 1) * HD]
            bv_[0, 65 * i:65 * i + 64] = \
                b_attn[2 * C + h * HD:2 * C + (h + 1) * HD]
            bv_[0, 65 * i + 64] = 1.0
        # [256, 1024] -> [128, 2, 1024]
        wp_ = np.ascontiguousarray(
            np.concatenate([W_proj[h * HD:(h + 1) * HD, :] for h in heads],
                           axis=0).astype(bf)
            .reshape(2, 128, 1024).transpose(1, 0, 2))
        per_group.append((wqk_, bqk_, wv_.astype(bf), bv_.astype(bf), wp_))
    in_maps = []
    for b in range(B):
        xT_b = np.ascontiguousarray(
            x[b].T.astype(bf).reshape(8, 128, T))
        for hg in range(NG):
            wqk_, bqk_, wv_, bv_, wp_ = per_group[hg]
            in_maps.append(
                dict(xT=xT_b, wqk=wqk_, bqk=bqk_, wv=wv_, bv=bv_, wp=wp_,
                     mask=mask, ones=ones, id16=id16)
            )
    return in_maps


_prog_cache = {}


def _get_program(qk_bias=False):
    key = ("nc", qk_bias)
    if key not in _prog_cache:
        _prog_cache[key] = build_program(qk_bias=qk_bias)
    return _prog_cache[key]


def run_cores(in_maps, trace=False, qk_bias=False, **kw):
    return run_bass_kernel_spmd(
        _get_program(qk_bias), in_maps, list(range(NCORES)), trace=trace, **kw
    )


def kernel(x, W_attn, b_attn, W_proj, b_proj):
    in_maps = host_prep(x, W_attn, b_attn, W_proj)
    qk_bias = bool(np.any(np.asarray(b_attn, np.float32)[: 2 * C]))
    br = run_cores(in_maps, qk_bias=qk_bias)
    b_proj = np.asarray(b_proj, np.float32)
    y = np.zeros((B, T, C), np.float32)
    for b in range(B):
        acc = np.zeros((T, C), np.float32)
        for hg in range(NG):
            acc += np.asarray(br.results[b * NG + hg]["out"])
        y[b] = acc + b_proj[None, :]
    return y


# revision 47
# speedup vs baseline: 1.3915x; 1.0108x over previous
"""Causal self-attention (B=2, T=2048, C=1024, H=16) on 8 TRN2 NeuronCores.

Sharding: core = b*4 + hg (data parallel over batch, tensor parallel over
4 head-groups of 4 heads). Each core computes its head-group's attention and
a partial output projection; the host sums the 4 partials per batch and adds
b_proj.

Per-core device program (v4 — fp8 DoubleRow scores + moving-v AV):
  - qk projection writes q8/k8 as fp8e4 [128, 2, T] tiles: partitions
    32h..32h+31 hold head h, slab i = head-dim half i. wqk's column order is
    permuted on the host so each [128,512] PSUM block lands with ONE copy.
  - scores use fp8 DoubleRow matmuls (0.5 cyc/row): per (head, key-block)
    one matmul, stationary k8 [32,2,128], moving q8 [32,2,512-L].
    Quantization error ~1% total, well under the 2e-2 gate.
  - AV is restructured: stationary p [128 keys, 128 q], moving v_aug
    [128, 65] (64 v cols + ones col accumulating the softmax denominator D
    per query ON the query partition). 65 moving cols per (key-block,
    q-subblock, head) beats the old 512-wide moving-p form ~2x, and D lands
    as a per-partition scalar so normalization is a DVE tensor_scalar
    (no PE broadcast matmuls).
  - y [q, hd] is normalized via reciprocal+tensor_scalar then PE-transposed
    (fp16, via identity) into yT for the projection. wp/yT are fp16.
  - ACT runs exps only; all copies go to DVE/Pool; DMA issue on SP/ACT
    (prologue) and gpsimd SWDGE for weights/consts.
  - PSUM: scores 2x[128,1024] (4 banks) + two AV group tiles [128,260]
    (qs01/qs23 x hl, 65-wide blocks, pending-zero init, 1 bank each) +
    shared [128,512] pool for qkv/proj/transpose (2 banks) = 8 banks.
  - Schedule: attention chunk (hp, qc) interleaves with qkv column-block
    ch=qc; qkv units and projection tiles are woven into the ACT-paced
    jt loops as slots to keep the PE fed.
"""

import math

import numpy as np

import concourse.bass as bass
import concourse.bacc as bacc
import concourse.mybir as mybir
from concourse import tile
from concourse.bass_utils import run_bass_kernel_spmd

B, T, C, H = 2, 2048, 1024, 16
HD = C // H   # 64
HPG = 4       # heads per group
NG = 4        # head groups
NCORES = 8
VW = 260      # v_aug width: 4 heads x (64 v + 1 ones)

F32 = mybir.dt.float32
F16 = mybir.dt.float16
F8 = mybir.dt.float8e4
AF = mybir.ActivationFunctionType
DR = mybir.MatmulPerfMode.DoubleRow
SCALE = 1.0 / math.sqrt(C)  # 1/32


def build_program(reps=1, qk_bias=False, v_bias=False):
    nc = bacc.Bacc()

    xT = nc.dram_tensor("xT", [8, 128, T], F16, kind="ExternalInput")
    wqk = nc.dram_tensor("wqk", [128, 8, 512], F16, kind="ExternalInput")
    bqk = nc.dram_tensor("bqk", [128, 4], F32, kind="ExternalInput")
    wv = nc.dram_tensor("wv", [C, VW], F16, kind="ExternalInput")
    bv = nc.dram_tensor("bv", [1, VW], F16, kind="ExternalInput")
    wp = nc.dram_tensor("wp", [128, 2, 1024], F16, kind="ExternalInput")
    mask = nc.dram_tensor("mask", [128, 128], F16, kind="ExternalInput")
    ones = nc.dram_tensor("ones", [1, 128], F16, kind="ExternalInput")
    id16 = nc.dram_tensor("id16", [128, 128], F16, kind="ExternalInput")
    onec = nc.dram_tensor("onec", [128, 1], F16, kind="ExternalInput")
    out = nc.dram_tensor("out", [T, C], F16, kind="ExternalOutput")

    with tile.TileContext(nc) as tc:
        with (
            tc.tile_pool(name="big", bufs=8) as big_pool,
            tc.tile_pool(name="wqk", bufs=1) as wqk_pool,
            tc.tile_pool(name="wv", bufs=8) as wv_pool,
            tc.tile_pool(name="wp", bufs=1) as wp_pool,
            tc.tile_pool(name="qk8", bufs=2) as qk8_pool,
            tc.tile_pool(name="pp", bufs=6) as p_pool,
            tc.tile_pool(name="vsb", bufs=16) as v_pool,
            tc.tile_pool(name="yT", bufs=2) as yT_pool,
            tc.tile_pool(name="yn", bufs=3) as yn_pool,
            tc.tile_pool(name="rr", bufs=3) as r_pool,
            tc.tile_pool(name="osb", bufs=3) as o_pool,
            tc.tile_pool(name="consts", bufs=1) as c_pool,
            tc.tile_pool(name="psA", bufs=2, space="PSUM") as psA,
            tc.tile_pool(name="psVa", bufs=1, space="PSUM") as psVa,
            tc.tile_pool(name="psVb", bufs=1, space="PSUM") as psVb,
            tc.tile_pool(name="psB", bufs=2, space="PSUM") as psB,
        ):
          for rep in range(reps):
            # ---- loads. HWDGE issue is a serialized ~630ns/DMA device, so
            # inputs use few big DMAs: wqk in 2 halves, x ch0 per-ct (fine
            # grain feeds the first chains), x ch1-3 as one [128,1536] DMA
            # per ct, wp as one DMA. consts/wv go via gpsimd SWDGE (bypasses
            # HWDGE entirely).
            qdma = [nc.sync, nc.scalar]
            wqk_sb = wqk_pool.tile([128, 8, 512], F16, tag="wqk")
            xtc0, xtcR = [], []
            for qtr in range(4):
                qdma[qtr % 2].dma_start(
                    wqk_sb[:, 2 * qtr:2 * qtr + 2, :],
                    wqk[:, 2 * qtr:2 * qtr + 2, :])
            for ct in range(8):
                t_ = big_pool.tile([128, 512], F16, tag="big0",
                                   name=f"x0_{ct}")
                qdma[ct % 2].dma_start(t_[:], xT[ct, :, 0:512])
                xtc0.append(t_)
            mask_sb = c_pool.tile([128, 128], F16, tag="mask")
            nc.gpsimd.dma_start(mask_sb[:], mask[:])
            id_sb = c_pool.tile([128, 128], F16, tag="id16")
            nc.gpsimd.dma_start(id_sb[:], id16[:])
            ones_sb = c_pool.tile([1, 128], F16, tag="ones")
            nc.gpsimd.dma_start(ones_sb[:], ones[:])
            onec_sb = c_pool.tile([128, 1], F16, tag="onec")
            nc.gpsimd.dma_start(onec_sb[:], onec[:])
            if v_bias:
                bv_sb = c_pool.tile([1, VW], F16, tag="bv")
                nc.gpsimd.dma_start(bv_sb[:], bv[:])
            if qk_bias:
                bqk_sb = c_pool.tile([128, 4], F32, tag="bqk")
                nc.gpsimd.dma_start(bqk_sb[:], bqk[:])
            wv_sb = []
            for ct in range(8):
                t_ = wv_pool.tile([128, VW], F16, tag="wv", name=f"wv{ct}")
                nc.gpsimd.dma_start(t_[:], wv[ct * 128:(ct + 1) * 128, :])
                wv_sb.append(t_)
            for ct in range(8):
                t_ = big_pool.tile([128, 1536], F16, tag="bigR",
                                   name=f"xR_{ct}")
                qdma[ct % 2].dma_start(t_[:], xT[ct, :, 512:2048])
                xtcR.append(t_)
            wp_sb = wp_pool.tile([128, 2, 1024], F16, tag="wp")
            nc.sync.dma_start(wp_sb[:], wp[:])

            def xs(ct, ch, c0, c1):
                if ch == 0:
                    return xtc0[ct][:, c0:c1]
                base = (ch - 1) * 512
                return xtcR[ct][:, base + c0:base + c1]

            q8 = qk8_pool.tile([128, 2, T], F8, tag="qk8", name="q8")
            k8 = qk8_pool.tile([128, 2, T], F8, tag="qk8", name="k8")
            yT_sb = [
                yT_pool.tile([128, T], F16, tag="yT", name=f"yT{m}")
                for m in range(2)
            ]
            v_sb = [None] * 16

            # ---- emitters ----------------------------------------------
            def qk_parts(ch, jt):
                """qk projection block as two half-chains (finer slots).
                psum rows = 4 heads x 32 hd-half."""
                cell = []

                def half(h0):
                    if h0 == 0:
                        cell.append(psB.tile([128, 512], F32, tag="mm",
                                             name=f"qk{ch}_{jt}"))
                    ps = cell[0]
                    for ct in range(h0, h0 + 4):
                        nc.tensor.matmul(
                            ps[:, 0:512],
                            wqk_sb[:, ct:ct + 1, jt * 128:(jt + 1) * 128],
                            xs(ct, ch, 0, 512),
                            start=(ct == 0),
                            stop=(ct == 7),
                        )
                    if h0 == 0:
                        return
                    dst_t = q8 if jt < 2 else k8
                    dst = dst_t[:, jt % 2:jt % 2 + 1, ch * 512:(ch + 1) * 512]
                    if qk_bias:
                        nc.vector.tensor_scalar_add(dst, ps[:, 0:512],
                                                    bqk_sb[:, jt:jt + 1])
                    elif ch == 0 and jt % 2 == 1:
                        nc.scalar.copy(dst, ps[:, 0:512])
                    else:
                        nc.vector.tensor_copy(dst, ps[:, 0:512])

                return [lambda: half(0), lambda: half(4)]

            def emit_qk_jt(ch, jt):
                for p in qk_parts(ch, jt):
                    p()

            def v_parts(ch, tt):
                """v_aug projection block as two half-chains."""
                cell = []
                xblk = (tt % 4) * 128

                vwid = VW if v_bias else 256

                def half(h0):
                    if h0 == 0:
                        cell.append(psB.tile([128, 512], F32, tag="mm",
                                             name=f"v{tt}"))
                        if v_bias:
                            nc.tensor.matmul(
                                cell[0][:, 0:VW],
                                ones_sb[0:1, 0:128],
                                bv_sb[0:1, 0:VW],
                                start=True,
                                stop=False,
                            )
                    ps = cell[0]
                    for ct in range(h0, h0 + 4):
                        nc.tensor.matmul(
                            ps[:, 0:vwid],
                            xs(ct, ch, xblk, xblk + 128),
                            wv_sb[ct][:, 0:vwid],
                            start=(not v_bias and ct == 0),
                            stop=(ct == 7),
                            skip_group_check=True,
                        )
                    if h0 == 0:
                        return
                    t_ = v_pool.tile([128, vwid], F16, tag="v", name=f"v{tt}")
                    # tt==0 runs in the prologue where ACT is still idle
                    if tt == 0:
                        nc.scalar.copy(t_[:], ps[:, 0:vwid])
                    else:
                        nc.vector.tensor_copy(t_[:], ps[:, 0:vwid])
                    v_sb[tt] = t_

                return [lambda: half(0), lambda: half(4)]

            def emit_v_tt(ch, tt):
                for p in v_parts(ch, tt):
                    p()

            def emit_proj_nch(tt, nch):
                ps = psB.tile([128, 512], F32, tag="mm", name=f"pj{tt}_{nch}")
                for mt in range(2):
                    nc.tensor.matmul(
                        ps[:, 0:512],
                        yT_sb[mt][:, tt * 128:(tt + 1) * 128],
                        wp_sb[:, mt:mt + 1, nch * 512:(nch + 1) * 512],
                        start=(mt == 0),
                        stop=(mt == 1),
                    )
                o = o_pool.tile([128, 512], F16, tag="o", name=f"o{tt}_{nch}")
                # tt>=14 is emitted after the last exp: ACT is idle there and
                # this skips the DVE tail backlog
                if tt >= 14:
                    nc.scalar.copy(o[:], ps[:, 0:512])
                else:
                    nc.vector.tensor_copy(o[:], ps[:, 0:512])
                nc.sync.dma_start(
                    out[tt * 128:(tt + 1) * 128, nch * 512:(nch + 1) * 512],
                    o[:],
                )

            def pj(tt):
                return [lambda: emit_proj_nch(tt, 0),
                        lambda: emit_proj_nch(tt, 1)]

            # ---- attention chunk ---------------------------------------
            def attn_qc(hp, qc, slots=(), post=()):
                """post[qs]: thunks emitted right after finish_qs(qs) —
                for work that depends on this chunk's own yT writes."""
                slots = list(slots)
                post = {i: list(p) for i, p in enumerate(post)}
                q0 = qc * 512
                njt = 4 * qc + 4
                av = [
                    psVa.tile([128, VW], F32, tag="avA", name=f"av{hp}{qc}a"),
                    psVb.tile([128, VW], F32, tag="avB", name=f"av{hp}{qc}b"),
                ]
                started = [False, False]
                pend, trail, tpend = [], 2, []
                n_slots, n_popped = len(slots), 0

                def emit_av(jp, pp):
                    for qs in range(max(0, jp - 4 * qc), 4):
                        ti = qs // 2
                        st = not started[ti]
                        started[ti] = True
                        for hl in range(2):
                            g = 2 * (qs % 2) + hl
                            h = 2 * hp + hl
                            stat = pp[:, hl * 512 + qs * 128:
                                      hl * 512 + qs * 128 + 128]
                            stop = jp == 4 * qc + qs
                            if v_bias:
                                nc.tensor.matmul(
                                    av[ti][:, 65 * g:65 * g + 65],
                                    stat,
                                    v_sb[jp][:, 65 * h:65 * h + 65],
                                    start=(st and hl == 0),
                                    stop=stop,
                                    skip_group_check=True,
                                )
                            else:
                                nc.tensor.matmul(
                                    av[ti][:, 64 * g:64 * g + 64],
                                    stat,
                                    v_sb[jp][:, 64 * h:64 * h + 64],
                                    start=(st and hl == 0),
                                    stop=stop,
                                    skip_group_check=True,
                                )
                                # D accumulates via a 1-col matmul sharing
                                # the stationary p (ldweights skipped)
                                nc.tensor.matmul(
                                    av[ti][:, 256 + g:257 + g],
                                    stat,
                                    onec_sb[:, 0:1],
                                    start=False,
                                    stop=stop,
                                    skip_group_check=True,
                                )

                def finish_qs(qs):
                    """Normalize (recip + scale) now; return a thunk doing
                    the PE transpose + yT copy, deferred so the PE never
                    waits on this DVE chain."""
                    ti, g0 = qs // 2, 2 * (qs % 2)
                    rs = r_pool.tile([128, 2], F32, tag="r",
                                     name=f"r{hp}_{qc}_{qs}")
                    yn = yn_pool.tile([128, 128], F16, tag="yn",
                                      name=f"yn{hp}_{qc}_{qs}")
                    # the very last finishes run after the final exp: use the
                    # idle ACT engine instead of the backlogged DVE
                    on_act = hp == 1 and qc == 3 and qs >= 2
                    for hl in range(2):
                        if v_bias:
                            c0, cd = 65 * (g0 + hl), 65 * (g0 + hl) + 64
                        else:
                            c0, cd = 64 * (g0 + hl), 256 + g0 + hl
                        nc.vector.reciprocal(rs[:, hl:hl + 1],
                                             av[ti][:, cd:cd + 1])
                        if on_act:
                            nc.scalar.activation(
                                yn[:, 64 * hl:64 * hl + 64],
                                av[ti][:, c0:c0 + 64],
                                AF.Copy, scale=rs[:, hl:hl + 1])
                        else:
                            nc.vector.tensor_scalar_mul(
                                yn[:, 64 * hl:64 * hl + 64],
                                av[ti][:, c0:c0 + 64],
                                rs[:, hl:hl + 1],
                            )

                    def transp():
                        tp = psB.tile([128, 128], F16, tag="mm",
                                      name=f"tp{hp}_{qc}_{qs}")
                        nc.tensor.transpose(tp[:, 0:128], yn[:, 0:128],
                                            id_sb[:, 0:128])
                        nc.vector.tensor_copy(
                            yT_sb[hp][:, q0 + qs * 128:q0 + qs * 128 + 128],
                            tp[:, 0:128],
                        )

                    return transp

                for jt in range(njt):
                    L = max(0, jt * 128 - q0)
                    s_ps = psA.tile([128, 1024], F32, tag="A", name="s_ps")
                    for hl in range(2):
                        h = 2 * hp + hl
                        nc.tensor.matmul(
                            s_ps[:, hl * 512 + L:(hl + 1) * 512],
                            k8[32 * h:32 * h + 32, :,
                               jt * 128:(jt + 1) * 128],
                            q8[32 * h:32 * h + 32, :, q0 + L:q0 + 512],
                            start=True,
                            stop=True,
                            perf_mode=DR,
                            tile_position=(32 * h, 0),
                        )
                    p_sb = p_pool.tile([128, 1024], F16, tag="p", name="p_sb")
                    nc.scalar.activation(
                        p_sb[:, L:1024], s_ps[:, L:1024], AF.Exp, scale=SCALE
                    )
                    if jt >= 4 * qc:
                        # DVE (not Pool): the mask sits on the exp->AV
                        # critical path; Pool's Q7 launch + 0.42x multiply
                        # would add ~400ns to every diagonal block.
                        nc.vector.tensor_mul(
                            p_sb[:, L:L + 128], p_sb[:, L:L + 128], mask_sb[:]
                        )
                        nc.vector.tensor_mul(
                            p_sb[:, 512 + L:512 + L + 128],
                            p_sb[:, 512 + L:512 + L + 128],
                            mask_sb[:],
                        )
                    pend.append((jt, p_sb))
                    if len(pend) > trail:
                        jp, pp = pend.pop(0)
                        emit_av(jp, pp)
                        if jp >= 4 * qc:
                            qs_done = jp - 4 * qc
                            tpend.append((qs_done, finish_qs(qs_done)))
                    # slots (PE filler) run between the normalize (DVE) and
                    # the transpose that consumes it, hiding that latency
                    while slots and n_popped < (jt + 1) * n_slots / njt:
                        slots.pop(0)()
                        n_popped += 1
                    if len(tpend) > 0:
                        qs_done, th = tpend.pop(0)
                        th()
                        for s in post.pop(qs_done, ()):
                            s()
                for jp, pp in pend:
                    emit_av(jp, pp)
                    if jp >= 4 * qc:
                        tpend.append((jp - 4 * qc, finish_qs(jp - 4 * qc)))
                for qs_done, th in tpend:
                    th()
                    for s in post.pop(qs_done, ()):
                        s()
                for s in slots:
                    s()
                for qs_done in sorted(post):
                    for s in post[qs_done]:
                        s()

            # ---- schedule ----------------------------------------------
            def qk_u(ch, jt):
                return qk_parts(ch, jt)

            def v_u(ch, tt):
                return v_parts(ch, tt)

            # prologue: qk blocks of ch0 + v0 (A0's first AV needs it)
            for jt in range(4):
                emit_qk_jt(0, jt)
            emit_v_tt(0, 0)
            attn_qc(0, 0, v_u(0, 1) + v_u(0, 2) + v_u(0, 3))
            # B0 carries the ch1 qk blocks (paced by the x ch1-3 DMAs)
            attn_qc(1, 0, qk_u(1, 0) + qk_u(1, 1) + qk_u(1, 2) + qk_u(1, 3))
            attn_qc(0, 1, v_u(1, 4) + v_u(1, 5) + v_u(1, 6) + v_u(1, 7))
            attn_qc(1, 1, qk_u(2, 0) + qk_u(2, 1) + qk_u(2, 2) + qk_u(2, 3))
            attn_qc(0, 2, v_u(2, 8) + v_u(2, 9) + v_u(2, 10) + v_u(2, 11)
                    + pj(0) + pj(1))
            attn_qc(1, 2, qk_u(3, 0) + qk_u(3, 1) + qk_u(3, 2) + qk_u(3, 3)
                    + pj(2) + pj(3))
            attn_qc(0, 3, v_u(3, 12) + v_u(3, 13) + v_u(3, 14) + v_u(3, 15)
                    + pj(4) + pj(5) + pj(6) + pj(7))
            # pj(12..15) read yT columns B3 itself writes: emit each right
            # after B3's finish_qs for that query sub-block.
            attn_qc(1, 3, pj(8) + pj(9) + pj(10) + pj(11),
                    post=[pj(12), pj(13), pj(14), pj(15)])

    if not nc.is_finalized():
        nc.finalize()
    return nc


def host_prep(x, W_attn, b_attn, W_proj):
    v_bias = bool(np.any(np.asarray(b_attn, np.float32)[2 * C:]))
    bf = np.float16
    x = np.ascontiguousarray(np.asarray(x, np.float32))
    W_attn = np.ascontiguousarray(np.asarray(W_attn, np.float32))
    b_attn = np.ascontiguousarray(np.asarray(b_attn, np.float32))
    W_proj = np.ascontiguousarray(np.asarray(W_proj, np.float32))
    mask = np.triu(np.ones((128, 128), np.float32)).astype(bf)
    ones = np.ones((1, 128), bf)
    id16 = np.eye(128, dtype=bf)
    onec_ = np.ones((128, 1), bf)
    per_group = []
    for hg in range(NG):
        heads = [hg * HPG + i for i in range(HPG)]
        # wqk column blocks jt: 0=q hd-lo, 1=q hd-hi, 2=k hd-lo, 3=k hd-hi;
        # within a block, 4 heads x 32 (head-major)
        cols, bias = [], []
        for base in (0, C):  # q then k
            for half in (0, 32):
                for h in heads:
                    c0 = base + h * HD + half
                    cols.append(W_attn[:, c0:c0 + 32])
                    bias.append(b_attn[c0:c0 + 32])
        # [C, 512] -> [128, 8, 512]: partition p, ct-slab, col
        wqk_ = np.ascontiguousarray(
            np.concatenate(cols, axis=1).astype(bf)
            .reshape(8, 128, 512).transpose(1, 0, 2))
        bqk_ = np.ascontiguousarray(
            np.concatenate(bias).reshape(4, 128).T.astype(np.float32))
        wv_ = np.zeros((C, VW), np.float32)
        bv_ = np.zeros((1, VW), np.float32)
        vb = 65 if v_bias else 64
        for i, h in enumerate(heads):
            wv_[:, vb * i:vb * i + 64] = \
                W_attn[:, 2 * C + h * HD:2 * C + (h + 1) * HD]
            if v_bias:
                bv_[0, 65 * i:65 * i + 64] = \
                    b_attn[2 * C + h * HD:2 * C + (h + 1) * HD]
                bv_[0, 65 * i + 64] = 1.0
        # [256, 1024] -> [128, 2, 1024]
        wp_ = np.ascontiguousarray(
            np.concatenate([W_proj[h * HD:(h + 1) * HD, :] for h in heads],
                           axis=0).astype(bf)
            .reshape(2, 128, 1024).transpose(1, 0, 2))
        per_group.append((wqk_, bqk_, wv_.astype(bf), bv_.astype(bf), wp_))
    in_maps = []
    for b in range(B):
        xT_b = np.ascontiguousarray(
            x[b].T.astype(bf).reshape(8, 128, T))
        for hg in range(NG):
            wqk_, bqk_, wv_, bv_, wp_ = per_group[hg]
            in_maps.append(
                dict(xT=xT_b, wqk=wqk_, bqk=bqk_, wv=wv_, bv=bv_, wp=wp_,
                     mask=mask, ones=ones, id16=id16, onec=onec_)
            )
    return in_maps


_prog_cache = {}


def _get_program(qk_bias=False, v_bias=False):
    key = ("nc", qk_bias, v_bias)
    if key not in _prog_cache:
        _prog_cache[key] = build_program(qk_bias=qk_bias, v_bias=v_bias)
    return _prog_cache[key]


def run_cores(in_maps, trace=False, qk_bias=False, v_bias=False, **kw):
    return run_bass_kernel_spmd(
        _get_program(qk_bias, v_bias), in_maps, list(range(NCORES)),
        trace=trace, **kw
    )


def kernel(x, W_attn, b_attn, W_proj, b_proj):
    in_maps = host_prep(x, W_attn, b_attn, W_proj)
    b_attn_f = np.asarray(b_attn, np.float32)
    qk_bias = bool(np.any(b_attn_f[: 2 * C]))
    v_bias = bool(np.any(b_attn_f[2 * C:]))
    br = run_cores(in_maps, qk_bias=qk_bias, v_bias=v_bias)
    b_proj = np.asarray(b_proj, np.float32)
    y = np.zeros((B, T, C), np.float32)
    for b in range(B):
        acc = np.zeros((T, C), np.float32)
        for hg in range(NG):
            acc += np.asarray(br.results[b * NG + hg]["out"])
        y[b] = acc + b_proj[None, :]
    return y


# revision 55
# speedup vs baseline: 1.4040x; 1.0090x over previous
"""Causal self-attention (B=2, T=2048, C=1024, H=16) on 8 TRN2 NeuronCores.

Sharding: core = b*4 + hg (data parallel over batch, tensor parallel over
4 head-groups of 4 heads). Each core computes its head-group's attention and
a partial output projection; the host sums the 4 partials per batch and adds
b_proj.

Per-core device program (v4 — fp8 DoubleRow scores + moving-v AV):
  - qk projection writes q8/k8 as fp8e4 [128, 2, T] tiles: partitions
    32h..32h+31 hold head h, slab i = head-dim half i. wqk's column order is
    permuted on the host so each [128,512] PSUM block lands with ONE copy.
  - scores use fp8 DoubleRow matmuls (0.5 cyc/row): per (head, key-block)
    one matmul, stationary k8 [32,2,128], moving q8 [32,2,512-L].
    Quantization error ~1% total, well under the 2e-2 gate.
  - AV is restructured: stationary p [128 keys, 128 q], moving v_aug
    [128, 65] (64 v cols + ones col accumulating the softmax denominator D
    per query ON the query partition). 65 moving cols per (key-block,
    q-subblock, head) beats the old 512-wide moving-p form ~2x, and D lands
    as a per-partition scalar so normalization is a DVE tensor_scalar
    (no PE broadcast matmuls).
  - y [q, hd] is normalized via reciprocal+tensor_scalar then PE-transposed
    (fp16, via identity) into yT for the projection. wp/yT are fp16.
  - ACT runs exps only; all copies go to DVE/Pool; DMA issue on SP/ACT
    (prologue) and gpsimd SWDGE for weights/consts.
  - PSUM: scores 2x[128,1024] (4 banks) + two AV group tiles [128,260]
    (qs01/qs23 x hl, 65-wide blocks, pending-zero init, 1 bank each) +
    shared [128,512] pool for qkv/proj/transpose (2 banks) = 8 banks.
  - Schedule: attention chunk (hp, qc) interleaves with qkv column-block
    ch=qc; qkv units and projection tiles are woven into the ACT-paced
    jt loops as slots to keep the PE fed.
"""

import math

import numpy as np

import concourse.bass as bass
import concourse.bacc as bacc
import concourse.mybir as mybir
from concourse import tile
from concourse.bass_utils import run_bass_kernel_spmd

B, T, C, H = 2, 2048, 1024, 16
HD = C // H   # 64
HPG = 4       # heads per group
NG = 4        # head groups
NCORES = 8
VW = 260      # v_aug width: 4 heads x (64 v + 1 ones)

F32 = mybir.dt.float32
F16 = mybir.dt.float16
F8 = mybir.dt.float8e4
AF = mybir.ActivationFunctionType
DR = mybir.MatmulPerfMode.DoubleRow
SCALE = 1.0 / math.sqrt(C)  # 1/32


def build_program(reps=1, qk_bias=False, v_bias=False):
    nc = bacc.Bacc()

    xT = nc.dram_tensor("xT", [8, 128, T], F16, kind="ExternalInput")
    wqk = nc.dram_tensor("wqk", [128, 8, 512], F16, kind="ExternalInput")
    bqk = nc.dram_tensor("bqk", [128, 4], F32, kind="ExternalInput")
    wv = nc.dram_tensor("wv", [C, VW], F16, kind="ExternalInput")
    bv = nc.dram_tensor("bv", [1, VW], F16, kind="ExternalInput")
    wp = nc.dram_tensor("wp", [128, 2, 1024], F16, kind="ExternalInput")
    mask = nc.dram_tensor("mask", [128, 128], F16, kind="ExternalInput")
    ones = nc.dram_tensor("ones", [1, 128], F16, kind="ExternalInput")
    id16 = nc.dram_tensor("id16", [128, 128], F16, kind="ExternalInput")
    onec = nc.dram_tensor("onec", [128, 1], F16, kind="ExternalInput")
    out = nc.dram_tensor("out", [T, C], F16, kind="ExternalOutput")

    with tile.TileContext(nc) as tc:
        with (
            tc.tile_pool(name="big", bufs=8) as big_pool,
            tc.tile_pool(name="wqk", bufs=1) as wqk_pool,
            tc.tile_pool(name="wv", bufs=8) as wv_pool,
            tc.tile_pool(name="wp", bufs=1) as wp_pool,
            tc.tile_pool(name="qk8", bufs=2) as qk8_pool,
            tc.tile_pool(name="pp", bufs=6) as p_pool,
            tc.tile_pool(name="vsb", bufs=16) as v_pool,
            tc.tile_pool(name="yT", bufs=2) as yT_pool,
            tc.tile_pool(name="yn", bufs=4) as yn_pool,
            tc.tile_pool(name="rr", bufs=3) as r_pool,
            tc.tile_pool(name="osb", bufs=4) as o_pool,
            tc.tile_pool(name="consts", bufs=1) as c_pool,
            tc.tile_pool(name="psA", bufs=2, space="PSUM") as psA,
            tc.tile_pool(name="psVa", bufs=1, space="PSUM") as psVa,
            tc.tile_pool(name="psVb", bufs=1, space="PSUM") as psVb,
            tc.tile_pool(name="psB", bufs=2, space="PSUM") as psB,
        ):
          for rep in range(reps):
            # ---- loads. HWDGE issue is a serialized ~630ns/DMA device, so
            # inputs use few big DMAs: wqk in 2 halves, x ch0 per-ct (fine
            # grain feeds the first chains), x ch1-3 as one [128,1536] DMA
            # per ct, wp as one DMA. consts/wv go via gpsimd SWDGE (bypasses
            # HWDGE entirely).
            qdma = [nc.sync, nc.scalar]
            wqk_sb = wqk_pool.tile([128, 8, 512], F16, tag="wqk")
            xtc0, xtcR = [], []
            for qtr in range(4):
                qdma[qtr % 2].dma_start(
                    wqk_sb[:, 2 * qtr:2 * qtr + 2, :],
                    wqk[:, 2 * qtr:2 * qtr + 2, :])
            for ct in range(8):
                t_ = big_pool.tile([128, 512], F16, tag="big0",
                                   name=f"x0_{ct}")
                qdma[ct % 2].dma_start(t_[:], xT[ct, :, 0:512])
                xtc0.append(t_)
            mask_sb = c_pool.tile([128, 128], F16, tag="mask")
            nc.gpsimd.dma_start(mask_sb[:], mask[:])
            id_sb = c_pool.tile([128, 128], F16, tag="id16")
            nc.gpsimd.dma_start(id_sb[:], id16[:])
            ones_sb = c_pool.tile([1, 128], F16, tag="ones")
            nc.gpsimd.dma_start(ones_sb[:], ones[:])
            onec_sb = c_pool.tile([128, 1], F16, tag="onec")
            nc.gpsimd.dma_start(onec_sb[:], onec[:])
            if v_bias:
                bv_sb = c_pool.tile([1, VW], F16, tag="bv")
                nc.gpsimd.dma_start(bv_sb[:], bv[:])
            if qk_bias:
                bqk_sb = c_pool.tile([128, 4], F32, tag="bqk")
                nc.gpsimd.dma_start(bqk_sb[:], bqk[:])
            wv_sb = []
            for ct in range(8):
                t_ = wv_pool.tile([128, VW], F16, tag="wv", name=f"wv{ct}")
                nc.gpsimd.dma_start(t_[:], wv[ct * 128:(ct + 1) * 128, :])
                wv_sb.append(t_)
            for ct in range(8):
                t_ = big_pool.tile([128, 1536], F16, tag="bigR",
                                   name=f"xR_{ct}")
                qdma[ct % 2].dma_start(t_[:], xT[ct, :, 512:2048])
                xtcR.append(t_)
            wp_sb = wp_pool.tile([128, 2, 1024], F16, tag="wp")
            nc.sync.dma_start(wp_sb[:], wp[:])

            def xs(ct, ch, c0, c1):
                if ch == 0:
                    return xtc0[ct][:, c0:c1]
                base = (ch - 1) * 512
                return xtcR[ct][:, base + c0:base + c1]

            q8 = qk8_pool.tile([128, 2, T], F8, tag="qk8", name="q8")
            k8 = qk8_pool.tile([128, 2, T], F8, tag="qk8", name="k8")
            yT_sb = [
                yT_pool.tile([128, T], F16, tag="yT", name=f"yT{m}")
                for m in range(2)
            ]
            v_sb = [None] * 16

            # ---- emitters ----------------------------------------------
            def qk_parts(ch, jt):
                """qk projection block as two half-chains (finer slots).
                psum rows = 4 heads x 32 hd-half."""
                cell = []

                def half(h0):
                    if h0 == 0:
                        cell.append(psB.tile([128, 512], F32, tag="mm",
                                             name=f"qk{ch}_{jt}"))
                    ps = cell[0]
                    for ct in range(h0, h0 + 4):
                        nc.tensor.matmul(
                            ps[:, 0:512],
                            wqk_sb[:, ct:ct + 1, jt * 128:(jt + 1) * 128],
                            xs(ct, ch, 0, 512),
                            start=(ct == 0),
                            stop=(ct == 7),
                        )
                    if h0 == 0:
                        return
                    dst_t = q8 if jt < 2 else k8
                    dst = dst_t[:, jt % 2:jt % 2 + 1, ch * 512:(ch + 1) * 512]
                    if qk_bias:
                        nc.vector.tensor_scalar_add(dst, ps[:, 0:512],
                                                    bqk_sb[:, jt:jt + 1])
                    elif ch == 0 and jt % 2 == 1:
                        nc.scalar.copy(dst, ps[:, 0:512])
                    elif ch == 1 and jt % 2 == 1:
                        nc.scalar.copy(dst, ps[:, 0:512])
                    else:
                        nc.vector.tensor_copy(dst, ps[:, 0:512])

                return [lambda: half(0), lambda: half(4)]

            def emit_qk_jt(ch, jt):
                for p in qk_parts(ch, jt):
                    p()

            def v_parts(ch, tt):
                """v_aug projection block as two half-chains."""
                cell = []
                xblk = (tt % 4) * 128

                vwid = VW if v_bias else 256

                def half(h0):
                    if h0 == 0:
                        cell.append(psB.tile([128, 512], F32, tag="mm",
                                             name=f"v{tt}"))
                        if v_bias:
                            nc.tensor.matmul(
                                cell[0][:, 0:VW],
                                ones_sb[0:1, 0:128],
                                bv_sb[0:1, 0:VW],
                                start=True,
                                stop=False,
                            )
                    ps = cell[0]
                    for ct in range(h0, h0 + 4):
                        nc.tensor.matmul(
                            ps[:, 0:vwid],
                            xs(ct, ch, xblk, xblk + 128),
                            wv_sb[ct][:, 0:vwid],
                            start=(not v_bias and ct == 0),
                            stop=(ct == 7),
                            skip_group_check=True,
                        )
                    if h0 == 0:
                        return
                    t_ = v_pool.tile([128, vwid], F16, tag="v", name=f"v{tt}")
                    # tt==0 runs in the prologue where ACT is still idle
                    if tt == 0:
                        nc.scalar.copy(t_[:], ps[:, 0:vwid])
                    else:
                        nc.vector.tensor_copy(t_[:], ps[:, 0:vwid])
                    v_sb[tt] = t_

                return [lambda: half(0), lambda: half(4)]

            def emit_v_tt(ch, tt):
                for p in v_parts(ch, tt):
                    p()

            o15 = [None]

            def emit_proj_nch(tt, nch):
                ps = psB.tile([128, 512], F32, tag="mm", name=f"pj{tt}_{nch}")
                for mt in range(2):
                    nc.tensor.matmul(
                        ps[:, 0:512],
                        yT_sb[mt][:, tt * 128:(tt + 1) * 128],
                        wp_sb[:, mt:mt + 1, nch * 512:(nch + 1) * 512],
                        start=(mt == 0),
                        stop=(mt == 1),
                    )
                if tt == 15:
                    # final tile: one [128,1024] DMA, copies split ACT/DVE in
                    # parallel to shorten the drain chain
                    if nch == 0:
                        o15[0] = o_pool.tile([128, 1024], F16, tag="o15", name="o15")
                        nc.vector.tensor_copy(o15[0][:, 0:512], ps[:, 0:512])
                    else:
                        nc.scalar.copy(o15[0][:, 512:1024], ps[:, 0:512])
                        nc.sync.dma_start(out[15 * 128:, :], o15[0][:])
                    return
                o = o_pool.tile([128, 512], F16, tag="o", name=f"o{tt}_{nch}")
                # tt==14 is emitted after the last exp: ACT is idle there and
                # this skips the DVE tail backlog
                if tt >= 14:
                    nc.scalar.copy(o[:], ps[:, 0:512])
                else:
                    nc.vector.tensor_copy(o[:], ps[:, 0:512])
                nc.sync.dma_start(
                    out[tt * 128:(tt + 1) * 128, nch * 512:(nch + 1) * 512],
                    o[:],
                )

            def pj(tt):
                return [lambda: emit_proj_nch(tt, 0),
                        lambda: emit_proj_nch(tt, 1)]

            # ---- attention chunk ---------------------------------------
            def attn_qc(hp, qc, slots=(), post=()):
                """post[qs]: thunks emitted right after finish_qs(qs) —
                for work that depends on this chunk's own yT writes."""
                slots = list(slots)
                post = {i: list(p) for i, p in enumerate(post)}
                q0 = qc * 512
                njt = 4 * qc + 4
                av = [
                    psVa.tile([128, VW], F32, tag="avA", name=f"av{hp}{qc}a"),
                    psVb.tile([128, VW], F32, tag="avB", name=f"av{hp}{qc}b"),
                ]
                started = [False, False]
                pend, trail, tpend = [], 2, []
                n_slots, n_popped = len(slots), 0

                def emit_av(jp, pp):
                    for qs in range(max(0, jp - 4 * qc), 4):
                        ti = qs // 2
                        st = not started[ti]
                        started[ti] = True
                        for hl in range(2):
                            g = 2 * (qs % 2) + hl
                            h = 2 * hp + hl
                            stat = pp[:, hl * 512 + qs * 128:
                                      hl * 512 + qs * 128 + 128]
                            stop = jp == 4 * qc + qs
                            if v_bias:
                                nc.tensor.matmul(
                                    av[ti][:, 65 * g:65 * g + 65],
                                    stat,
                                    v_sb[jp][:, 65 * h:65 * h + 65],
                                    start=(st and hl == 0),
                                    stop=stop,
                                    skip_group_check=True,
                                )
                            else:
                                nc.tensor.matmul(
                                    av[ti][:, 64 * g:64 * g + 64],
                                    stat,
                                    v_sb[jp][:, 64 * h:64 * h + 64],
                                    start=(st and hl == 0),
                                    stop=stop,
                                    skip_group_check=True,
                                )
                                # D accumulates via a 1-col matmul sharing
                                # the stationary p (ldweights skipped)
                                nc.tensor.matmul(
                                    av[ti][:, 256 + g:257 + g],
                                    stat,
                                    onec_sb[:, 0:1],
                                    start=False,
                                    stop=stop,
                                    skip_group_check=True,
                                )

                def finish_qs(qs):
                    """Normalize (recip + scale) now; return a thunk doing
                    the PE transpose + yT copy, deferred so the PE never
                    waits on this DVE chain."""
                    ti, g0 = qs // 2, 2 * (qs % 2)
                    rs = r_pool.tile([128, 2], F32, tag="r",
                                     name=f"r{hp}_{qc}_{qs}")
                    yn = yn_pool.tile([128, 128], F16, tag="yn",
                                      name=f"yn{hp}_{qc}_{qs}")
                    # the very last finishes run after the final exp: use the
                    # idle ACT engine instead of the backlogged DVE
                    on_act = hp == 1 and qc == 3 and qs >= 2
                    for hl in range(2):
                        if v_bias:
                            c0, cd = 65 * (g0 + hl), 65 * (g0 + hl) + 64
                        else:
                            c0, cd = 64 * (g0 + hl), 256 + g0 + hl
                        nc.vector.reciprocal(rs[:, hl:hl + 1],
                                             av[ti][:, cd:cd + 1])
                        if on_act:
                            nc.scalar.activation(
                                yn[:, 64 * hl:64 * hl + 64],
                                av[ti][:, c0:c0 + 64],
                                AF.Copy, scale=rs[:, hl:hl + 1])
                        else:
                            nc.vector.tensor_scalar_mul(
                                yn[:, 64 * hl:64 * hl + 64],
                                av[ti][:, c0:c0 + 64],
                                rs[:, hl:hl + 1],
                            )

                    def transp():
                        tp = psB.tile([128, 128], F16, tag="mm",
                                      name=f"tp{hp}_{qc}_{qs}")
                        nc.tensor.transpose(tp[:, 0:128], yn[:, 0:128],
                                            id_sb[:, 0:128])
                        dst = yT_sb[hp][:, q0 + qs * 128:q0 + qs * 128 + 128]
                        if on_act:
                            nc.scalar.copy(dst, tp[:, 0:128])
                        else:
                            nc.vector.tensor_copy(dst, tp[:, 0:128])

                    return transp

                for jt in range(njt):
                    L = max(0, jt * 128 - q0)
                    s_ps = psA.tile([128, 1024], F32, tag="A", name="s_ps")
                    for hl in range(2):
                        h = 2 * hp + hl
                        nc.tensor.matmul(
                            s_ps[:, hl * 512 + L:(hl + 1) * 512],
                            k8[32 * h:32 * h + 32, :,
                               jt * 128:(jt + 1) * 128],
                            q8[32 * h:32 * h + 32, :, q0 + L:q0 + 512],
                            start=True,
                            stop=True,
                            perf_mode=DR,
                            tile_position=(32 * h, 0),
                        )
                    p_sb = p_pool.tile([128, 1024], F16, tag="p", name="p_sb")
                    nc.scalar.activation(
                        p_sb[:, L:1024], s_ps[:, L:1024], AF.Exp, scale=SCALE
                    )
                    if jt >= 4 * qc:
                        # DVE (not Pool): the mask sits on the exp->AV
                        # critical path; Pool's Q7 launch + 0.42x multiply
                        # would add ~400ns to every diagonal block.
                        nc.vector.tensor_mul(
                            p_sb[:, L:L + 128], p_sb[:, L:L + 128], mask_sb[:]
                        )
                        nc.vector.tensor_mul(
                            p_sb[:, 512 + L:512 + L + 128],
                            p_sb[:, 512 + L:512 + L + 128],
                            mask_sb[:],
                        )
                    pend.append((jt, p_sb))
                    if len(pend) > trail:
                        jp, pp = pend.pop(0)
                        emit_av(jp, pp)
                        if jp >= 4 * qc:
                            qs_done = jp - 4 * qc
                            tpend.append((qs_done, finish_qs(qs_done)))
                    # slots (PE filler) run between the normalize (DVE) and
                    # the transpose that consumes it, hiding that latency
                    while slots and n_popped < (jt + 1) * n_slots / njt:
                        slots.pop(0)()
                        n_popped += 1
                    if len(tpend) > 0:
                        qs_done, th = tpend.pop(0)
                        th()
                        for s in post.pop(qs_done, ()):
                            s()
                for jp, pp in pend:
                    emit_av(jp, pp)
                    if jp >= 4 * qc:
                        tpend.append((jp - 4 * qc, finish_qs(jp - 4 * qc)))
                for qs_done, th in tpend:
                    th()
                    for s in post.pop(qs_done, ()):
                        s()
                for s in slots:
                    s()
                for qs_done in sorted(post):
                    for s in post[qs_done]:
                        s()

            # ---- schedule ----------------------------------------------
            def qk_u(ch, jt):
                return qk_parts(ch, jt)

            def v_u(ch, tt):
                return v_parts(ch, tt)

            # prologue: qk blocks of ch0 + v0 (A0's first AV needs it)
            for jt in range(4):
                emit_qk_jt(0, jt)
            emit_v_tt(0, 0)
            attn_qc(0, 0, v_u(0, 1) + v_u(0, 2) + v_u(0, 3))
            # B0 carries the ch1 qk blocks (paced by the x ch1-3 DMAs)
            attn_qc(1, 0, qk_u(1, 0) + qk_u(1, 1) + qk_u(1, 2) + qk_u(1, 3))
            attn_qc(0, 1, v_u(1, 4) + v_u(1, 5) + v_u(1, 6) + v_u(1, 7))
            attn_qc(1, 1, qk_u(2, 0) + qk_u(2, 1) + qk_u(2, 2) + qk_u(2, 3))
            attn_qc(0, 2, v_u(2, 8) + v_u(2, 9) + v_u(2, 10) + v_u(2, 11)
                    + pj(0) + pj(1))
            attn_qc(1, 2, qk_u(3, 0) + qk_u(3, 1) + qk_u(3, 2) + qk_u(3, 3)
                    + pj(2) + pj(3))
            attn_qc(0, 3, v_u(3, 12) + v_u(3, 13) + v_u(3, 14) + v_u(3, 15)
                    + pj(4) + pj(5) + pj(6) + pj(7))
            # pj(12..15) read yT columns B3 itself writes: emit each right
            # after B3's finish_qs for that query sub-block.
            attn_qc(1, 3, pj(8) + pj(9) + pj(10) + pj(11),
                    post=[pj(12), pj(13), pj(14), pj(15)])

    if not nc.is_finalized():
        nc.finalize()
    return nc


def host_prep(x, W_attn, b_attn, W_proj):
    v_bias = bool(np.any(np.asarray(b_attn, np.float32)[2 * C:]))
    bf = np.float16
    x = np.ascontiguousarray(np.asarray(x, np.float32))
    W_attn = np.ascontiguousarray(np.asarray(W_attn, np.float32))
    b_attn = np.ascontiguousarray(np.asarray(b_attn, np.float32))
    W_proj = np.ascontiguousarray(np.asarray(W_proj, np.float32))
    mask = np.triu(np.ones((128, 128), np.float32)).astype(bf)
    ones = np.ones((1, 128), bf)
    id16 = np.eye(128, dtype=bf)
    onec_ = np.ones((128, 1), bf)
    per_group = []
    for hg in range(NG):
        heads = [hg * HPG + i for i in range(HPG)]
        # wqk column blocks jt: 0=q hd-lo, 1=q hd-hi, 2=k hd-lo, 3=k hd-hi;
        # within a block, 4 heads x 32 (head-major)
        cols, bias = [], []
        for base in (0, C):  # q then k
            for half in (0, 32):
                for h in heads:
                    c0 = base + h * HD + half
                    cols.append(W_attn[:, c0:c0 + 32])
                    bias.append(b_attn[c0:c0 + 32])
        # [C, 512] -> [128, 8, 512]: partition p, ct-slab, col
        wqk_ = np.ascontiguousarray(
            np.concatenate(cols, axis=1).astype(bf)
            .reshape(8, 128, 512).transpose(1, 0, 2))
        bqk_ = np.ascontiguousarray(
            np.concatenate(bias).reshape(4, 128).T.astype(np.float32))
        wv_ = np.zeros((C, VW), np.float32)
        bv_ = np.zeros((1, VW), np.float32)
        vb = 65 if v_bias else 64
        for i, h in enumerate(heads):
            wv_[:, vb * i:vb * i + 64] = \
                W_attn[:, 2 * C + h * HD:2 * C + (h + 1) * HD]
            if v_bias:
                bv_[0, 65 * i:65 * i + 64] = \
                    b_attn[2 * C + h * HD:2 * C + (h + 1) * HD]
                bv_[0, 65 * i + 64] = 1.0
        # [256, 1024] -> [128, 2, 1024]
        wp_ = np.ascontiguousarray(
            np.concatenate([W_proj[h * HD:(h + 1) * HD, :] for h in heads],
                           axis=0).astype(bf)
            .reshape(2, 128, 1024).transpose(1, 0, 2))
        per_group.append((wqk_, bqk_, wv_.astype(bf), bv_.astype(bf), wp_))
    in_maps = []
    for b in range(B):
        xT_b = np.ascontiguousarray(
            x[b].T.astype(bf).reshape(8, 128, T))
        for hg in range(NG):
            wqk_, bqk_, wv_, bv_, wp_ = per_group[hg]
            in_maps.append(
                dict(xT=xT_b, wqk=wqk_, bqk=bqk_, wv=wv_, bv=bv_, wp=wp_,
                     mask=mask, ones=ones, id16=id16, onec=onec_)
            )
    return in_maps


_prog_cache = {}


def _get_program(qk_bias=False, v_bias=False):
    key = ("nc", qk_bias, v_bias)
    if key not in _prog_cache:
        _prog_cache[key] = build_program(qk_bias=qk_bias, v_bias=v_bias)
    return _prog_cache[key]


def run_cores(in_maps, trace=False, qk_bias=False, v_bias=False, **kw):
    return run_bass_kernel_spmd(
        _get_program(qk_bias, v_bias), in_maps, list(range(NCORES)),
        trace=trace, **kw
    )


def kernel(x, W_attn, b_attn, W_proj, b_proj):
    in_maps = host_prep(x, W_attn, b_attn, W_proj)
    b_attn_f = np.asarray(b_attn, np.float32)
    qk_bias = bool(np.any(b_attn_f[: 2 * C]))
    v_bias = bool(np.any(b_attn_f[2 * C:]))
    br = run_cores(in_maps, qk_bias=qk_bias, v_bias=v_bias)
    b_proj = np.asarray(b_proj, np.float32)
    y = np.zeros((B, T, C), np.float32)
    for b in range(B):
        acc = np.zeros((T, C), np.float32)
        for hg in range(NG):
            acc += np.asarray(br.results[b * NG + hg]["out"])
        y[b] = acc + b_proj[None, :]
    return y


# revision 60
# speedup vs baseline: 1.4085x; 1.0032x over previous
"""Causal self-attention (B=2, T=2048, C=1024, H=16) on 8 TRN2 NeuronCores.

Sharding: core = b*4 + hg (data parallel over batch, tensor parallel over
4 head-groups of 4 heads). Each core computes its head-group's attention and
a partial output projection; the host sums the 4 partials per batch and adds
b_proj.

Per-core device program (v4 — fp8 DoubleRow scores + moving-v AV;
TimelineSim 130.4us vs the 183.6us v3 baseline):
  - qk projection writes q8/k8 as fp8e4 [128, 2, T] tiles: partitions
    32h..32h+31 hold head h, slab i = head-dim half i. wqk's column order is
    permuted on the host so each [128,512] PSUM block lands with ONE copy.
  - scores use fp8 DoubleRow matmuls (0.5 cyc/row in the cost model): per
    (head, key-block) one matmul, stationary k8 [32,2,128], moving q8
    [32,2,512-L]. Quantization error ~1% total vs the 2e-2 gate.
  - AV is restructured: stationary p [128 keys, 128 q], moving v [128, 64]
    per (key-block, q-subblock, head) — 64 moving cols beat the old
    512-wide moving-p form ~2x. A 1-col matmul vs a ones vector (same
    stationary, ldweights-free) accumulates the softmax denominator D per
    query ON the query partition, so normalization is a per-partition
    tensor_scalar (no PE broadcast matmuls). With v_bias, v blocks carry a
    65th ones column initialized by a bias matmul instead.
  - y [q, hd] is normalized via reciprocal+tensor_scalar then PE-transposed
    (fp16, via identity) into yT for the projection. wp/yT are fp16.
  - ACT runs exps (the pacing engine, ~80us) plus prologue/tail copies;
    everything else copies on DVE. Masks multiply on DVE (Pool's Q7
    launch would sit on the exp->AV critical path). HWDGE issue is a
    serialized ~630ns/DMA device, so inputs load as few big DMAs with a
    host-side relayout; consts/wv go via gpsimd SWDGE.
  - PSUM: scores 2x[128,1024] (4 banks) + AV pool 2x[128,260] (4 groups of
    64 + 4 D cols each, pending-zero init, 1 bank each) + shared [128,512]
    pool for qkv/proj/transpose (2 banks) = 8 banks.
  - Schedule: attention chunk (hp, qc) consumes qkv column-block ch=qc;
    qkv half-chains and projection tiles are paced into the ACT-bound jt
    loops as ~0.5-0.9us slots (popped between the scores and the trailing
    AV, which runs 2 key-blocks behind its exp); per-qs finish chains
    (recip/normalize -> transpose -> yT) emit at each diagonal, and the
    last 4 projection tiles hang off B3's own finishes via post-hooks.
"""

import math

import numpy as np

import concourse.bass as bass
import concourse.bacc as bacc
import concourse.mybir as mybir
from concourse import tile
from concourse.bass_utils import run_bass_kernel_spmd

B, T, C, H = 2, 2048, 1024, 16
HD = C // H   # 64
HPG = 4       # heads per group
NG = 4        # head groups
NCORES = 8
VW = 260      # v_aug width: 4 heads x (64 v + 1 ones)

F32 = mybir.dt.float32
F16 = mybir.dt.float16
F8 = mybir.dt.float8e4
AF = mybir.ActivationFunctionType
DR = mybir.MatmulPerfMode.DoubleRow
SCALE = 1.0 / math.sqrt(C)  # 1/32


def build_program(reps=1, qk_bias=False, v_bias=False):
    nc = bacc.Bacc()

    xT = nc.dram_tensor("xT", [8, 128, T], F16, kind="ExternalInput")
    wqk = nc.dram_tensor("wqk", [128, 8, 512], F16, kind="ExternalInput")
    bqk = nc.dram_tensor("bqk", [128, 4], F32, kind="ExternalInput")
    wv = nc.dram_tensor("wv", [C, VW], F16, kind="ExternalInput")
    bv = nc.dram_tensor("bv", [1, VW], F16, kind="ExternalInput")
    wp = nc.dram_tensor("wp", [128, 2, 1024], F16, kind="ExternalInput")
    mask = nc.dram_tensor("mask", [128, 128], F16, kind="ExternalInput")
    ones = nc.dram_tensor("ones", [1, 128], F16, kind="ExternalInput")
    id16 = nc.dram_tensor("id16", [128, 128], F16, kind="ExternalInput")
    onec = nc.dram_tensor("onec", [128, 1], F16, kind="ExternalInput")
    out = nc.dram_tensor("out", [T, C], F16, kind="ExternalOutput")

    with tile.TileContext(nc) as tc:
        with (
            tc.tile_pool(name="big", bufs=8) as big_pool,
            tc.tile_pool(name="wqk", bufs=1) as wqk_pool,
            tc.tile_pool(name="wv", bufs=8) as wv_pool,
            tc.tile_pool(name="wp", bufs=1) as wp_pool,
            tc.tile_pool(name="qk8", bufs=2) as qk8_pool,
            tc.tile_pool(name="pp", bufs=6) as p_pool,
            tc.tile_pool(name="vsb", bufs=16) as v_pool,
            tc.tile_pool(name="yT", bufs=2) as yT_pool,
            tc.tile_pool(name="yn", bufs=4) as yn_pool,
            tc.tile_pool(name="rr", bufs=3) as r_pool,
            tc.tile_pool(name="osb", bufs=4) as o_pool,
            tc.tile_pool(name="consts", bufs=1) as c_pool,
            tc.tile_pool(name="psA", bufs=2, space="PSUM") as psA,
            tc.tile_pool(name="psV", bufs=2, space="PSUM") as psV,
            tc.tile_pool(name="psB", bufs=2, space="PSUM") as psB,
        ):
          for rep in range(reps):
            # ---- loads. HWDGE issue is a serialized ~630ns/DMA device, so
            # inputs use few big DMAs: wqk in 2 halves, x ch0 per-ct (fine
            # grain feeds the first chains), x ch1-3 as one [128,1536] DMA
            # per ct, wp as one DMA. consts/wv go via gpsimd SWDGE (bypasses
            # HWDGE entirely).
            qdma = [nc.sync, nc.scalar]
            wqk_sb = wqk_pool.tile([128, 8, 512], F16, tag="wqk")
            xtc0, xtcR = [], []
            for qtr in range(4):
                qdma[qtr % 2].dma_start(
                    wqk_sb[:, 2 * qtr:2 * qtr + 2, :],
                    wqk[:, 2 * qtr:2 * qtr + 2, :])
            for ct in range(8):
                t_ = big_pool.tile([128, 512], F16, tag="big0",
                                   name=f"x0_{ct}")
                qdma[ct % 2].dma_start(t_[:], xT[ct, :, 0:512])
                xtc0.append(t_)
            mask_sb = c_pool.tile([128, 128], F16, tag="mask")
            nc.gpsimd.dma_start(mask_sb[:], mask[:])
            id_sb = c_pool.tile([128, 128], F16, tag="id16")
            nc.gpsimd.dma_start(id_sb[:], id16[:])
            ones_sb = c_pool.tile([1, 128], F16, tag="ones")
            nc.gpsimd.dma_start(ones_sb[:], ones[:])
            onec_sb = c_pool.tile([128, 1], F16, tag="onec")
            nc.gpsimd.dma_start(onec_sb[:], onec[:])
            if v_bias:
                bv_sb = c_pool.tile([1, VW], F16, tag="bv")
                nc.gpsimd.dma_start(bv_sb[:], bv[:])
            if qk_bias:
                bqk_sb = c_pool.tile([128, 4], F32, tag="bqk")
                nc.gpsimd.dma_start(bqk_sb[:], bqk[:])
            wv_sb = []
            for ct in range(8):
                t_ = wv_pool.tile([128, VW], F16, tag="wv", name=f"wv{ct}")
                nc.gpsimd.dma_start(t_[:], wv[ct * 128:(ct + 1) * 128, :])
                wv_sb.append(t_)
            for ct in range(8):
                t_ = big_pool.tile([128, 1536], F16, tag="bigR",
                                   name=f"xR_{ct}")
                qdma[ct % 2].dma_start(t_[:], xT[ct, :, 512:2048])
                xtcR.append(t_)
            wp_sb = wp_pool.tile([128, 2, 1024], F16, tag="wp")
            nc.sync.dma_start(wp_sb[:], wp[:])

            def xs(ct, ch, c0, c1):
                if ch == 0:
                    return xtc0[ct][:, c0:c1]
                base = (ch - 1) * 512
                return xtcR[ct][:, base + c0:base + c1]

            q8 = qk8_pool.tile([128, 2, T], F8, tag="qk8", name="q8")
            k8 = qk8_pool.tile([128, 2, T], F8, tag="qk8", name="k8")
            yT_sb = [
                yT_pool.tile([128, T], F16, tag="yT", name=f"yT{m}")
                for m in range(2)
            ]
            v_sb = [None] * 16

            # ---- emitters ----------------------------------------------
            def qk_parts(ch, jt):
                """qk projection block as two half-chains (finer slots).
                psum rows = 4 heads x 32 hd-half."""
                cell = []

                def half(h0):
                    if h0 == 0:
                        cell.append(psB.tile([128, 512], F32, tag="mm",
                                             name=f"qk{ch}_{jt}"))
                    ps = cell[0]
                    for ct in range(h0, h0 + 4):
                        nc.tensor.matmul(
                            ps[:, 0:512],
                            wqk_sb[:, ct:ct + 1, jt * 128:(jt + 1) * 128],
                            xs(ct, ch, 0, 512),
                            start=(ct == 0),
                            stop=(ct == 7),
                        )
                    if h0 == 0:
                        return
                    dst_t = q8 if jt < 2 else k8
                    dst = dst_t[:, jt % 2:jt % 2 + 1, ch * 512:(ch + 1) * 512]
                    if qk_bias:
                        nc.vector.tensor_scalar_add(dst, ps[:, 0:512],
                                                    bqk_sb[:, jt:jt + 1])
                    elif ch == 0 and jt % 2 == 1:
                        nc.scalar.copy(dst, ps[:, 0:512])
                    elif ch in (1, 2) and jt % 2 == 1:
                        nc.scalar.copy(dst, ps[:, 0:512])
                    else:
                        nc.vector.tensor_copy(dst, ps[:, 0:512])

                return [lambda: half(0), lambda: half(4)]

            def emit_qk_jt(ch, jt):
                for p in qk_parts(ch, jt):
                    p()

            def v_parts(ch, tt):
                """v_aug projection block as two half-chains."""
                cell = []
                xblk = (tt % 4) * 128

                vwid = VW if v_bias else 256

                def half(h0):
                    if h0 == 0:
                        cell.append(psB.tile([128, 512], F32, tag="mm",
                                             name=f"v{tt}"))
                        if v_bias:
                            nc.tensor.matmul(
                                cell[0][:, 0:VW],
                                ones_sb[0:1, 0:128],
                                bv_sb[0:1, 0:VW],
                                start=True,
                                stop=False,
                            )
                    ps = cell[0]
                    for ct in range(h0, h0 + 4):
                        nc.tensor.matmul(
                            ps[:, 0:vwid],
                            xs(ct, ch, xblk, xblk + 128),
                            wv_sb[ct][:, 0:vwid],
                            start=(not v_bias and ct == 0),
                            stop=(ct == 7),
                            skip_group_check=True,
                        )
                    if h0 == 0:
                        return
                    t_ = v_pool.tile([128, vwid], F16, tag="v", name=f"v{tt}")
                    # tt==0 runs in the prologue where ACT is still idle
                    if tt == 0:
                        nc.scalar.copy(t_[:], ps[:, 0:vwid])
                    else:
                        nc.vector.tensor_copy(t_[:], ps[:, 0:vwid])
                    v_sb[tt] = t_

                return [lambda: half(0), lambda: half(4)]

            def emit_v_tt(ch, tt):
                for p in v_parts(ch, tt):
                    p()

            o15 = [None]

            def emit_proj_nch(tt, nch):
                ps = psB.tile([128, 512], F32, tag="mm", name=f"pj{tt}_{nch}")
                for mt in range(2):
                    nc.tensor.matmul(
                        ps[:, 0:512],
                        yT_sb[mt][:, tt * 128:(tt + 1) * 128],
                        wp_sb[:, mt:mt + 1, nch * 512:(nch + 1) * 512],
                        start=(mt == 0),
                        stop=(mt == 1),
                    )
                if tt == 15:
                    # final tile: one [128,1024] DMA, copies split ACT/DVE in
                    # parallel to shorten the drain chain
                    if nch == 0:
                        o15[0] = o_pool.tile([128, 1024], F16, tag="o15", name="o15")
                        nc.vector.tensor_copy(o15[0][:, 0:512], ps[:, 0:512])
                    else:
                        nc.scalar.copy(o15[0][:, 512:1024], ps[:, 0:512])
                        nc.sync.dma_start(out[15 * 128:, :], o15[0][:])
                    return
                o = o_pool.tile([128, 512], F16, tag="o", name=f"o{tt}_{nch}")
                # tt==14 is emitted after the last exp: ACT is idle there and
                # this skips the DVE tail backlog
                if tt >= 14:
                    nc.scalar.copy(o[:], ps[:, 0:512])
                else:
                    nc.vector.tensor_copy(o[:], ps[:, 0:512])
                nc.sync.dma_start(
                    out[tt * 128:(tt + 1) * 128, nch * 512:(nch + 1) * 512],
                    o[:],
                )

            def pj(tt):
                return [lambda: emit_proj_nch(tt, 0),
                        lambda: emit_proj_nch(tt, 1)]

            # ---- attention chunk ---------------------------------------
            def attn_qc(hp, qc, slots=(), post=()):
                """post[qs]: thunks emitted right after finish_qs(qs) —
                for work that depends on this chunk's own yT writes."""
                slots = list(slots)
                post = {i: list(p) for i, p in enumerate(post)}
                q0 = qc * 512
                njt = 4 * qc + 4
                av = [
                    psV.tile([128, VW], F32, tag="av", name=f"av{hp}{qc}a"),
                    psV.tile([128, VW], F32, tag="av", name=f"av{hp}{qc}b"),
                ]
                started = [False, False]
                pend, trail, tpend = [], 2, []
                n_slots, n_popped = len(slots), 0

                def emit_av(jp, pp):
                    for qs in range(max(0, jp - 4 * qc), 4):
                        ti = qs // 2
                        st = not started[ti]
                        started[ti] = True
                        for hl in range(2):
                            g = 2 * (qs % 2) + hl
                            h = 2 * hp + hl
                            stat = pp[:, hl * 512 + qs * 128:
                                      hl * 512 + qs * 128 + 128]
                            stop = jp == 4 * qc + qs
                            if v_bias:
                                nc.tensor.matmul(
                                    av[ti][:, 65 * g:65 * g + 65],
                                    stat,
                                    v_sb[jp][:, 65 * h:65 * h + 65],
                                    start=(st and hl == 0),
                                    stop=stop,
                                    skip_group_check=True,
                                )
                            else:
                                nc.tensor.matmul(
                                    av[ti][:, 64 * g:64 * g + 64],
                                    stat,
                                    v_sb[jp][:, 64 * h:64 * h + 64],
                                    start=(st and hl == 0),
                                    stop=stop,
                                    skip_group_check=True,
                                )
                                # D accumulates via a 1-col matmul sharing
                                # the stationary p (ldweights skipped)
                                nc.tensor.matmul(
                                    av[ti][:, 256 + g:257 + g],
                                    stat,
                                    onec_sb[:, 0:1],
                                    start=False,
                                    stop=stop,
                                    skip_group_check=True,
                                )

                def finish_qs(qs):
                    """Normalize (recip + scale) now; return a thunk doing
                    the PE transpose + yT copy, deferred so the PE never
                    waits on this DVE chain."""
                    ti, g0 = qs // 2, 2 * (qs % 2)
                    rs = r_pool.tile([128, 2], F32, tag="r",
                                     name=f"r{hp}_{qc}_{qs}")
                    yn = yn_pool.tile([128, 128], F16, tag="yn",
                                      name=f"yn{hp}_{qc}_{qs}")
                    # the very last finishes run after the final exp: use the
                    # idle ACT engine instead of the backlogged DVE
                    on_act = hp == 1 and qc == 3 and qs >= 2
                    for hl in range(2):
                        if v_bias:
                            c0, cd = 65 * (g0 + hl), 65 * (g0 + hl) + 64
                        else:
                            c0, cd = 64 * (g0 + hl), 256 + g0 + hl
                        nc.vector.reciprocal(rs[:, hl:hl + 1],
                                             av[ti][:, cd:cd + 1])
                        if on_act:
                            nc.scalar.activation(
                                yn[:, 64 * hl:64 * hl + 64],
                                av[ti][:, c0:c0 + 64],
                                AF.Copy, scale=rs[:, hl:hl + 1])
                        else:
                            nc.vector.tensor_scalar_mul(
                                yn[:, 64 * hl:64 * hl + 64],
                                av[ti][:, c0:c0 + 64],
                                rs[:, hl:hl + 1],
                            )

                    def transp():
                        tp = psB.tile([128, 128], F16, tag="mm",
                                      name=f"tp{hp}_{qc}_{qs}")
                        nc.tensor.transpose(tp[:, 0:128], yn[:, 0:128],
                                            id_sb[:, 0:128])
                        dst = yT_sb[hp][:, q0 + qs * 128:q0 + qs * 128 + 128]
                        if on_act:
                            nc.scalar.copy(dst, tp[:, 0:128])
                        else:
                            nc.vector.tensor_copy(dst, tp[:, 0:128])

                    return transp

                for jt in range(njt):
                    L = max(0, jt * 128 - q0)
                    s_ps = psA.tile([128, 1024], F32, tag="A", name="s_ps")
                    for hl in range(2):
                        h = 2 * hp + hl
                        nc.tensor.matmul(
                            s_ps[:, hl * 512 + L:(hl + 1) * 512],
                            k8[32 * h:32 * h + 32, :,
                               jt * 128:(jt + 1) * 128],
                            q8[32 * h:32 * h + 32, :, q0 + L:q0 + 512],
                            start=True,
                            stop=True,
                            perf_mode=DR,
                            tile_position=(32 * h, 0),
                        )
                    p_sb = p_pool.tile([128, 1024], F16, tag="p", name="p_sb")
                    nc.scalar.activation(
                        p_sb[:, L:1024], s_ps[:, L:1024], AF.Exp, scale=SCALE
                    )
                    if jt >= 4 * qc:
                        # DVE (not Pool): the mask sits on the exp->AV
                        # critical path; Pool's Q7 launch + 0.42x multiply
                        # would add ~400ns to every diagonal block.
                        nc.vector.tensor_mul(
                            p_sb[:, L:L + 128], p_sb[:, L:L + 128], mask_sb[:]
                        )
                        nc.vector.tensor_mul(
                            p_sb[:, 512 + L:512 + L + 128],
                            p_sb[:, 512 + L:512 + L + 128],
                            mask_sb[:],
                        )
                    pend.append((jt, p_sb))
                    if len(pend) > trail:
                        jp, pp = pend.pop(0)
                        emit_av(jp, pp)
                        if jp >= 4 * qc:
                            qs_done = jp - 4 * qc
                            tpend.append((qs_done, finish_qs(qs_done)))
                    # slots (PE filler) run between the normalize (DVE) and
                    # the transpose that consumes it, hiding that latency
                    while slots and n_popped < (jt + 1) * n_slots / njt:
                        slots.pop(0)()
                        n_popped += 1
                    if len(tpend) > 0:
                        qs_done, th = tpend.pop(0)
                        th()
                        for s in post.pop(qs_done, ()):
                            s()
                for jp, pp in pend:
                    emit_av(jp, pp)
                    if jp >= 4 * qc:
                        tpend.append((jp - 4 * qc, finish_qs(jp - 4 * qc)))
                for qs_done, th in tpend:
                    th()
                    for s in post.pop(qs_done, ()):
                        s()
                for s in slots:
                    s()
                for qs_done in sorted(post):
                    for s in post[qs_done]:
                        s()

            # ---- schedule ----------------------------------------------
            def qk_u(ch, jt):
                return qk_parts(ch, jt)

            def v_u(ch, tt):
                return v_parts(ch, tt)

            # prologue: qk blocks of ch0 + v0 (A0's first AV needs it)
            for jt in range(4):
                emit_qk_jt(0, jt)
            emit_v_tt(0, 0)
            attn_qc(0, 0, v_u(0, 1) + v_u(0, 2) + v_u(0, 3))
            # B0 carries the ch1 qk blocks (paced by the x ch1-3 DMAs)
            attn_qc(1, 0, qk_u(1, 0) + qk_u(1, 1) + qk_u(1, 2) + qk_u(1, 3))
            attn_qc(0, 1, v_u(1, 4) + v_u(1, 5) + v_u(1, 6) + v_u(1, 7))
            attn_qc(1, 1, qk_u(2, 0) + qk_u(2, 1) + qk_u(2, 2) + qk_u(2, 3))
            attn_qc(0, 2, v_u(2, 8) + v_u(2, 9) + v_u(2, 10) + v_u(2, 11)
                    + pj(0) + pj(1))
            attn_qc(1, 2, qk_u(3, 0) + qk_u(3, 1) + qk_u(3, 2) + qk_u(3, 3)
                    + pj(2) + pj(3))
            attn_qc(0, 3, v_u(3, 12) + v_u(3, 13) + v_u(3, 14) + v_u(3, 15)
                    + pj(4) + pj(5) + pj(6) + pj(7))
            # pj(12..15) read yT columns B3 itself writes: emit each right
            # after B3's finish_qs for that query sub-block.
            attn_qc(1, 3, pj(8) + pj(9) + pj(10) + pj(11),
                    post=[pj(12), pj(13), pj(14), pj(15)])

    if not nc.is_finalized():
        nc.finalize()
    return nc


def host_prep(x, W_attn, b_attn, W_proj):
    v_bias = bool(np.any(np.asarray(b_attn, np.float32)[2 * C:]))
    bf = np.float16
    x = np.ascontiguousarray(np.asarray(x, np.float32))
    W_attn = np.ascontiguousarray(np.asarray(W_attn, np.float32))
    b_attn = np.ascontiguousarray(np.asarray(b_attn, np.float32))
    W_proj = np.ascontiguousarray(np.asarray(W_proj, np.float32))
    mask = np.triu(np.ones((128, 128), np.float32)).astype(bf)
    ones = np.ones((1, 128), bf)
    id16 = np.eye(128, dtype=bf)
    onec_ = np.ones((128, 1), bf)
    per_group = []
    for hg in range(NG):
        heads = [hg * HPG + i for i in range(HPG)]
        # wqk column blocks jt: 0=q hd-lo, 1=q hd-hi, 2=k hd-lo, 3=k hd-hi;
        # within a block, 4 heads x 32 (head-major)
        cols, bias = [], []
        for base in (0, C):  # q then k
            for half in (0, 32):
                for h in heads:
                    c0 = base + h * HD + half
                    cols.append(W_attn[:, c0:c0 + 32])
                    bias.append(b_attn[c0:c0 + 32])
        # [C, 512] -> [128, 8, 512]: partition p, ct-slab, col
        wqk_ = np.ascontiguousarray(
            np.concatenate(cols, axis=1).astype(bf)
            .reshape(8, 128, 512).transpose(1, 0, 2))
        bqk_ = np.ascontiguousarray(
            np.concatenate(bias).reshape(4, 128).T.astype(np.float32))
        wv_ = np.zeros((C, VW), np.float32)
        bv_ = np.zeros((1, VW), np.float32)
        vb = 65 if v_bias else 64
        for i, h in enumerate(heads):
            wv_[:, vb * i:vb * i + 64] = \
                W_attn[:, 2 * C + h * HD:2 * C + (h + 1) * HD]
            if v_bias:
                bv_[0, 65 * i:65 * i + 64] = \
                    b_attn[2 * C + h * HD:2 * C + (h + 1) * HD]
                bv_[0, 65 * i + 64] = 1.0
        # [256, 1024] -> [128, 2, 1024]
        wp_ = np.ascontiguousarray(
            np.concatenate([W_proj[h * HD:(h + 1) * HD, :] for h in heads],
                           axis=0).astype(bf)
            .reshape(2, 128, 1024).transpose(1, 0, 2))
        per_group.append((wqk_, bqk_, wv_.astype(bf), bv_.astype(bf), wp_))
    in_maps = []
    for b in range(B):
        xT_b = np.ascontiguousarray(
            x[b].T.astype(bf).reshape(8, 128, T))
        for hg in range(NG):
            wqk_, bqk_, wv_, bv_, wp_ = per_group[hg]
            in_maps.append(
                dict(xT=xT_b, wqk=wqk_, bqk=bqk_, wv=wv_, bv=bv_, wp=wp_,
                     mask=mask, ones=ones, id16=id16, onec=onec_)
            )
    return in_maps


_prog_cache = {}


def _get_program(qk_bias=False, v_bias=False):
    key = ("nc", qk_bias, v_bias)
    if key not in _prog_cache:
        _prog_cache[key] = build_program(qk_bias=qk_bias, v_bias=v_bias)
    return _prog_cache[key]


def run_cores(in_maps, trace=False, qk_bias=False, v_bias=False, **kw):
    return run_bass_kernel_spmd(
        _get_program(qk_bias, v_bias), in_maps, list(range(NCORES)),
        trace=trace, **kw
    )


def kernel(x, W_attn, b_attn, W_proj, b_proj):
    in_maps = host_prep(x, W_attn, b_attn, W_proj)
    b_attn_f = np.asarray(b_attn, np.float32)
    qk_bias = bool(np.any(b_attn_f[: 2 * C]))
    v_bias = bool(np.any(b_attn_f[2 * C:]))
    br = run_cores(in_maps, qk_bias=qk_bias, v_bias=v_bias)
    b_proj = np.asarray(b_proj, np.float32)
    y = np.zeros((B, T, C), np.float32)
    for b in range(B):
        acc = np.zeros((T, C), np.float32)
        for hg in range(NG):
            acc += np.asarray(br.results[b * NG + hg]["out"])
        y[b] = acc + b_proj[None, :]
    return y


# revision 66
# speedup vs baseline: 1.4833x; 1.0531x over previous
"""Causal self-attention (B=2, T=2048, C=1024, H=16) on 8 TRN2 NeuronCores.

Sharding: core = b*4 + hg (data parallel over batch, tensor parallel over
4 head-groups of 4 heads). Each core computes its head-group's attention and
a partial output projection; the host sums the 4 partials per batch and adds
b_proj.

Per-core device program (v4 — fp8 DoubleRow scores + moving-v AV;
TimelineSim 130.4us vs the 183.6us v3 baseline):
  - qk projection writes q8/k8 as fp8e4 [128, 2, T] tiles: partitions
    32h..32h+31 hold head h, slab i = head-dim half i. wqk's column order is
    permuted on the host so each [128,512] PSUM block lands with ONE copy.
  - scores use fp8 DoubleRow matmuls (0.5 cyc/row in the cost model): per
    (head, key-block) one matmul, stationary k8 [32,2,128], moving q8
    [32,2,512-L]. Quantization error ~1% total vs the 2e-2 gate.
  - AV is restructured: stationary p [128 keys, 128 q], moving v [128, 64]
    per (key-block, q-subblock, head) — 64 moving cols beat the old
    512-wide moving-p form ~2x. A 1-col matmul vs a ones vector (same
    stationary, ldweights-free) accumulates the softmax denominator D per
    query ON the query partition, so normalization is a per-partition
    tensor_scalar (no PE broadcast matmuls). With v_bias, v blocks carry a
    65th ones column initialized by a bias matmul instead.
  - y [q, hd] is normalized via reciprocal+tensor_scalar then PE-transposed
    (fp16, via identity) into yT for the projection. wp/yT are fp16.
  - ACT runs exps (the pacing engine, ~80us) plus prologue/tail copies;
    everything else copies on DVE. Masks multiply on DVE (Pool's Q7
    launch would sit on the exp->AV critical path). HWDGE issue is a
    serialized ~630ns/DMA device, so inputs load as few big DMAs with a
    host-side relayout; consts/wv go via gpsimd SWDGE.
  - PSUM: scores 2x[128,1024] (4 banks) + AV pool 2x[128,260] (4 groups of
    64 + 4 D cols each, pending-zero init, 1 bank each) + shared [128,512]
    pool for qkv/proj/transpose (2 banks) = 8 banks.
  - Schedule: attention chunk (hp, qc) consumes qkv column-block ch=qc;
    qkv half-chains and projection tiles are paced into the ACT-bound jt
    loops as ~0.5-0.9us slots (popped between the scores and the trailing
    AV, which runs 2 key-blocks behind its exp); per-qs finish chains
    (recip/normalize -> transpose -> yT) emit at each diagonal, and the
    last 4 projection tiles hang off B3's own finishes via post-hooks.
"""

import math

import numpy as np

import concourse.bass as bass
import concourse.bacc as bacc
import concourse.mybir as mybir
from concourse import tile
from concourse.bass_utils import run_bass_kernel_spmd

B, T, C, H = 2, 2048, 1024, 16
HD = C // H   # 64
HPG = 4       # heads per group
NG = 4        # head groups
NCORES = 8
VW = 260      # v_aug width: 4 heads x (64 v + 1 ones)

F32 = mybir.dt.float32
F16 = mybir.dt.float16
F8 = mybir.dt.float8e4
AF = mybir.ActivationFunctionType
DR = mybir.MatmulPerfMode.DoubleRow
SCALE = 1.0 / math.sqrt(C)  # 1/32


def build_program(reps=1, qk_bias=False, v_bias=False):
    nc = bacc.Bacc()

    xT = nc.dram_tensor("xT", [8, 128, T], F16, kind="ExternalInput")
    wqk = nc.dram_tensor("wqk", [128, 8, 512], F16, kind="ExternalInput")
    bqk = nc.dram_tensor("bqk", [128, 4], F32, kind="ExternalInput")
    wv = nc.dram_tensor("wv", [C, VW], F16, kind="ExternalInput")
    bv = nc.dram_tensor("bv", [1, VW], F16, kind="ExternalInput")
    wp = nc.dram_tensor("wp", [128, 2, 1024], F16, kind="ExternalInput")
    mask = nc.dram_tensor("mask", [128, 128], F16, kind="ExternalInput")
    ones = nc.dram_tensor("ones", [1, 128], F16, kind="ExternalInput")
    id16 = nc.dram_tensor("id16", [128, 128], F16, kind="ExternalInput")
    onec = nc.dram_tensor("onec", [128, 1], F16, kind="ExternalInput")
    out = nc.dram_tensor("out", [T, C], F16, kind="ExternalOutput")

    with tile.TileContext(nc) as tc:
        with (
            tc.tile_pool(name="big", bufs=8) as big_pool,
            tc.tile_pool(name="wqk", bufs=1) as wqk_pool,
            tc.tile_pool(name="wv", bufs=8) as wv_pool,
            tc.tile_pool(name="wp", bufs=1) as wp_pool,
            tc.tile_pool(name="qk8", bufs=2) as qk8_pool,
            tc.tile_pool(name="pp", bufs=6) as p_pool,
            tc.tile_pool(name="vsb", bufs=16) as v_pool,
            tc.tile_pool(name="yT", bufs=2) as yT_pool,
            tc.tile_pool(name="yn", bufs=4) as yn_pool,
            tc.tile_pool(name="rr", bufs=3) as r_pool,
            tc.tile_pool(name="osb", bufs=4) as o_pool,
            tc.tile_pool(name="consts", bufs=1) as c_pool,
            tc.tile_pool(name="psA", bufs=2, space="PSUM") as psA,
            tc.tile_pool(name="psV", bufs=2, space="PSUM") as psV,
            tc.tile_pool(name="psB", bufs=2, space="PSUM") as psB,
        ):
          for rep in range(reps):
            # ---- loads. HWDGE issue is a serialized ~630ns/DMA device, so
            # inputs use few big DMAs: wqk in 2 halves, x ch0 per-ct (fine
            # grain feeds the first chains), x ch1-3 as one [128,1536] DMA
            # per ct, wp as one DMA. consts/wv go via gpsimd SWDGE (bypasses
            # HWDGE entirely).
            qdma = [nc.sync, nc.scalar]
            wqk_sb = wqk_pool.tile([128, 8, 512], F16, tag="wqk")
            xtc0, xtcR = [], []
            for qtr in range(4):
                qdma[qtr % 2].dma_start(
                    wqk_sb[:, 2 * qtr:2 * qtr + 2, :],
                    wqk[:, 2 * qtr:2 * qtr + 2, :])
            for ct in range(8):
                t_ = big_pool.tile([128, 512], F16, tag="big0",
                                   name=f"x0_{ct}")
                qdma[ct % 2].dma_start(t_[:], xT[ct, :, 0:512])
                xtc0.append(t_)
            mask_sb = c_pool.tile([128, 128], F16, tag="mask")
            nc.gpsimd.dma_start(mask_sb[:], mask[:])
            id_sb = c_pool.tile([128, 128], F16, tag="id16")
            nc.gpsimd.dma_start(id_sb[:], id16[:])
            ones_sb = c_pool.tile([1, 128], F16, tag="ones")
            nc.gpsimd.dma_start(ones_sb[:], ones[:])
            onec_sb = c_pool.tile([128, 1], F16, tag="onec")
            nc.gpsimd.dma_start(onec_sb[:], onec[:])
            if v_bias:
                bv_sb = c_pool.tile([1, VW], F16, tag="bv")
                nc.gpsimd.dma_start(bv_sb[:], bv[:])
            if qk_bias:
                bqk_sb = c_pool.tile([128, 4], F32, tag="bqk")
                nc.gpsimd.dma_start(bqk_sb[:], bqk[:])
            wv_sb = []
            for ct in range(8):
                t_ = wv_pool.tile([128, VW], F16, tag="wv", name=f"wv{ct}")
                nc.gpsimd.dma_start(t_[:], wv[ct * 128:(ct + 1) * 128, :])
                wv_sb.append(t_)
            for ct in range(8):
                t_ = big_pool.tile([128, 1536], F16, tag="bigR",
                                   name=f"xR_{ct}")
                qdma[ct % 2].dma_start(t_[:], xT[ct, :, 512:2048])
                xtcR.append(t_)
            wp_sb = wp_pool.tile([128, 2, 1024], F16, tag="wp")
            nc.sync.dma_start(wp_sb[:], wp[:])

            def xs(ct, ch, c0, c1):
                if ch == 0:
                    return xtc0[ct][:, c0:c1]
                base = (ch - 1) * 512
                return xtcR[ct][:, base + c0:base + c1]

            q8 = qk8_pool.tile([128, 2, T], F8, tag="qk8", name="q8")
            k8 = qk8_pool.tile([128, 2, T], F8, tag="qk8", name="k8")
            yT_sb = [
                yT_pool.tile([128, T], F16, tag="yT", name=f"yT{m}")
                for m in range(2)
            ]
            v_sb = [None] * 16

            # ---- emitters ----------------------------------------------
            def qk_parts(ch, jt):
                """qk projection block as two half-chains (finer slots).
                psum rows = 4 heads x 32 hd-half."""
                cell = []

                def half(h0):
                    if h0 == 0:
                        cell.append(psB.tile([128, 512], F32, tag="mm",
                                             name=f"qk{ch}_{jt}"))
                    ps = cell[0]
                    for ct in range(h0, h0 + 4):
                        nc.tensor.matmul(
                            ps[:, 0:512],
                            wqk_sb[:, ct:ct + 1, jt * 128:(jt + 1) * 128],
                            xs(ct, ch, 0, 512),
                            start=(ct == 0),
                            stop=(ct == 7),
                        )
                    if h0 == 0:
                        return
                    dst_t = q8 if jt < 2 else k8
                    dst = dst_t[:, jt % 2:jt % 2 + 1, ch * 512:(ch + 1) * 512]
                    if qk_bias:
                        nc.vector.tensor_scalar_add(dst, ps[:, 0:512],
                                                    bqk_sb[:, jt:jt + 1])
                    elif ch == 0 and jt % 2 == 1:
                        nc.scalar.copy(dst, ps[:, 0:512])
                    elif ch in (1, 2) and jt % 2 == 1:
                        nc.scalar.copy(dst, ps[:, 0:512])
                    else:
                        nc.vector.tensor_copy(dst, ps[:, 0:512])

                return [lambda: half(0), lambda: half(4)]

            def emit_qk_jt(ch, jt):
                for p in qk_parts(ch, jt):
                    p()

            def v_parts(ch, tt):
                """v_aug projection block as two half-chains."""
                cell = []
                xblk = (tt % 4) * 128

                vwid = VW if v_bias else 256

                def half(h0):
                    if h0 == 0:
                        cell.append(psB.tile([128, 512], F32, tag="mm",
                                             name=f"v{tt}"))
                        if v_bias:
                            nc.tensor.matmul(
                                cell[0][:, 0:VW],
                                ones_sb[0:1, 0:128],
                                bv_sb[0:1, 0:VW],
                                start=True,
                                stop=False,
                            )
                    ps = cell[0]
                    for ct in range(h0, h0 + 4):
                        nc.tensor.matmul(
                            ps[:, 0:vwid],
                            xs(ct, ch, xblk, xblk + 128),
                            wv_sb[ct][:, 0:vwid],
                            start=(not v_bias and ct == 0),
                            stop=(ct == 7),
                            skip_group_check=True,
                        )
                    if h0 == 0:
                        return
                    t_ = v_pool.tile([128, vwid], F16, tag="v", name=f"v{tt}")
                    # tt==0 runs in the prologue where ACT is still idle
                    if tt == 0:
                        nc.scalar.copy(t_[:], ps[:, 0:vwid])
                    else:
                        nc.vector.tensor_copy(t_[:], ps[:, 0:vwid])
                    v_sb[tt] = t_

                return [lambda: half(0), lambda: half(4)]

            def emit_v_tt(ch, tt):
                for p in v_parts(ch, tt):
                    p()

            o15 = [None]

            def emit_proj_nch(tt, nch):
                ps = psB.tile([128, 512], F32, tag="mm", name=f"pj{tt}_{nch}")
                for mt in range(2):
                    nc.tensor.matmul(
                        ps[:, 0:512],
                        yT_sb[mt][:, tt * 128:(tt + 1) * 128],
                        wp_sb[:, mt:mt + 1, nch * 512:(nch + 1) * 512],
                        start=(mt == 0),
                        stop=(mt == 1),
                    )
                if tt == 15:
                    # final tile: one [128,1024] DMA, copies split ACT/DVE in
                    # parallel to shorten the drain chain
                    if nch == 0:
                        o15[0] = o_pool.tile([128, 1024], F16, tag="o15", name="o15")
                        nc.vector.tensor_copy(o15[0][:, 0:512], ps[:, 0:512])
                    else:
                        nc.scalar.copy(o15[0][:, 512:1024], ps[:, 0:512])
                        nc.sync.dma_start(out[15 * 128:, :], o15[0][:])
                    return
                o = o_pool.tile([128, 512], F16, tag="o", name=f"o{tt}_{nch}")
                # tt==14 is emitted after the last exp: ACT is idle there and
                # this skips the DVE tail backlog
                if tt >= 14:
                    nc.scalar.copy(o[:], ps[:, 0:512])
                else:
                    nc.vector.tensor_copy(o[:], ps[:, 0:512])
                nc.sync.dma_start(
                    out[tt * 128:(tt + 1) * 128, nch * 512:(nch + 1) * 512],
                    o[:],
                )

            def pj(tt):
                return [lambda: emit_proj_nch(tt, 0),
                        lambda: emit_proj_nch(tt, 1)]

            # ---- attention chunk ---------------------------------------
            def attn_qc(hp, qc, slots=(), post=()):
                """post[qs]: thunks emitted right after finish_qs(qs) —
                for work that depends on this chunk's own yT writes."""
                slots = list(slots)
                post = {i: list(p) for i, p in enumerate(post)}
                q0 = qc * 512
                njt = 4 * qc + 4
                av = [
                    psV.tile([128, VW], F32, tag="av", name=f"av{hp}{qc}a"),
                    psV.tile([128, VW], F32, tag="av", name=f"av{hp}{qc}b"),
                ]
                started = [False, False]
                pend, trail, tpend = [], 2, []
                n_slots, n_popped = len(slots), 0

                def emit_av(jp, pp):
                    for qs in range(max(0, jp - 4 * qc), 4):
                        ti = qs // 2
                        st = not started[ti]
                        started[ti] = True
                        for hl in range(2):
                            g = 2 * (qs % 2) + hl
                            h = 2 * hp + hl
                            stat = pp[:, hl * 512 + qs * 128:
                                      hl * 512 + qs * 128 + 128]
                            stop = jp == 4 * qc + qs
                            if v_bias:
                                nc.tensor.matmul(
                                    av[ti][:, 65 * g:65 * g + 65],
                                    stat,
                                    v_sb[jp][:, 65 * h:65 * h + 65],
                                    start=(st and hl == 0),
                                    stop=stop,
                                    skip_group_check=True,
                                )
                            else:
                                nc.tensor.matmul(
                                    av[ti][:, 64 * g:64 * g + 64],
                                    stat,
                                    v_sb[jp][:, 64 * h:64 * h + 64],
                                    start=(st and hl == 0),
                                    stop=stop,
                                    skip_group_check=True,
                                )
                                # D accumulates via a 1-col matmul sharing
                                # the stationary p (ldweights skipped)
                                nc.tensor.matmul(
                                    av[ti][:, 256 + g:257 + g],
                                    stat,
                                    onec_sb[:, 0:1],
                                    start=False,
                                    stop=stop,
                                    skip_group_check=True,
                                )

                def finish_qs(qs):
                    """Normalize (recip + scale) now; return a thunk doing
                    the PE transpose + yT copy, deferred so the PE never
                    waits on this DVE chain."""
                    ti, g0 = qs // 2, 2 * (qs % 2)
                    rs = r_pool.tile([128, 2], F32, tag="r",
                                     name=f"r{hp}_{qc}_{qs}")
                    yn = yn_pool.tile([128, 128], F16, tag="yn",
                                      name=f"yn{hp}_{qc}_{qs}")
                    # the very last finishes run after the final exp: use the
                    # idle ACT engine instead of the backlogged DVE
                    on_act = hp == 1 and qc == 3 and qs >= 2
                    for hl in range(2):
                        if v_bias:
                            c0, cd = 65 * (g0 + hl), 65 * (g0 + hl) + 64
                        else:
                            c0, cd = 64 * (g0 + hl), 256 + g0 + hl
                        nc.vector.reciprocal(rs[:, hl:hl + 1],
                                             av[ti][:, cd:cd + 1])
                        if on_act:
                            nc.scalar.activation(
                                yn[:, 64 * hl:64 * hl + 64],
                                av[ti][:, c0:c0 + 64],
                                AF.Copy, scale=rs[:, hl:hl + 1])
                        else:
                            nc.vector.tensor_scalar_mul(
                                yn[:, 64 * hl:64 * hl + 64],
                                av[ti][:, c0:c0 + 64],
                                rs[:, hl:hl + 1],
                            )

                    def transp():
                        dst = yT_sb[hp][:, q0 + qs * 128:q0 + qs * 128 + 128]
                        if hp == 1 and qc == 3:
                            # tail-critical: PE transpose + engine copy is
                            # ~0.35us vs the ~2.5us DMA-transpose latency
                            tp = psB.tile([128, 128], F16, tag="mm",
                                          name=f"tp{hp}_{qc}_{qs}")
                            nc.tensor.transpose(tp[:, 0:128], yn[:, 0:128],
                                                id_sb[:, 0:128])
                            if on_act:
                                nc.scalar.copy(dst, tp[:, 0:128])
                            else:
                                nc.vector.tensor_copy(dst, tp[:, 0:128])
                        else:
                            # off the critical path: xbar DMA transpose frees
                            # the PE, DVE, and the psB rotation entirely
                            nc.sync.dma_start_transpose(dst, yn[:, 0:128])

                    return transp

                for jt in range(njt):
                    L = max(0, jt * 128 - q0)
                    s_ps = psA.tile([128, 1024], F32, tag="A", name="s_ps")
                    for hl in range(2):
                        h = 2 * hp + hl
                        nc.tensor.matmul(
                            s_ps[:, hl * 512 + L:(hl + 1) * 512],
                            k8[32 * h:32 * h + 32, :,
                               jt * 128:(jt + 1) * 128],
                            q8[32 * h:32 * h + 32, :, q0 + L:q0 + 512],
                            start=True,
                            stop=True,
                            perf_mode=DR,
                            tile_position=(32 * h, 0),
                        )
                    p_sb = p_pool.tile([128, 1024], F16, tag="p", name="p_sb")
                    nc.scalar.activation(
                        p_sb[:, L:1024], s_ps[:, L:1024], AF.Exp, scale=SCALE
                    )
                    if jt >= 4 * qc:
                        # DVE (not Pool): the mask sits on the exp->AV
                        # critical path; Pool's Q7 launch + 0.42x multiply
                        # would add ~400ns to every diagonal block.
                        nc.vector.tensor_mul(
                            p_sb[:, L:L + 128], p_sb[:, L:L + 128], mask_sb[:]
                        )
                        nc.vector.tensor_mul(
                            p_sb[:, 512 + L:512 + L + 128],
                            p_sb[:, 512 + L:512 + L + 128],
                            mask_sb[:],
                        )
                    pend.append((jt, p_sb))
                    if len(pend) > trail:
                        jp, pp = pend.pop(0)
                        emit_av(jp, pp)
                        if jp >= 4 * qc:
                            qs_done = jp - 4 * qc
                            tpend.append((qs_done, finish_qs(qs_done)))
                    # slots (PE filler) run between the normalize (DVE) and
                    # the transpose that consumes it, hiding that latency
                    while slots and n_popped < (jt + 1) * n_slots / njt:
                        slots.pop(0)()
                        n_popped += 1
                    if len(tpend) > 0:
                        qs_done, th = tpend.pop(0)
                        th()
                        for s in post.pop(qs_done, ()):
                            s()
                for jp, pp in pend:
                    emit_av(jp, pp)
                    if jp >= 4 * qc:
                        tpend.append((jp - 4 * qc, finish_qs(jp - 4 * qc)))
                for qs_done, th in tpend:
                    th()
                    for s in post.pop(qs_done, ()):
                        s()
                for s in slots:
                    s()
                for qs_done in sorted(post):
                    for s in post[qs_done]:
                        s()

            # ---- schedule ----------------------------------------------
            def qk_u(ch, jt):
                return qk_parts(ch, jt)

            def v_u(ch, tt):
                return v_parts(ch, tt)

            # prologue: qk blocks of ch0 + v0 (A0's first AV needs it)
            for jt in range(4):
                emit_qk_jt(0, jt)
            emit_v_tt(0, 0)
            attn_qc(0, 0, v_u(0, 1) + v_u(0, 2) + v_u(0, 3))
            # B0 carries the ch1 qk blocks (paced by the x ch1-3 DMAs)
            attn_qc(1, 0, qk_u(1, 0) + qk_u(1, 1) + qk_u(1, 2) + qk_u(1, 3))
            attn_qc(0, 1, v_u(1, 4) + v_u(1, 5) + v_u(1, 6) + v_u(1, 7))
            attn_qc(1, 1, qk_u(2, 0) + qk_u(2, 1) + qk_u(2, 2) + qk_u(2, 3))
            attn_qc(0, 2, v_u(2, 8) + v_u(2, 9) + v_u(2, 10) + v_u(2, 11)
                    + pj(0) + pj(1))
            attn_qc(1, 2, qk_u(3, 0) + qk_u(3, 1) + qk_u(3, 2) + qk_u(3, 3)
                    + pj(2) + pj(3))
            attn_qc(0, 3, v_u(3, 12) + v_u(3, 13) + v_u(3, 14) + v_u(3, 15)
                    + pj(4) + pj(5) + pj(6) + pj(7))
            # pj(12..15) read yT columns B3 itself writes: emit each right
            # after B3's finish_qs for that query sub-block.
            attn_qc(1, 3, pj(8) + pj(9) + pj(10) + pj(11),
                    post=[pj(12), pj(13), pj(14), pj(15)])

    if not nc.is_finalized():
        nc.finalize()
    return nc


def host_prep(x, W_attn, b_attn, W_proj):
    v_bias = bool(np.any(np.asarray(b_attn, np.float32)[2 * C:]))
    bf = np.float16
    x = np.ascontiguousarray(np.asarray(x, np.float32))
    W_attn = np.ascontiguousarray(np.asarray(W_attn, np.float32))
    b_attn = np.ascontiguousarray(np.asarray(b_attn, np.float32))
    W_proj = np.ascontiguousarray(np.asarray(W_proj, np.float32))
    mask = np.triu(np.ones((128, 128), np.float32)).astype(bf)
    ones = np.ones((1, 128), bf)
    id16 = np.eye(128, dtype=bf)
    onec_ = np.ones((128, 1), bf)
    per_group = []
    for hg in range(NG):
        heads = [hg * HPG + i for i in range(HPG)]
        # wqk column blocks jt: 0=q hd-lo, 1=q hd-hi, 2=k hd-lo, 3=k hd-hi;
        # within a block, 4 heads x 32 (head-major)
        cols, bias = [], []
        for base in (0, C):  # q then k
            for half in (0, 32):
                for h in heads:
                    c0 = base + h * HD + half
                    cols.append(W_attn[:, c0:c0 + 32])
                    bias.append(b_attn[c0:c0 + 32])
        # [C, 512] -> [128, 8, 512]: partition p, ct-slab, col
        wqk_ = np.ascontiguousarray(
            np.concatenate(cols, axis=1).astype(bf)
            .reshape(8, 128, 512).transpose(1, 0, 2))
        bqk_ = np.ascontiguousarray(
            np.concatenate(bias).reshape(4, 128).T.astype(np.float32))
        wv_ = np.zeros((C, VW), np.float32)
        bv_ = np.zeros((1, VW), np.float32)
        vb = 65 if v_bias else 64
        for i, h in enumerate(heads):
            wv_[:, vb * i:vb * i + 64] = \
                W_attn[:, 2 * C + h * HD:2 * C + (h + 1) * HD]
            if v_bias:
                bv_[0, 65 * i:65 * i + 64] = \
                    b_attn[2 * C + h * HD:2 * C + (h + 1) * HD]
                bv_[0, 65 * i + 64] = 1.0
        # [256, 1024] -> [128, 2, 1024]
        wp_ = np.ascontiguousarray(
            np.concatenate([W_proj[h * HD:(h + 1) * HD, :] for h in heads],
                           axis=0).astype(bf)
            .reshape(2, 128, 1024).transpose(1, 0, 2))
        per_group.append((wqk_, bqk_, wv_.astype(bf), bv_.astype(bf), wp_))
    in_maps = []
    for b in range(B):
        xT_b = np.ascontiguousarray(
            x[b].T.astype(bf).reshape(8, 128, T))
        for hg in range(NG):
            wqk_, bqk_, wv_, bv_, wp_ = per_group[hg]
            in_maps.append(
                dict(xT=xT_b, wqk=wqk_, bqk=bqk_, wv=wv_, bv=bv_, wp=wp_,
                     mask=mask, ones=ones, id16=id16, onec=onec_)
            )
    return in_maps


_prog_cache = {}


def _get_program(qk_bias=False, v_bias=False):
    key = ("nc", qk_bias, v_bias)
    if key not in _prog_cache:
        _prog_cache[key] = build_program(qk_bias=qk_bias, v_bias=v_bias)
    return _prog_cache[key]


def run_cores(in_maps, trace=False, qk_bias=False, v_bias=False, **kw):
    return run_bass_kernel_spmd(
        _get_program(qk_bias, v_bias), in_maps, list(range(NCORES)),
        trace=trace, **kw
    )


def kernel(x, W_attn, b_attn, W_proj, b_proj):
    in_maps = host_prep(x, W_attn, b_attn, W_proj)
    b_attn_f = np.asarray(b_attn, np.float32)
    qk_bias = bool(np.any(b_attn_f[: 2 * C]))
    v_bias = bool(np.any(b_attn_f[2 * C:]))
    br = run_cores(in_maps, qk_bias=qk_bias, v_bias=v_bias)
    b_proj = np.asarray(b_proj, np.float32)
    y = np.zeros((B, T, C), np.float32)
    for b in range(B):
        acc = np.zeros((T, C), np.float32)
        for hg in range(NG):
            acc += np.asarray(br.results[b * NG + hg]["out"])
        y[b] = acc + b_proj[None, :]
    return y


# revision 78
# speedup vs baseline: 1.6193x; 1.0917x over previous
"""Causal self-attention (B=2, T=2048, C=1024, H=16) on 8 TRN2 NeuronCores.

Sharding: core = b*4 + hg (data parallel over batch, tensor parallel over
4 head-groups of 4 heads). Each core computes its head-group's attention and
a partial output projection; the host sums the 4 partials per batch and adds
b_proj.

Per-core device program (v4 — fp8 DoubleRow scores + moving-v AV;
TimelineSim 113.4us vs the 183.6us v3 baseline):
  - qk projection writes q8/k8 as fp8e4 [128, 2, T] tiles: partitions
    32h..32h+31 hold head h, slab i = head-dim half i. wqk's column order is
    permuted on the host so each [128,512] PSUM block lands with ONE copy.
  - scores use fp8 DoubleRow matmuls (0.5 cyc/row in the cost model): per
    (head, key-block) one matmul, stationary k8 [32,2,128], moving q8
    [32,2,512-L]. Quantization error ~1% total vs the 2e-2 gate.
  - AV is restructured: stationary p [128 keys, 128 q], moving v [128, 64]
    per (key-block, q-subblock, head) — 64 moving cols beat the old
    512-wide moving-p form ~2x. A 1-col matmul vs a ones vector (same
    stationary, ldweights-free) accumulates the softmax denominator D per
    query ON the query partition, so normalization is a per-partition
    tensor_scalar (no PE broadcast matmuls). With v_bias, v blocks carry a
    65th ones column initialized by a bias matmul instead.
  - y [q, hd] is normalized via reciprocal+tensor_scalar then transposed
    into yT for the projection (wp/yT fp16). 28 of 32 transposes go through
    the xbar DMA-transpose (frees PE/DVE and, critically, the psB rotation);
    only B3's four tail-critical ones use the PE+identity path.
  - ACT runs exps (the pacing engine, ~80us) plus prologue/tail copies;
    everything else copies on DVE. Masks multiply on DVE (Pool's Q7
    launch would sit on the exp->AV critical path). HWDGE issue is a
    serialized ~630ns/DMA device, so inputs load as few big DMAs with a
    host-side relayout; consts/wv go via gpsimd SWDGE.
  - PSUM: scores 2x[128,1024] (4 banks) + AV pool 2x[128,260] (4 groups of
    64 + 4 D cols each, pending-zero init, 1 bank each) + shared [128,512]
    pool for qkv/proj/transpose (2 banks) = 8 banks.
  - Schedule: attention chunk (hp, qc) consumes qkv column-block ch=qc;
    qkv half-chains and projection tiles are paced into the ACT-bound jt
    loops as ~0.5-0.9us slots (popped between the scores and the trailing
    AV, which runs 2 key-blocks behind its exp); per-qs finish chains
    (recip/normalize -> transpose -> yT) emit at each diagonal, and the
    last 4 projection tiles hang off B3's own finishes via post-hooks.
"""

import math

import numpy as np

import concourse.bass as bass
import concourse.bacc as bacc
import concourse.mybir as mybir
from concourse import tile
from concourse.bass_utils import run_bass_kernel_spmd

B, T, C, H = 2, 2048, 1024, 16
HD = C // H   # 64
HPG = 4       # heads per group
NG = 4        # head groups
NCORES = 8
VW = 260      # v_aug width: 4 heads x (64 v + 1 ones)

F32 = mybir.dt.float32
F16 = mybir.dt.float16
F8 = mybir.dt.float8e4
AF = mybir.ActivationFunctionType
DR = mybir.MatmulPerfMode.DoubleRow
SCALE = 1.0 / math.sqrt(C)  # 1/32


def build_program(reps=1, qk_bias=False, v_bias=False):
    nc = bacc.Bacc()

    xT = nc.dram_tensor("xT", [8, 128, T], F16, kind="ExternalInput")
    wqk = nc.dram_tensor("wqk", [128, 8, 512], F16, kind="ExternalInput")
    bqk = nc.dram_tensor("bqk", [128, 4], F32, kind="ExternalInput")
    wv = nc.dram_tensor("wv", [C, VW], F16, kind="ExternalInput")
    bv = nc.dram_tensor("bv", [1, VW], F16, kind="ExternalInput")
    wp = nc.dram_tensor("wp", [128, 2, 1024], F16, kind="ExternalInput")
    mask = nc.dram_tensor("mask", [128, 128], F16, kind="ExternalInput")
    ones = nc.dram_tensor("ones", [1, 128], F16, kind="ExternalInput")
    id16 = nc.dram_tensor("id16", [128, 128], F16, kind="ExternalInput")
    onec = nc.dram_tensor("onec", [128, 1], F16, kind="ExternalInput")
    out = nc.dram_tensor("out", [T, C], F16, kind="ExternalOutput")

    with tile.TileContext(nc) as tc:
        with (
            tc.tile_pool(name="big", bufs=8) as big_pool,
            tc.tile_pool(name="wqk", bufs=1) as wqk_pool,
            tc.tile_pool(name="wv", bufs=8) as wv_pool,
            tc.tile_pool(name="wp", bufs=1) as wp_pool,
            tc.tile_pool(name="qk8", bufs=2) as qk8_pool,
            tc.tile_pool(name="pp", bufs=6) as p_pool,
            tc.tile_pool(name="vsb", bufs=16) as v_pool,
            tc.tile_pool(name="yT", bufs=2) as yT_pool,
            tc.tile_pool(name="yn", bufs=6) as yn_pool,
            tc.tile_pool(name="rr", bufs=6) as r_pool,
            tc.tile_pool(name="osb", bufs=12) as o_pool,
            tc.tile_pool(name="consts", bufs=1) as c_pool,
            tc.tile_pool(name="psA", bufs=2, space="PSUM") as psA,
            tc.tile_pool(name="psV", bufs=2, space="PSUM") as psV,
            tc.tile_pool(name="psB", bufs=2, space="PSUM") as psB,
        ):
          for rep in range(reps):
            # ---- loads. HWDGE issue is a serialized ~630ns/DMA device, so
            # inputs use few big DMAs: wqk in 2 halves, x ch0 per-ct (fine
            # grain feeds the first chains), x ch1-3 as one [128,1536] DMA
            # per ct, wp as one DMA. consts/wv go via gpsimd SWDGE (bypasses
            # HWDGE entirely).
            qdma = [nc.sync, nc.scalar]
            wqk_sb = wqk_pool.tile([128, 8, 512], F16, tag="wqk")
            xtc0, xtcR = [], []
            for qtr in range(4):
                qdma[qtr % 2].dma_start(
                    wqk_sb[:, 2 * qtr:2 * qtr + 2, :],
                    wqk[:, 2 * qtr:2 * qtr + 2, :])
            for ct in range(8):
                t_ = big_pool.tile([128, 512], F16, tag="big0",
                                   name=f"x0_{ct}")
                qdma[ct % 2].dma_start(t_[:], xT[ct, :, 0:512])
                xtc0.append(t_)
            # SWDGE order: wv first (the prologue v0 chain wants it ~3us
            # in; each gpsimd DMA costs ~1us of Pool prep, so consts go
            # after), then mask (A0's first diagonal), onec, id16 (only
            # used by B3's tail transposes).
            wv_sb = []
            for ct in range(8):
                t_ = wv_pool.tile([128, VW], F16, tag="wv", name=f"wv{ct}")
                nc.gpsimd.dma_start(t_[:], wv[ct * 128:(ct + 1) * 128, :])
                wv_sb.append(t_)
            mask_sb = c_pool.tile([128, 128], F16, tag="mask")
            nc.gpsimd.dma_start(mask_sb[:], mask[:])
            onec_sb = c_pool.tile([128, 1], F16, tag="onec")
            nc.gpsimd.dma_start(onec_sb[:], onec[:])
            id_sb = c_pool.tile([128, 128], F16, tag="id16")
            nc.gpsimd.dma_start(id_sb[:], id16[:])
            if v_bias:
                ones_sb = c_pool.tile([1, 128], F16, tag="ones")
                nc.gpsimd.dma_start(ones_sb[:], ones[:])
                bv_sb = c_pool.tile([1, VW], F16, tag="bv")
                nc.gpsimd.dma_start(bv_sb[:], bv[:])
            if qk_bias:
                bqk_sb = c_pool.tile([128, 4], F32, tag="bqk")
                nc.gpsimd.dma_start(bqk_sb[:], bqk[:])
            for ct in range(8):
                t_ = big_pool.tile([128, 1536], F16, tag="bigR",
                                   name=f"xR_{ct}")
                qdma[ct % 2].dma_start(t_[:], xT[ct, :, 512:2048])
                xtcR.append(t_)
            wp_sb = wp_pool.tile([128, 2, 1024], F16, tag="wp")
            nc.sync.dma_start(wp_sb[:], wp[:])

            def xs(ct, ch, c0, c1):
                if ch == 0:
                    return xtc0[ct][:, c0:c1]
                base = (ch - 1) * 512
                return xtcR[ct][:, base + c0:base + c1]

            q8 = qk8_pool.tile([128, 2, T], F8, tag="qk8", name="q8")
            k8 = qk8_pool.tile([128, 2, T], F8, tag="qk8", name="k8")
            yT_sb = [
                yT_pool.tile([128, T], F16, tag="yT", name=f"yT{m}")
                for m in range(2)
            ]
            v_sb = [None] * 16

            # ---- emitters ----------------------------------------------
            def qk_parts(ch, jt):
                """qk projection block as two half-chains (finer slots).
                psum rows = 4 heads x 32 hd-half."""
                cell = []

                def half(h0):
                    if h0 == 0:
                        cell.append(psB.tile([128, 512], F32, tag="mm",
                                             name=f"qk{ch}_{jt}"))
                    ps = cell[0]
                    for ct in range(h0, h0 + 4):
                        nc.tensor.matmul(
                            ps[:, 0:512],
                            wqk_sb[:, ct:ct + 1, jt * 128:(jt + 1) * 128],
                            xs(ct, ch, 0, 512),
                            start=(ct == 0),
                            stop=(ct == 7),
                        )
                    if h0 == 0:
                        return
                    dst_t = q8 if jt < 2 else k8
                    dst = dst_t[:, jt % 2:jt % 2 + 1, ch * 512:(ch + 1) * 512]
                    if qk_bias:
                        nc.vector.tensor_scalar_add(dst, ps[:, 0:512],
                                                    bqk_sb[:, jt:jt + 1])
                    elif ch == 0 and jt % 2 == 1:
                        nc.scalar.copy(dst, ps[:, 0:512])
                    elif False:
                        nc.scalar.copy(dst, ps[:, 0:512])
                    else:
                        nc.vector.tensor_copy(dst, ps[:, 0:512])

                return [lambda: half(0), lambda: half(4)]

            def emit_qk_jt(ch, jt):
                for p in qk_parts(ch, jt):
                    p()

            def v_parts(ch, tt):
                """v_aug projection block as two half-chains."""
                cell = []
                xblk = (tt % 4) * 128

                vwid = VW if v_bias else 256

                def half(h0):
                    if h0 == 0:
                        cell.append(psB.tile([128, 512], F32, tag="mm",
                                             name=f"v{tt}"))
                        if v_bias:
                            nc.tensor.matmul(
                                cell[0][:, 0:VW],
                                ones_sb[0:1, 0:128],
                                bv_sb[0:1, 0:VW],
                                start=True,
                                stop=False,
                            )
                    ps = cell[0]
                    for ct in range(h0, h0 + 4):
                        nc.tensor.matmul(
                            ps[:, 0:vwid],
                            xs(ct, ch, xblk, xblk + 128),
                            wv_sb[ct][:, 0:vwid],
                            start=(not v_bias and ct == 0),
                            stop=(ct == 7),
                            skip_group_check=True,
                        )
                    if h0 == 0:
                        return
                    t_ = v_pool.tile([128, vwid], F16, tag="v", name=f"v{tt}")
                    # tt==0 runs in the prologue where ACT is still idle
                    if tt == 0:
                        nc.scalar.copy(t_[:], ps[:, 0:vwid])
                    else:
                        nc.vector.tensor_copy(t_[:], ps[:, 0:vwid])
                    v_sb[tt] = t_

                return [lambda: half(0), lambda: half(4)]

            def emit_v_tt(ch, tt):
                for p in v_parts(ch, tt):
                    p()

            o15 = [None]

            def emit_proj_nch(tt, nch):
                ps = psB.tile([128, 512], F32, tag="mm", name=f"pj{tt}_{nch}")
                for mt in range(2):
                    nc.tensor.matmul(
                        ps[:, 0:512],
                        yT_sb[mt][:, tt * 128:(tt + 1) * 128],
                        wp_sb[:, mt:mt + 1, nch * 512:(nch + 1) * 512],
                        start=(mt == 0),
                        stop=(mt == 1),
                    )
                if tt == 15:
                    # final tile: one [128,1024] DMA, copies split ACT/DVE in
                    # parallel to shorten the drain chain
                    if nch == 0:
                        o15[0] = o_pool.tile([128, 1024], F16, tag="o15", name="o15")
                        nc.vector.tensor_copy(o15[0][:, 0:512], ps[:, 0:512])
                    else:
                        nc.scalar.copy(o15[0][:, 512:1024], ps[:, 0:512])
                        nc.sync.dma_start(out[15 * 128:, :], o15[0][:])
                    return
                o = o_pool.tile([128, 512], F16, tag="o", name=f"o{tt}_{nch}")
                # tt==14 is emitted after the last exp: ACT is idle there and
                # this skips the DVE tail backlog
                if tt >= 14:
                    nc.scalar.copy(o[:], ps[:, 0:512])
                else:
                    nc.vector.tensor_copy(o[:], ps[:, 0:512])
                nc.sync.dma_start(
                    out[tt * 128:(tt + 1) * 128, nch * 512:(nch + 1) * 512],
                    o[:],
                )

            def pj(tt):
                return [lambda: emit_proj_nch(tt, 0),
                        lambda: emit_proj_nch(tt, 1)]

            # ---- attention chunk ---------------------------------------
            def attn_qc(hp, qc, slots=(), post=()):
                """post[qs]: thunks emitted right after finish_qs(qs) —
                for work that depends on this chunk's own yT writes."""
                slots = list(slots)
                post = {i: list(p) for i, p in enumerate(post)}
                q0 = qc * 512
                njt = 4 * qc + 4
                av = [
                    psV.tile([128, VW], F32, tag="av", name=f"av{hp}{qc}a"),
                    psV.tile([128, VW], F32, tag="av", name=f"av{hp}{qc}b"),
                ]
                started = [False, False]
                pend, trail, tpend = [], 2, []
                n_slots, n_popped = len(slots), 0

                def emit_av(jp, pp):
                    for qs in range(max(0, jp - 4 * qc), 4):
                        ti = qs // 2
                        st = not started[ti]
                        started[ti] = True
                        for hl in range(2):
                            g = 2 * (qs % 2) + hl
                            h = 2 * hp + hl
                            stat = pp[:, hl * 512 + qs * 128:
                                      hl * 512 + qs * 128 + 128]
                            stop = jp == 4 * qc + qs
                            if v_bias:
                                nc.tensor.matmul(
                                    av[ti][:, 65 * g:65 * g + 65],
                                    stat,
                                    v_sb[jp][:, 65 * h:65 * h + 65],
                                    start=(st and hl == 0),
                                    stop=stop,
                                    skip_group_check=True,
                                )
                            else:
                                nc.tensor.matmul(
                                    av[ti][:, 64 * g:64 * g + 64],
                                    stat,
                                    v_sb[jp][:, 64 * h:64 * h + 64],
                                    start=(st and hl == 0),
                                    stop=stop,
                                    skip_group_check=True,
                                )
                                # D accumulates via a 1-col matmul sharing
                                # the stationary p (ldweights skipped)
                                nc.tensor.matmul(
                                    av[ti][:, 256 + g:257 + g],
                                    stat,
                                    onec_sb[:, 0:1],
                                    start=False,
                                    stop=stop,
                                    skip_group_check=True,
                                )

                def finish_qs(qs):
                    """Normalize (recip + scale) now; return a thunk doing
                    the PE transpose + yT copy, deferred so the PE never
                    waits on this DVE chain."""
                    ti, g0 = qs // 2, 2 * (qs % 2)
                    rs = r_pool.tile([128, 2], F32, tag="r",
                                     name=f"r{hp}_{qc}_{qs}")
                    yn = yn_pool.tile([128, 128], F16, tag="yn",
                                      name=f"yn{hp}_{qc}_{qs}")
                    # the very last finishes run after the final exp: use the
                    # idle ACT engine instead of the backlogged DVE
                    on_act = hp == 1 and qc == 3 and qs >= 2
                    for hl in range(2):
                        if v_bias:
                            c0, cd = 65 * (g0 + hl), 65 * (g0 + hl) + 64
                        else:
                            c0, cd = 64 * (g0 + hl), 256 + g0 + hl
                        nc.vector.reciprocal(rs[:, hl:hl + 1],
                                             av[ti][:, cd:cd + 1])
                        if on_act:
                            nc.scalar.activation(
                                yn[:, 64 * hl:64 * hl + 64],
                                av[ti][:, c0:c0 + 64],
                                AF.Copy, scale=rs[:, hl:hl + 1])
                        else:
                            nc.vector.tensor_scalar_mul(
                                yn[:, 64 * hl:64 * hl + 64],
                                av[ti][:, c0:c0 + 64],
                                rs[:, hl:hl + 1],
                            )

                    def transp():
                        dst = yT_sb[hp][:, q0 + qs * 128:q0 + qs * 128 + 128]
                        if hp == 1 and qc == 3:
                            # tail-critical: PE transpose + engine copy is
                            # ~0.35us vs the ~2.5us DMA-transpose latency
                            tp = psB.tile([128, 128], F16, tag="mm",
                                          name=f"tp{hp}_{qc}_{qs}")
                            nc.tensor.transpose(tp[:, 0:128], yn[:, 0:128],
                                                id_sb[:, 0:128])
                            if on_act:
                                nc.scalar.copy(dst, tp[:, 0:128])
                            else:
                                nc.vector.tensor_copy(dst, tp[:, 0:128])
                        else:
                            # off the critical path: xbar DMA transpose frees
                            # the PE, DVE, and the psB rotation entirely
                            nc.sync.dma_start_transpose(dst, yn[:, 0:128])

                    return transp

                for jt in range(njt):
                    L = max(0, jt * 128 - q0)
                    s_ps = psA.tile([128, 1024], F32, tag="A", name="s_ps")
                    for hl in range(2):
                        h = 2 * hp + hl
                        nc.tensor.matmul(
                            s_ps[:, hl * 512 + L:(hl + 1) * 512],
                            k8[32 * h:32 * h + 32, :,
                               jt * 128:(jt + 1) * 128],
                            q8[32 * h:32 * h + 32, :, q0 + L:q0 + 512],
                            start=True,
                            stop=True,
                            perf_mode=DR,
                            tile_position=(32 * h, 0),
                        )
                    p_sb = p_pool.tile([128, 1024], F16, tag="p", name="p_sb")
                    nc.scalar.activation(
                        p_sb[:, L:1024], s_ps[:, L:1024], AF.Exp, scale=SCALE
                    )
                    if jt >= 4 * qc:
                        # DVE (not Pool): the mask sits on the exp->AV
                        # critical path; Pool's Q7 launch + 0.42x multiply
                        # would add ~400ns to every diagonal block.
                        nc.vector.tensor_mul(
                            p_sb[:, L:L + 128], p_sb[:, L:L + 128], mask_sb[:]
                        )
                        nc.vector.tensor_mul(
                            p_sb[:, 512 + L:512 + L + 128],
                            p_sb[:, 512 + L:512 + L + 128],
                            mask_sb[:],
                        )
                    pend.append((jt, p_sb))
                    if len(pend) > trail:
                        jp, pp = pend.pop(0)
                        emit_av(jp, pp)
                        if jp >= 4 * qc:
                            qs_done = jp - 4 * qc
                            tpend.append((qs_done, finish_qs(qs_done)))
                    # slots (PE filler) run between the normalize (DVE) and
                    # the transpose that consumes it, hiding that latency
                    while slots and n_popped < (jt + 1) * n_slots / njt:
                        slots.pop(0)()
                        n_popped += 1
                    if len(tpend) > 0:
                        qs_done, th = tpend.pop(0)
                        th()
                        for s in post.pop(qs_done, ()):
                            s()
                for jp, pp in pend:
                    emit_av(jp, pp)
                    if jp >= 4 * qc:
                        tpend.append((jp - 4 * qc, finish_qs(jp - 4 * qc)))
                for qs_done, th in tpend:
                    th()
                    for s in post.pop(qs_done, ()):
                        s()
                for s in slots:
                    s()
                for qs_done in sorted(post):
                    for s in post[qs_done]:
                        s()

            # ---- schedule ----------------------------------------------
            def qk_u(ch, jt):
                return qk_parts(ch, jt)

            def v_u(ch, tt):
                return v_parts(ch, tt)

            # prologue: qk blocks of ch0 with v0's halves woven between
            # (v0 is paced by the SWDGE wv stream, qk by the x0 DMAs)
            v0a, v0b = v_parts(0, 0)
            emit_qk_jt(0, 0)
            emit_qk_jt(0, 1)
            emit_qk_jt(0, 2)
            emit_qk_jt(0, 3)
            v0a()
            v0b()
            attn_qc(0, 0, v_u(0, 1) + v_u(0, 2) + v_u(0, 3))
            # B0 carries the ch1 qk blocks (paced by the x ch1-3 DMAs)
            attn_qc(1, 0, qk_u(1, 0) + qk_u(1, 1) + qk_u(1, 2) + qk_u(1, 3))
            attn_qc(0, 1, v_u(1, 4) + v_u(1, 5) + v_u(1, 6) + v_u(1, 7))
            attn_qc(1, 1, qk_u(2, 0) + qk_u(2, 1) + qk_u(2, 2) + qk_u(2, 3))
            attn_qc(0, 2, v_u(2, 8) + v_u(2, 9) + v_u(2, 10) + v_u(2, 11)
                    + pj(0) + pj(1))
            attn_qc(1, 2, qk_u(3, 0) + qk_u(3, 1) + qk_u(3, 2) + qk_u(3, 3)
                    + pj(2) + pj(3))
            attn_qc(0, 3, v_u(3, 12) + v_u(3, 13) + v_u(3, 14) + v_u(3, 15)
                    + pj(4) + pj(5))
            # pj(12..15) read yT columns B3 itself writes: emit each right
            # after B3's finish_qs for that query sub-block.
            attn_qc(1, 3, pj(6) + pj(7) + pj(8) + pj(9) + pj(10) + pj(11),
                    post=[pj(12), pj(13), pj(14), pj(15)])

    if not nc.is_finalized():
        nc.finalize()
    return nc


def host_prep(x, W_attn, b_attn, W_proj):
    v_bias = bool(np.any(np.asarray(b_attn, np.float32)[2 * C:]))
    bf = np.float16
    x = np.ascontiguousarray(np.asarray(x, np.float32))
    W_attn = np.ascontiguousarray(np.asarray(W_attn, np.float32))
    b_attn = np.ascontiguousarray(np.asarray(b_attn, np.float32))
    W_proj = np.ascontiguousarray(np.asarray(W_proj, np.float32))
    mask = np.triu(np.ones((128, 128), np.float32)).astype(bf)
    ones = np.ones((1, 128), bf)
    id16 = np.eye(128, dtype=bf)
    onec_ = np.ones((128, 1), bf)
    per_group = []
    for hg in range(NG):
        heads = [hg * HPG + i for i in range(HPG)]
        # wqk column blocks jt: 0=q hd-lo, 1=q hd-hi, 2=k hd-lo, 3=k hd-hi;
        # within a block, 4 heads x 32 (head-major)
        cols, bias = [], []
        for base in (0, C):  # q then k
            for half in (0, 32):
                for h in heads:
                    c0 = base + h * HD + half
                    cols.append(W_attn[:, c0:c0 + 32])
                    bias.append(b_attn[c0:c0 + 32])
        # [C, 512] -> [128, 8, 512]: partition p, ct-slab, col
        wqk_ = np.ascontiguousarray(
            np.concatenate(cols, axis=1).astype(bf)
            .reshape(8, 128, 512).transpose(1, 0, 2))
        bqk_ = np.ascontiguousarray(
            np.concatenate(bias).reshape(4, 128).T.astype(np.float32))
        wv_ = np.zeros((C, VW), np.float32)
        bv_ = np.zeros((1, VW), np.float32)
        vb = 65 if v_bias else 64
        for i, h in enumerate(heads):
            wv_[:, vb * i:vb * i + 64] = \
                W_attn[:, 2 * C + h * HD:2 * C + (h + 1) * HD]
            if v_bias:
                bv_[0, 65 * i:65 * i + 64] = \
                    b_attn[2 * C + h * HD:2 * C + (h + 1) * HD]
                bv_[0, 65 * i + 64] = 1.0
        # [256, 1024] -> [128, 2, 1024]
        wp_ = np.ascontiguousarray(
            np.concatenate([W_proj[h * HD:(h + 1) * HD, :] for h in heads],
                           axis=0).astype(bf)
            .reshape(2, 128, 1024).transpose(1, 0, 2))
        per_group.append((wqk_, bqk_, wv_.astype(bf), bv_.astype(bf), wp_))
    in_maps = []
    for b in range(B):
        xT_b = np.ascontiguousarray(
            x[b].T.astype(bf).reshape(8, 128, T))
        for hg in range(NG):
            wqk_, bqk_, wv_, bv_, wp_ = per_group[hg]
            in_maps.append(
                dict(xT=xT_b, wqk=wqk_, bqk=bqk_, wv=wv_, bv=bv_, wp=wp_,
                     mask=mask, ones=ones, id16=id16, onec=onec_)
            )
    return in_maps


_prog_cache = {}


def _get_program(qk_bias=False, v_bias=False):
    key = ("nc", qk_bias, v_bias)
    if key not in _prog_cache:
        _prog_cache[key] = build_program(qk_bias=qk_bias, v_bias=v_bias)
    return _prog_cache[key]


def run_cores(in_maps, trace=False, qk_bias=False, v_bias=False, **kw):
    return run_bass_kernel_spmd(
        _get_program(qk_bias, v_bias), in_maps, list(range(NCORES)),
        trace=trace, **kw
    )


def kernel(x, W_attn, b_attn, W_proj, b_proj):
    in_maps = host_prep(x, W_attn, b_attn, W_proj)
    b_attn_f = np.asarray(b_attn, np.float32)
    qk_bias = bool(np.any(b_attn_f[: 2 * C]))
    v_bias = bool(np.any(b_attn_f[2 * C:]))
    br = run_cores(in_maps, qk_bias=qk_bias, v_bias=v_bias)
    b_proj = np.asarray(b_proj, np.float32)
    y = np.zeros((B, T, C), np.float32)
    for b in range(B):
        acc = np.zeros((T, C), np.float32)
        for hg in range(NG):
            acc += np.asarray(br.results[b * NG + hg]["out"])
        y[b] = acc + b_proj[None, :]
    return y


# revision 83
# speedup vs baseline: 1.6378x; 1.0114x over previous
"""Causal self-attention (B=2, T=2048, C=1024, H=16) on 8 TRN2 NeuronCores.

Sharding: core = b*4 + hg (data parallel over batch, tensor parallel over
4 head-groups of 4 heads). Each core computes its head-group's attention and
a partial output projection; the host sums the 4 partials per batch and adds
b_proj.

Per-core device program (v4 — fp8 DoubleRow scores + moving-v AV;
TimelineSim 112.1us vs the 183.6us v3 baseline):
  - qk projection writes q8/k8 as fp8e4 [128, 2, T] tiles: partitions
    32h..32h+31 hold head h, slab i = head-dim half i. wqk's column order is
    permuted on the host so each [128,512] PSUM block lands with ONE copy.
  - scores use fp8 DoubleRow matmuls (0.5 cyc/row in the cost model): per
    (head, key-block) one matmul, stationary k8 [32,2,128], moving q8
    [32,2,512-L]. Quantization error ~1% total vs the 2e-2 gate.
  - AV is restructured: stationary p [128 keys, 128 q], moving v [128, 64]
    per (key-block, q-subblock, head) — 64 moving cols beat the old
    512-wide moving-p form ~2x. A 1-col matmul vs a ones vector (same
    stationary, ldweights-free) accumulates the softmax denominator D per
    query ON the query partition, so normalization is a per-partition
    tensor_scalar (no PE broadcast matmuls). With v_bias, v blocks carry a
    65th ones column initialized by a bias matmul instead.
  - y [q, hd] is normalized via reciprocal+tensor_scalar then transposed
    into yT for the projection (wp/yT fp16). 28 of 32 transposes go through
    the xbar DMA-transpose (frees PE/DVE and, critically, the psB rotation);
    only B3's four tail-critical ones use the PE+identity path.
  - ACT runs exps (the pacing engine, ~80us) plus prologue/tail copies;
    everything else copies on DVE. Masks multiply on DVE (Pool's Q7
    launch would sit on the exp->AV critical path). HWDGE issue is a
    serialized ~630ns/DMA device, so inputs load as few big DMAs with a
    host-side relayout; consts/wv go via gpsimd SWDGE.
  - PSUM: scores 2x[128,1024] (4 banks) + AV pool 2x[128,260] (4 groups of
    64 + 4 D cols each, pending-zero init, 1 bank each) + shared [128,512]
    pool for qkv/proj/transpose (2 banks) = 8 banks.
  - Schedule: attention chunk (hp, qc) consumes qkv column-block ch=qc;
    qkv half-chains and projection tiles are paced into the ACT-bound jt
    loops as ~0.5-0.9us slots (popped between the scores and the trailing
    AV, which runs 2 key-blocks behind its exp); per-qs finish chains
    (recip/normalize -> transpose -> yT) emit at each diagonal, and the
    last 4 projection tiles hang off B3's own finishes via post-hooks.
"""

import math

import numpy as np

import concourse.bass as bass
import concourse.bacc as bacc
import concourse.mybir as mybir
from concourse import tile
from concourse.bass_utils import run_bass_kernel_spmd

B, T, C, H = 2, 2048, 1024, 16
HD = C // H   # 64
HPG = 4       # heads per group
NG = 4        # head groups
NCORES = 8
VW = 260      # v_aug width: 4 heads x (64 v + 1 ones)

F32 = mybir.dt.float32
F16 = mybir.dt.float16
F8 = mybir.dt.float8e4
AF = mybir.ActivationFunctionType
DR = mybir.MatmulPerfMode.DoubleRow
SCALE = 1.0 / math.sqrt(C)  # 1/32


def build_program(reps=1, qk_bias=False, v_bias=False):
    nc = bacc.Bacc()

    xT = nc.dram_tensor("xT", [8, 128, T], F16, kind="ExternalInput")
    wqk = nc.dram_tensor("wqk", [128, 8, 512], F16, kind="ExternalInput")
    bqk = nc.dram_tensor("bqk", [128, 4], F32, kind="ExternalInput")
    wv = nc.dram_tensor("wv", [C, VW], F16, kind="ExternalInput")
    bv = nc.dram_tensor("bv", [1, VW], F16, kind="ExternalInput")
    wp = nc.dram_tensor("wp", [128, 2, 1024], F16, kind="ExternalInput")
    mask = nc.dram_tensor("mask", [128, 128], F16, kind="ExternalInput")
    ones = nc.dram_tensor("ones", [1, 128], F16, kind="ExternalInput")
    id16 = nc.dram_tensor("id16", [128, 128], F16, kind="ExternalInput")
    onec = nc.dram_tensor("onec", [128, 1], F16, kind="ExternalInput")
    out = nc.dram_tensor("out", [T, C], F16, kind="ExternalOutput")

    with tile.TileContext(nc) as tc:
        with (
            tc.tile_pool(name="big", bufs=8) as big_pool,
            tc.tile_pool(name="wqk", bufs=1) as wqk_pool,
            tc.tile_pool(name="wv", bufs=8) as wv_pool,
            tc.tile_pool(name="wp", bufs=1) as wp_pool,
            tc.tile_pool(name="qk8", bufs=2) as qk8_pool,
            tc.tile_pool(name="pp", bufs=6) as p_pool,
            tc.tile_pool(name="vsb", bufs=16) as v_pool,
            tc.tile_pool(name="yT", bufs=2) as yT_pool,
            tc.tile_pool(name="yn", bufs=6) as yn_pool,
            tc.tile_pool(name="rr", bufs=6) as r_pool,
            tc.tile_pool(name="osb", bufs=12) as o_pool,
            tc.tile_pool(name="consts", bufs=1) as c_pool,
            tc.tile_pool(name="psA", bufs=2, space="PSUM") as psA,
            tc.tile_pool(name="psV", bufs=2, space="PSUM") as psV,
            tc.tile_pool(name="psB", bufs=2, space="PSUM") as psB,
        ):
          for rep in range(reps):
            # ---- loads. HWDGE issue is a serialized ~630ns/DMA device, so
            # inputs use few big DMAs: wqk in 2 halves, x ch0 per-ct (fine
            # grain feeds the first chains), x ch1-3 as one [128,1536] DMA
            # per ct, wp as one DMA. consts/wv go via gpsimd SWDGE (bypasses
            # HWDGE entirely).
            qdma = [nc.sync, nc.scalar]
            wqk_sb = wqk_pool.tile([128, 8, 512], F16, tag="wqk")
            xtc0, xtcR = [], []
            for qtr in range(2):
                qdma[qtr % 2].dma_start(
                    wqk_sb[:, 2 * qtr:2 * qtr + 2, :],
                    wqk[:, 2 * qtr:2 * qtr + 2, :])
            for ct in range(8):
                t_ = big_pool.tile([128, 512], F16, tag="big0",
                                   name=f"x0_{ct}")
                qdma[ct % 2].dma_start(t_[:], xT[ct, :, 0:512])
                xtc0.append(t_)
            # quarters 2-3 (cts 4-7) aren't needed until ~7us: issue after
            # the x ch0 stream so x00 clears DMA_ENGINES sooner
            for qtr in range(2, 4):
                qdma[qtr % 2].dma_start(
                    wqk_sb[:, 2 * qtr:2 * qtr + 2, :],
                    wqk[:, 2 * qtr:2 * qtr + 2, :])
            # SWDGE order: wv first (the prologue v0 chain wants it ~3us
            # in; each gpsimd DMA costs ~1us of Pool prep, so consts go
            # after), then mask (A0's first diagonal), onec, id16 (only
            # used by B3's tail transposes).
            wv_sb = []
            for ct in range(8):
                t_ = wv_pool.tile([128, VW], F16, tag="wv", name=f"wv{ct}")
                nc.gpsimd.dma_start(t_[:], wv[ct * 128:(ct + 1) * 128, :])
                wv_sb.append(t_)
            mask_sb = c_pool.tile([128, 128], F16, tag="mask")
            nc.gpsimd.dma_start(mask_sb[:], mask[:])
            onec_sb = c_pool.tile([128, 1], F16, tag="onec")
            nc.gpsimd.dma_start(onec_sb[:], onec[:])
            id_sb = c_pool.tile([128, 128], F16, tag="id16")
            nc.gpsimd.dma_start(id_sb[:], id16[:])
            if v_bias:
                ones_sb = c_pool.tile([1, 128], F16, tag="ones")
                nc.gpsimd.dma_start(ones_sb[:], ones[:])
                bv_sb = c_pool.tile([1, VW], F16, tag="bv")
                nc.gpsimd.dma_start(bv_sb[:], bv[:])
            if qk_bias:
                bqk_sb = c_pool.tile([128, 4], F32, tag="bqk")
                nc.gpsimd.dma_start(bqk_sb[:], bqk[:])
            for ct in range(8):
                t_ = big_pool.tile([128, 1536], F16, tag="bigR",
                                   name=f"xR_{ct}")
                qdma[ct % 2].dma_start(t_[:], xT[ct, :, 512:2048])
                xtcR.append(t_)
            wp_sb = wp_pool.tile([128, 2, 1024], F16, tag="wp")
            nc.sync.dma_start(wp_sb[:], wp[:])

            def xs(ct, ch, c0, c1):
                if ch == 0:
                    return xtc0[ct][:, c0:c1]
                base = (ch - 1) * 512
                return xtcR[ct][:, base + c0:base + c1]

            q8 = qk8_pool.tile([128, 2, T], F8, tag="qk8", name="q8")
            k8 = qk8_pool.tile([128, 2, T], F8, tag="qk8", name="k8")
            yT_sb = [
                yT_pool.tile([128, T], F16, tag="yT", name=f"yT{m}")
                for m in range(2)
            ]
            v_sb = [None] * 16

            # ---- emitters ----------------------------------------------
            def qk_parts(ch, jt):
                """qk projection block as two half-chains (finer slots).
                psum rows = 4 heads x 32 hd-half."""
                cell = []

                def half(h0):
                    if h0 == 0:
                        cell.append(psB.tile([128, 512], F32, tag="mm",
                                             name=f"qk{ch}_{jt}"))
                    ps = cell[0]
                    for ct in range(h0, h0 + 4):
                        nc.tensor.matmul(
                            ps[:, 0:512],
                            wqk_sb[:, ct:ct + 1, jt * 128:(jt + 1) * 128],
                            xs(ct, ch, 0, 512),
                            start=(ct == 0),
                            stop=(ct == 7),
                        )
                    if h0 == 0:
                        return
                    dst_t = q8 if jt < 2 else k8
                    dst = dst_t[:, jt % 2:jt % 2 + 1, ch * 512:(ch + 1) * 512]
                    if qk_bias:
                        nc.vector.tensor_scalar_add(dst, ps[:, 0:512],
                                                    bqk_sb[:, jt:jt + 1])
                    elif ch == 0 and jt % 2 == 1:
                        nc.scalar.copy(dst, ps[:, 0:512])
                    elif False:
                        nc.scalar.copy(dst, ps[:, 0:512])
                    else:
                        nc.vector.tensor_copy(dst, ps[:, 0:512])

                return [lambda: half(0), lambda: half(4)]

            def emit_qk_jt(ch, jt):
                for p in qk_parts(ch, jt):
                    p()

            def v_parts(ch, tt):
                """v_aug projection block as two half-chains."""
                cell = []
                xblk = (tt % 4) * 128

                vwid = VW if v_bias else 256

                def half(h0):
                    if h0 == 0:
                        cell.append(psB.tile([128, 512], F32, tag="mm",
                                             name=f"v{tt}"))
                        if v_bias:
                            nc.tensor.matmul(
                                cell[0][:, 0:VW],
                                ones_sb[0:1, 0:128],
                                bv_sb[0:1, 0:VW],
                                start=True,
                                stop=False,
                            )
                    ps = cell[0]
                    for ct in range(h0, h0 + 4):
                        nc.tensor.matmul(
                            ps[:, 0:vwid],
                            xs(ct, ch, xblk, xblk + 128),
                            wv_sb[ct][:, 0:vwid],
                            start=(not v_bias and ct == 0),
                            stop=(ct == 7),
                            skip_group_check=True,
                        )
                    if h0 == 0:
                        return
                    t_ = v_pool.tile([128, vwid], F16, tag="v", name=f"v{tt}")
                    # tt==0 runs in the prologue where ACT is still idle
                    if tt == 0:
                        nc.scalar.copy(t_[:], ps[:, 0:vwid])
                    else:
                        nc.vector.tensor_copy(t_[:], ps[:, 0:vwid])
                    v_sb[tt] = t_

                return [lambda: half(0), lambda: half(4)]

            def emit_v_tt(ch, tt):
                for p in v_parts(ch, tt):
                    p()

            o15 = [None]

            def emit_proj_nch(tt, nch):
                ps = psB.tile([128, 512], F32, tag="mm", name=f"pj{tt}_{nch}")
                for mt in range(2):
                    nc.tensor.matmul(
                        ps[:, 0:512],
                        yT_sb[mt][:, tt * 128:(tt + 1) * 128],
                        wp_sb[:, mt:mt + 1, nch * 512:(nch + 1) * 512],
                        start=(mt == 0),
                        stop=(mt == 1),
                    )
                if tt == 15:
                    # final tile: one [128,1024] DMA, copies split ACT/DVE in
                    # parallel to shorten the drain chain
                    if nch == 0:
                        o15[0] = o_pool.tile([128, 1024], F16, tag="o15", name="o15")
                        nc.vector.tensor_copy(o15[0][:, 0:512], ps[:, 0:512])
                    else:
                        nc.scalar.copy(o15[0][:, 512:1024], ps[:, 0:512])
                        nc.sync.dma_start(out[15 * 128:, :], o15[0][:])
                    return
                o = o_pool.tile([128, 512], F16, tag="o", name=f"o{tt}_{nch}")
                # tt==14 is emitted after the last exp: ACT is idle there and
                # this skips the DVE tail backlog
                if tt >= 14:
                    nc.scalar.copy(o[:], ps[:, 0:512])
                else:
                    nc.vector.tensor_copy(o[:], ps[:, 0:512])
                nc.sync.dma_start(
                    out[tt * 128:(tt + 1) * 128, nch * 512:(nch + 1) * 512],
                    o[:],
                )

            def pj(tt):
                return [lambda: emit_proj_nch(tt, 0),
                        lambda: emit_proj_nch(tt, 1)]

            # ---- attention chunk ---------------------------------------
            def attn_qc(hp, qc, slots=(), post=()):
                """post[qs]: thunks emitted right after finish_qs(qs) —
                for work that depends on this chunk's own yT writes."""
                slots = list(slots)
                post = {i: list(p) for i, p in enumerate(post)}
                q0 = qc * 512
                njt = 4 * qc + 4
                av = [
                    psV.tile([128, VW], F32, tag="av", name=f"av{hp}{qc}a"),
                    psV.tile([128, VW], F32, tag="av", name=f"av{hp}{qc}b"),
                ]
                started = [False, False]
                pend, trail, tpend = [], 2, []
                n_slots, n_popped = len(slots), 0

                def emit_av(jp, pp):
                    for qs in range(max(0, jp - 4 * qc), 4):
                        ti = qs // 2
                        st = not started[ti]
                        started[ti] = True
                        for hl in range(2):
                            g = 2 * (qs % 2) + hl
                            h = 2 * hp + hl
                            stat = pp[:, hl * 512 + qs * 128:
                                      hl * 512 + qs * 128 + 128]
                            stop = jp == 4 * qc + qs
                            if v_bias:
                                nc.tensor.matmul(
                                    av[ti][:, 65 * g:65 * g + 65],
                                    stat,
                                    v_sb[jp][:, 65 * h:65 * h + 65],
                                    start=(st and hl == 0),
                                    stop=stop,
                                    skip_group_check=True,
                                )
                            else:
                                nc.tensor.matmul(
                                    av[ti][:, 64 * g:64 * g + 64],
                                    stat,
                                    v_sb[jp][:, 64 * h:64 * h + 64],
                                    start=(st and hl == 0),
                                    stop=stop,
                                    skip_group_check=True,
                                )
                                # D accumulates via a 1-col matmul sharing
                                # the stationary p (ldweights skipped)
                                nc.tensor.matmul(
                                    av[ti][:, 256 + g:257 + g],
                                    stat,
                                    onec_sb[:, 0:1],
                                    start=False,
                                    stop=stop,
                                    skip_group_check=True,
                                )

                def finish_qs(qs):
                    """Normalize (recip + scale) now; return a thunk doing
                    the PE transpose + yT copy, deferred so the PE never
                    waits on this DVE chain."""
                    ti, g0 = qs // 2, 2 * (qs % 2)
                    rs = r_pool.tile([128, 2], F32, tag="r",
                                     name=f"r{hp}_{qc}_{qs}")
                    yn = yn_pool.tile([128, 128], F16, tag="yn",
                                      name=f"yn{hp}_{qc}_{qs}")
                    # the very last finishes run after the final exp: use the
                    # idle ACT engine instead of the backlogged DVE
                    on_act = hp == 1 and qc == 3 and qs >= 2
                    for hl in range(2):
                        if v_bias:
                            c0, cd = 65 * (g0 + hl), 65 * (g0 + hl) + 64
                        else:
                            c0, cd = 64 * (g0 + hl), 256 + g0 + hl
                        nc.vector.reciprocal(rs[:, hl:hl + 1],
                                             av[ti][:, cd:cd + 1])
                        if on_act:
                            nc.scalar.activation(
                                yn[:, 64 * hl:64 * hl + 64],
                                av[ti][:, c0:c0 + 64],
                                AF.Copy, scale=rs[:, hl:hl + 1])
                        else:
                            nc.vector.tensor_scalar_mul(
                                yn[:, 64 * hl:64 * hl + 64],
                                av[ti][:, c0:c0 + 64],
                                rs[:, hl:hl + 1],
                            )

                    def transp():
                        dst = yT_sb[hp][:, q0 + qs * 128:q0 + qs * 128 + 128]
                        if hp == 1 and qc == 3:
                            # tail-critical: PE transpose + engine copy is
                            # ~0.35us vs the ~2.5us DMA-transpose latency
                            tp = psB.tile([128, 128], F16, tag="mm",
                                          name=f"tp{hp}_{qc}_{qs}")
                            nc.tensor.transpose(tp[:, 0:128], yn[:, 0:128],
                                                id_sb[:, 0:128])
                            if on_act:
                                nc.scalar.copy(dst, tp[:, 0:128])
                            else:
                                nc.vector.tensor_copy(dst, tp[:, 0:128])
                        else:
                            # off the critical path: xbar DMA transpose frees
                            # the PE, DVE, and the psB rotation entirely
                            nc.sync.dma_start_transpose(dst, yn[:, 0:128])

                    return transp

                for jt in range(njt):
                    L = max(0, jt * 128 - q0)
                    s_ps = psA.tile([128, 1024], F32, tag="A", name="s_ps")
                    for hl in range(2):
                        h = 2 * hp + hl
                        nc.tensor.matmul(
                            s_ps[:, hl * 512 + L:(hl + 1) * 512],
                            k8[32 * h:32 * h + 32, :,
                               jt * 128:(jt + 1) * 128],
                            q8[32 * h:32 * h + 32, :, q0 + L:q0 + 512],
                            start=True,
                            stop=True,
                            perf_mode=DR,
                            tile_position=(32 * h, 0),
                        )
                    p_sb = p_pool.tile([128, 1024], F16, tag="p", name="p_sb")
                    nc.scalar.activation(
                        p_sb[:, L:1024], s_ps[:, L:1024], AF.Exp, scale=SCALE
                    )
                    if jt >= 4 * qc:
                        # DVE (not Pool): the mask sits on the exp->AV
                        # critical path; Pool's Q7 launch + 0.42x multiply
                        # would add ~400ns to every diagonal block.
                        nc.vector.tensor_mul(
                            p_sb[:, L:L + 128], p_sb[:, L:L + 128], mask_sb[:]
                        )
                        nc.vector.tensor_mul(
                            p_sb[:, 512 + L:512 + L + 128],
                            p_sb[:, 512 + L:512 + L + 128],
                            mask_sb[:],
                        )
                    pend.append((jt, p_sb))
                    if len(pend) > trail:
                        jp, pp = pend.pop(0)
                        emit_av(jp, pp)
                        if jp >= 4 * qc:
                            qs_done = jp - 4 * qc
                            tpend.append((qs_done, finish_qs(qs_done)))
                    # slots (PE filler) run between the normalize (DVE) and
                    # the transpose that consumes it, hiding that latency
                    while slots and n_popped < (jt + 1) * n_slots / njt:
                        slots.pop(0)()
                        n_popped += 1
                    if len(tpend) > 0:
                        qs_done, th = tpend.pop(0)
                        th()
                        for s in post.pop(qs_done, ()):
                            s()
                for jp, pp in pend:
                    emit_av(jp, pp)
                    if jp >= 4 * qc:
                        tpend.append((jp - 4 * qc, finish_qs(jp - 4 * qc)))
                for qs_done, th in tpend:
                    th()
                    for s in post.pop(qs_done, ()):
                        s()
                for s in slots:
                    s()
                for qs_done in sorted(post):
                    for s in post[qs_done]:
                        s()

            # ---- schedule ----------------------------------------------
            def qk_u(ch, jt):
                return qk_parts(ch, jt)

            def v_u(ch, tt):
                return v_parts(ch, tt)

            # prologue: qk blocks of ch0 with v0's halves woven between
            # (v0 is paced by the SWDGE wv stream, qk by the x0 DMAs)
            v0a, v0b = v_parts(0, 0)
            emit_qk_jt(0, 0)
            emit_qk_jt(0, 1)
            emit_qk_jt(0, 2)
            emit_qk_jt(0, 3)
            v0a()
            v0b()
            attn_qc(0, 0, v_u(0, 1) + v_u(0, 2) + v_u(0, 3))
            # B0 carries the ch1 qk blocks (paced by the x ch1-3 DMAs)
            attn_qc(1, 0, qk_u(1, 0) + qk_u(1, 1) + qk_u(1, 2) + qk_u(1, 3))
            attn_qc(0, 1, v_u(1, 4) + v_u(1, 5) + v_u(1, 6) + v_u(1, 7))
            attn_qc(1, 1, qk_u(2, 0) + qk_u(2, 1) + qk_u(2, 2) + qk_u(2, 3))
            attn_qc(0, 2, v_u(2, 8) + v_u(2, 9) + v_u(2, 10) + v_u(2, 11)
                    + pj(0) + pj(1))
            attn_qc(1, 2, qk_u(3, 0) + qk_u(3, 1) + qk_u(3, 2) + qk_u(3, 3)
                    + pj(2) + pj(3))
            attn_qc(0, 3, v_u(3, 12) + v_u(3, 13) + v_u(3, 14) + v_u(3, 15)
                    + pj(4) + pj(5))
            # pj(12..15) read yT columns B3 itself writes: emit each right
            # after B3's finish_qs for that query sub-block.
            attn_qc(1, 3, pj(6) + pj(7) + pj(8) + pj(9) + pj(10) + pj(11),
                    post=[pj(12), pj(13), pj(14), pj(15)])

    if not nc.is_finalized():
        nc.finalize()
    return nc


def host_prep(x, W_attn, b_attn, W_proj):
    v_bias = bool(np.any(np.asarray(b_attn, np.float32)[2 * C:]))
    bf = np.float16
    x = np.ascontiguousarray(np.asarray(x, np.float32))
    W_attn = np.ascontiguousarray(np.asarray(W_attn, np.float32))
    b_attn = np.ascontiguousarray(np.asarray(b_attn, np.float32))
    W_proj = np.ascontiguousarray(np.asarray(W_proj, np.float32))
    mask = np.triu(np.ones((128, 128), np.float32)).astype(bf)
    ones = np.ones((1, 128), bf)
    id16 = np.eye(128, dtype=bf)
    onec_ = np.ones((128, 1), bf)
    per_group = []
    for hg in range(NG):
        heads = [hg * HPG + i for i in range(HPG)]
        # wqk column blocks jt: 0=q hd-lo, 1=q hd-hi, 2=k hd-lo, 3=k hd-hi;
        # within a block, 4 heads x 32 (head-major)
        cols, bias = [], []
        for base in (0, C):  # q then k
            for half in (0, 32):
                for h in heads:
                    c0 = base + h * HD + half
                    cols.append(W_attn[:, c0:c0 + 32])
                    bias.append(b_attn[c0:c0 + 32])
        # [C, 512] -> [128, 8, 512]: partition p, ct-slab, col
        wqk_ = np.ascontiguousarray(
            np.concatenate(cols, axis=1).astype(bf)
            .reshape(8, 128, 512).transpose(1, 0, 2))
        bqk_ = np.ascontiguousarray(
            np.concatenate(bias).reshape(4, 128).T.astype(np.float32))
        wv_ = np.zeros((C, VW), np.float32)
        bv_ = np.zeros((1, VW), np.float32)
        vb = 65 if v_bias else 64
        for i, h in enumerate(heads):
            wv_[:, vb * i:vb * i + 64] = \
                W_attn[:, 2 * C + h * HD:2 * C + (h + 1) * HD]
            if v_bias:
                bv_[0, 65 * i:65 * i + 64] = \
                    b_attn[2 * C + h * HD:2 * C + (h + 1) * HD]
                bv_[0, 65 * i + 64] = 1.0
        # [256, 1024] -> [128, 2, 1024]
        wp_ = np.ascontiguousarray(
            np.concatenate([W_proj[h * HD:(h + 1) * HD, :] for h in heads],
                           axis=0).astype(bf)
            .reshape(2, 128, 1024).transpose(1, 0, 2))
        per_group.append((wqk_, bqk_, wv_.astype(bf), bv_.astype(bf), wp_))
    in_maps = []
    for b in range(B):
        xT_b = np.ascontiguousarray(
            x[b].T.astype(bf).reshape(8, 128, T))
        for hg in range(NG):
            wqk_, bqk_, wv_, bv_, wp_ = per_group[hg]
            in_maps.append(
                dict(xT=xT_b, wqk=wqk_, bqk=bqk_, wv=wv_, bv=bv_, wp=wp_,
                     mask=mask, ones=ones, id16=id16, onec=onec_)
            )
    return in_maps


_prog_cache = {}


def _get_program(qk_bias=False, v_bias=False):
    key = ("nc", qk_bias, v_bias)
    if key not in _prog_cache:
        _prog_cache[key] = build_program(qk_bias=qk_bias, v_bias=v_bias)
    return _prog_cache[key]


def run_cores(in_maps, trace=False, qk_bias=False, v_bias=False, **kw):
    return run_bass_kernel_spmd(
        _get_program(qk_bias, v_bias), in_maps, list(range(NCORES)),
        trace=trace, **kw
    )


def kernel(x, W_attn, b_attn, W_proj, b_proj):
    in_maps = host_prep(x, W_attn, b_attn, W_proj)
    b_attn_f = np.asarray(b_attn, np.float32)
    qk_bias = bool(np.any(b_attn_f[: 2 * C]))
    v_bias = bool(np.any(b_attn_f[2 * C:]))
    br = run_cores(in_maps, qk_bias=qk_bias, v_bias=v_bias)
    b_proj = np.asarray(b_proj, np.float32)
    y = np.zeros((B, T, C), np.float32)
    for b in range(B):
        acc = np.zeros((T, C), np.float32)
        for hg in range(NG):
            acc += np.asarray(br.results[b * NG + hg]["out"])
        y[b] = acc + b_proj[None, :]
    return y
